# revision 16
# baseline (speedup 1.0000x reference)
"""EdgeDecoder Trainium2 kernel: out = 5*sigmoid(w2 . relu([z_u[row]; z_m[col]] @ W1.T + b1) + b2).

v3 strategy (8 NeuronCores, data-parallel over edges):
  No precomputed node tables. Per edge, gather the raw bf16 z_user[row] and
  z_movie[col] rows straight from HBM with batched dma_gather(transpose=True)
  (one SWDGE instruction per 2048 edges instead of one indirect DMA per 128
  edges), which lands z-components on partitions. The gathered tiles feed the
  PE as the *stationary* operand so edges land on PSUM partitions: per
  512-edge group (4 tiles of 128 edges sharing one PSUM bank), ACT preloads
  b1*w2 into the bank, 8 matmuls (4 tiles x {W1u', W1m'}) accumulate on top
  (W1 columns pre-scaled by |w2| with positive-w2 hidden units permuted
  first), one ACT relu drains the bank to SBUF, and DVE does two free-dim
  tensor_reduces (pos / neg ranges) + subtract -> per-edge logits in an SBUF
  block. Every 512 tile-columns: ACT sigmoid(+b2)*5 and one DMA out.

  dma_gather indices are int16, so node tables are split into <=32768-row
  banks and edges are bucketed by (user-bank, movie-bank) on the host; bucket
  capacities are padded to the max across cores so all 8 cores share one
  compiled program.

v1 (fallback): precomputed A/B tables + per-128-edge indirect DMA gathers.
"""
import sys
import numpy as np

sys.path.insert(0, '/opt/trn_rl_repo')

import concourse.bass as bass
import concourse.bacc as bacc
import concourse.mybir as mybir
import concourse.tile as tile
from concourse import library_config
from concourse.bass_utils import run_bass_kernel_spmd

N_CORES = 8
P = 128
H = 128          # hidden
BANK = 32768     # rows per gather bank (int16 index limit)
CH = 8192        # edges per dma_gather call
TILE = 512       # edges per PE tile (psum bank = 512 f32)
G = 32           # v1: gather-loop cols per iteration
ZBODY = 1024     # v1: precompute rows per loop body

_LAST_STATS = {}


# ---------------------------------------------------------------------------
# v3
# ---------------------------------------------------------------------------

GRP = 4          # 128-edge tiles per PSUM group (group = 512 edges = 1 bank)
BCOLS = 512      # logit-block tile-columns (block = 65536 edges)


def _build_nc_v3(rows_u, rows_m, caps, Hp, ch=CH, repeat=1):
    """rows_u/rows_m: rows per user/movie bank. caps: per-bucket edge capacity
    (each a multiple of TILE; bucket b = ubank*len(rows_m) + mbank).
    Hp: # hidden units with w2 >= 0 (after the pos-first permutation)."""
    f32 = mybir.dt.float32
    bf16 = mybir.dt.bfloat16
    i16 = mybir.dt.int16
    nbM = len(rows_m)
    L = int(sum(caps))
    ncols = L // P                   # total tile-columns
    nblk = -(-ncols // BCOLS)

    import os
    dbg_no_gather = os.environ.get("EDGE_V3_NO_GATHER") == "1"
    dbg_no_preload = os.environ.get("EDGE_V3_NO_PRELOAD") == "1"
    dbg_no_compute = os.environ.get("EDGE_V3_NO_COMPUTE") == "1"
    dbg_two_queue = os.environ.get("EDGE_V3_TWO_QUEUE") == "1"

    nc = bacc.Bacc(None, target_bir_lowering=False,
                   dynamic_dma_scratch_size=32768, num_swdge_queues=4)

    if dbg_no_gather:
        zdummy = nc.dram_tensor("zdummy", [P, ch], bf16, kind="ExternalInput")
    zu_b = [nc.dram_tensor(f"zu{i}", [r, H], bf16, kind="ExternalInput")
            for i, r in enumerate(rows_u)]
    zm_b = [nc.dram_tensor(f"zm{i}", [r, H], bf16, kind="ExternalInput")
            for i, r in enumerate(rows_m)]
    w1ut = nc.dram_tensor("w1ut", [H, H], bf16, kind="ExternalInput")
    w1mt = nc.dram_tensor("w1mt", [H, H], bf16, kind="ExternalInput")
    b1r4 = nc.dram_tensor("b1r4", [P, GRP * H], f32, kind="ExternalInput")
    b2c = nc.dram_tensor("b2c", [P, 1], f32, kind="ExternalInput")
    idxU = nc.dram_tensor("idxU", [P, L // 16], i16, kind="ExternalInput")
    idxM = nc.dram_tensor("idxM", [P, L // 16], i16, kind="ExternalInput")
    out_d = nc.dram_tensor("out", [nblk, P, BCOLS], f32, kind="ExternalOutput")

    with tile.TileContext(nc) as tc:
        with (
            tc.tile_pool(name="const", bufs=1) as cpool,
            tc.tile_pool(name="gat", bufs=4) as gpool,
            tc.tile_pool(name="idx", bufs=4) as ipool,
            tc.tile_pool(name="rel", bufs=4) as rpool,
            tc.tile_pool(name="lgb", bufs=2) as lpool,
            tc.tile_pool(name="obuf", bufs=2) as opool,
            tc.tile_pool(name="psum", bufs=4, space="PSUM") as ppool,
        ):
            nc.gpsimd.load_library(library_config.mlp)
            w1ut_t = cpool.tile([H, H], bf16)
            w1mt_t = cpool.tile([H, H], bf16)
            b1r4_t = cpool.tile([P, GRP * H], f32)
            b2_t = cpool.tile([P, 1], f32)
            nc.sync.dma_start(out=w1ut_t[:], in_=w1ut[:])
            nc.sync.dma_start(out=w1mt_t[:], in_=w1mt[:])
            nc.sync.dma_start(out=b1r4_t[:], in_=b1r4[:])
            nc.sync.dma_start(out=b2_t[:], in_=b2c[:])

            for _rep in range(repeat):
                col = 0              # global tile-column index
                lg_blk = None

                def drain_block(c0):
                    nco = c0 % BCOLS if c0 % BCOLS else BCOLS
                    blk = (c0 - 1) // BCOLS
                    ot = opool.tile([P, BCOLS], f32, tag="ot")
                    nc.scalar.activation(
                        out=ot[:, 0:nco], in_=lg_blk[:, 0:nco],
                        func=mybir.ActivationFunctionType.Sigmoid,
                        bias=b2_t[:, 0:1], scale=1.0)
                    nc.scalar.mul(out=ot[:, 0:nco], in_=ot[:, 0:nco], mul=5.0)
                    nc.sync.dma_start(out=out_d[blk, :, 0:nco], in_=ot[:, 0:nco])

                chunk_no = 0
                for b, cap in enumerate(caps):
                    bu, bm = divmod(b, nbM)
                    base = int(sum(caps[:b]))
                    pos = 0
                    while pos < cap:
                        cur = int(min(ch, cap - pos))
                        o16 = (base + pos) // 16
                        iu_t = ipool.tile([P, ch // 16], i16, tag="iu")
                        im_t = ipool.tile([P, ch // 16], i16, tag="im")
                        nc.sync.dma_start(out=iu_t[:, 0:cur // 16],
                                          in_=idxU[:, o16:o16 + cur // 16])
                        nc.sync.dma_start(out=im_t[:, 0:cur // 16],
                                          in_=idxM[:, o16:o16 + cur // 16])
                        ut = gpool.tile([P, cur], bf16, tag="ut")
                        mt = gpool.tile([P, cur], bf16, tag="mt")
                        if dbg_no_gather:
                            nc.sync.dma_start(out=ut[:], in_=zdummy[:, 0:cur])
                            nc.sync.dma_start(out=mt[:], in_=zdummy[:, 0:cur])
                        else:
                            # NOTE: transpose gathers corrupt data when run
                            # concurrently on multiple queues (shared xbar
                            # scratch) - keep both on queue 0.
                            nc.gpsimd.dma_gather(
                                out_ap=ut[:].rearrange("p (a n) -> p a n", a=1),
                                in_ap=zu_b[bu][:],
                                idxs_ap=iu_t[:, 0:cur // 16],
                                num_idxs=cur, num_idxs_reg=cur, elem_size=H,
                                transpose=True, queue_num=0,
                                single_packet=False)
                            nc.gpsimd.dma_gather(
                                out_ap=mt[:].rearrange("p (a n) -> p a n", a=1),
                                in_ap=zm_b[bm][:],
                                idxs_ap=im_t[:, 0:cur // 16],
                                num_idxs=cur, num_idxs_reg=cur, elem_size=H,
                                transpose=True, queue_num=0,
                                single_packet=False)
                        chunk_no += 1
                        for g in range(cur // TILE):
                            if col % BCOLS == 0:
                                lg_blk = lpool.tile([P, BCOLS], f32, tag="lg")
                            if dbg_no_compute:
                                col += GRP
                                continue
                            ps = ppool.tile([P, GRP * H], f32, tag="ps")
                            if not dbg_no_preload:
                                nc.scalar.copy(out=ps[:], in_=b1r4_t[:])
                            for t in range(GRP):
                                e0 = (g * GRP + t) * P
                                nc.tensor.matmul(
                                    out=ps[:, t * H:(t + 1) * H],
                                    lhsT=ut[:, e0:e0 + P], rhs=w1ut_t[:],
                                    start=dbg_no_preload, stop=False,
                                    skip_group_check=True)
                                nc.tensor.matmul(
                                    out=ps[:, t * H:(t + 1) * H],
                                    lhsT=mt[:, e0:e0 + P], rhs=w1mt_t[:],
                                    start=False, stop=True,
                                    skip_group_check=True)
                            rl = rpool.tile([P, GRP * H], bf16, tag="rl")
                            nc.scalar.activation(
                                out=rl[:], in_=ps[:],
                                func=mybir.ActivationFunctionType.Relu)
                            rv = rl[:].rearrange("p (g h) -> p g h", h=H)
                            c4 = col % BCOLS
                            if Hp == H:
                                nc.vector.tensor_reduce(
                                    out=lg_blk[:, c4:c4 + GRP], in_=rv[:, :, :],
                                    axis=mybir.AxisListType.X,
                                    op=mybir.AluOpType.add)
                            else:
                                lgp = rpool.tile([P, GRP], f32, tag="lgp")
                                lgn = rpool.tile([P, GRP], f32, tag="lgn")
                                if Hp > 0:
                                    nc.vector.tensor_reduce(
                                        out=lgp[:], in_=rv[:, :, 0:Hp],
                                        axis=mybir.AxisListType.X,
                                        op=mybir.AluOpType.add)
                                nc.vector.tensor_reduce(
                                    out=lgn[:], in_=rv[:, :, Hp:H],
                                    axis=mybir.AxisListType.X,
                                    op=mybir.AluOpType.add)
                                if Hp > 0:
                                    nc.vector.tensor_sub(
                                        out=lg_blk[:, c4:c4 + GRP],
                                        in0=lgp[:], in1=lgn[:])
                                else:
                                    nc.vector.tensor_scalar_mul(
                                        out=lg_blk[:, c4:c4 + GRP],
                                        in0=lgn[:], scalar1=-1.0)
                            col += GRP
                            if col % BCOLS == 0 or col == ncols:
                                drain_block(col)
                        pos += cur
    nc.finalize()
    return nc


def _roundup(n, m):
    return ((n + m - 1) // m) * m


def _prepare_v3(z_user, z_movie, edge_index, W1, b1, W2, b2,
                n_cores=N_CORES, bank=BANK):
    import ml_dtypes
    bf16 = ml_dtypes.bfloat16
    z_user = np.asarray(z_user, dtype=np.float32)
    z_movie = np.asarray(z_movie, dtype=np.float32)
    edge_index = np.asarray(edge_index)
    W1 = np.asarray(W1, dtype=np.float32)
    b1 = np.asarray(b1, dtype=np.float32)
    W2 = np.asarray(W2, dtype=np.float32)
    b2 = np.asarray(b2, dtype=np.float32)

    E = edge_index.shape[1]
    rows = edge_index[0].astype(np.int64)
    cols = edge_index[1].astype(np.int64)
    NU, NM = z_user.shape[0], z_movie.shape[0]
    nbU, nbM = -(-NU // bank), -(-NM // bank)
    nbkt = nbU * nbM
    Epc = -(-E // n_cores)

    per_core = []
    cnts = np.zeros((n_cores, nbkt), dtype=np.int64)
    for c in range(n_cores):
        sl = slice(c * Epc, min((c + 1) * Epc, E))
        r, co = rows[sl], cols[sl]
        bkt = (r // bank) * nbM + (co // bank)
        order = np.argsort(bkt, kind="stable")
        cnts[c] = np.bincount(bkt, minlength=nbkt)
        per_core.append((sl, order, r, co, bkt))

    caps = np.maximum(_roundup(cnts.max(axis=0), TILE), TILE)
    offs = np.concatenate([[0], np.cumsum(caps)])
    L = int(offs[-1])

    # permute hidden units w2>=0 first; fold |w2| into W1 rows and b1.
    # logit = sum_pos relu(|w2|y) - sum_neg relu(|w2|y)
    w2 = W2.reshape(-1)
    perm = np.argsort(w2 < 0, kind="stable")
    Hp = int((w2 >= 0).sum())
    w2sc = np.abs(w2[perm])
    W1p = W1[perm] * w2sc[:, None]          # [h', 2H]
    b1p = b1[perm] * w2sc                   # [h']

    zu16 = np.ascontiguousarray(z_user.astype(bf16))
    zm16 = np.ascontiguousarray(z_movie.astype(bf16))
    shared = {"w1ut": np.ascontiguousarray(W1p[:, :H].T).astype(bf16),
              "w1mt": np.ascontiguousarray(W1p[:, H:].T).astype(bf16),
              "b1r4": np.ascontiguousarray(
                  np.tile(b1p, (P, GRP)).astype(np.float32)),
              "b2c": np.full((P, 1), float(b2.reshape(-1)[0]), np.float32)}
    rows_u, rows_m = [], []
    for i in range(nbU):
        bk = np.ascontiguousarray(zu16[i * bank:(i + 1) * bank])
        shared[f"zu{i}"] = bk
        rows_u.append(bk.shape[0])
    for i in range(nbM):
        bk = np.ascontiguousarray(zm16[i * bank:(i + 1) * bank])
        shared[f"zm{i}"] = bk
        rows_m.append(bk.shape[0])

    in_maps, backmaps = [], []
    for c in range(n_cores):
        sl, order, r, co, bkt = per_core[c]
        n = len(r)
        starts = np.concatenate([[0], np.cumsum(cnts[c])])
        sorted_bkt = bkt[order]
        k = np.arange(n) - starts[sorted_bkt]
        spos = offs[sorted_bkt] + k          # slot of edge order[i]
        iu = np.zeros(L, np.int16)
        im = np.zeros(L, np.int16)
        iu[spos] = (r[order] % bank).astype(np.int16)
        im[spos] = (co[order] % bank).astype(np.int16)
        slot = np.empty(n, np.int64)
        slot[order] = spos
        iu_w = np.ascontiguousarray(np.tile(iu.reshape(L // 16, 16).T, (8, 1)))
        im_w = np.ascontiguousarray(np.tile(im.reshape(L // 16, 16).T, (8, 1)))
        in_maps.append({**shared, "idxU": iu_w, "idxM": im_w})
        backmaps.append((sl, slot))
    return in_maps, dict(rows_u=rows_u, rows_m=rows_m,
                         caps=[int(x) for x in caps], L=L, E=E, Hp=Hp,
                         backmaps=backmaps)


def _unpack_v3(res, meta):
    out = np.empty(meta["E"], dtype=np.float32)
    for c, (sl, slot) in enumerate(meta["backmaps"]):
        flat = np.asarray(res.results[c]["out"], dtype=np.float32).reshape(-1)
        # edge at stream slot s -> tile-column s//128, partition s%128;
        # out tensor is [nblk, 128, BCOLS]
        tc_ = slot // P
        p = slot % P
        fidx = (tc_ // BCOLS) * (P * BCOLS) + p * BCOLS + (tc_ % BCOLS)
        out[sl] = flat[fidx]
    return out


# ---------------------------------------------------------------------------
# v6: device-precomputed A/B node tables (A=W1u z_u + b1, B=W1m z_m, |w2|
# folded, pos-w2-first permutation) + per-edge dual NON-transpose dma_gather
# spread over all 4 SWDGE queues (measured: 1 queue = 7.9 ns/row, 4 queues =
# 1.79 ns/row). Edges land on partitions, H on free dim: DVE add, ACT relu,
# DVE pos/neg reduces -> logits. No per-edge PE work.
# Slot mapping identical to v3 (slot s -> partition s%128, tile-col s//128).
# ---------------------------------------------------------------------------

NT = 50176       # referenced node rows padded to 98*512 (indices < 50000)


def _build_nc_v6(rows_u, rows_m, caps, Hp, ch=CH, repeat=1):
    """rows_u/rows_m: rows per user/movie table bank (sum = NT each).
    caps: per-bucket edge capacity (multiples of TILE; bucket b =
    ubank*len(rows_m) + mbank). Hp: # hidden units with w2 >= 0."""
    f32 = mybir.dt.float32
    bf16 = mybir.dt.bfloat16
    i16 = mybir.dt.int16
    nbM = len(rows_m)
    L = int(sum(caps))
    ncols = L // P                   # total tile-columns
    nblk = -(-ncols // BCOLS)
    assert 0 < Hp < H

    import os
    dbg_no_gather = os.environ.get("EDGE_V6_NO_GATHER") == "1"
    dbg_no_precomp = os.environ.get("EDGE_V6_NO_PRECOMP") == "1"
    dbg_no_compute = os.environ.get("EDGE_V6_NO_COMPUTE") == "1"

    nc = bacc.Bacc(None, target_bir_lowering=False,
                   dynamic_dma_scratch_size=32768, num_swdge_queues=4)

    zuT = nc.dram_tensor("zuT", [P, NT], bf16, kind="ExternalInput")
    zmT = nc.dram_tensor("zmT", [P, NT], bf16, kind="ExternalInput")
    w1ut = nc.dram_tensor("w1ut", [H, H], bf16, kind="ExternalInput")
    w1mt = nc.dram_tensor("w1mt", [H, H], bf16, kind="ExternalInput")
    b1r4 = nc.dram_tensor("b1r4", [P, GRP * H], f32, kind="ExternalInput")
    b2c = nc.dram_tensor("b2c", [P, 1], f32, kind="ExternalInput")
    idxU = nc.dram_tensor("idxU", [P, L // 16], i16, kind="ExternalInput")
    idxM = nc.dram_tensor("idxM", [P, L // 16], i16, kind="ExternalInput")
    out_d = nc.dram_tensor("out", [nblk, P, BCOLS], f32, kind="ExternalOutput")

    tabs_u = [nc.dram_tensor(f"tabU{i}", [r, H], bf16) for i, r in enumerate(rows_u)]
    tabs_m = [nc.dram_tensor(f"tabM{i}", [r, H], bf16) for i, r in enumerate(rows_m)]

    with tile.TileContext(nc) as tc:
        with (
            tc.tile_pool(name="const", bufs=1) as cpool,
            tc.tile_pool(name="pre", bufs=2) as prepool,
            tc.tile_pool(name="gat", bufs=3) as gpool,
            tc.tile_pool(name="idx", bufs=4) as ipool,
            tc.tile_pool(name="rel", bufs=2) as rpool,
            tc.tile_pool(name="lgs", bufs=3) as spool,
            tc.tile_pool(name="lgb", bufs=2) as lpool,
            tc.tile_pool(name="obuf", bufs=2) as opool,
            tc.tile_pool(name="psum", bufs=4, space="PSUM") as ppool,
        ):
            nc.gpsimd.load_library(library_config.mlp)
            w1ut_t = cpool.tile([H, H], bf16)
            w1mt_t = cpool.tile([H, H], bf16)
            b1r4_t = cpool.tile([P, GRP * H], f32)
            b2_t = cpool.tile([P, 1], f32)
            nc.sync.dma_start(out=w1ut_t[:], in_=w1ut[:])
            nc.sync.dma_start(out=w1mt_t[:], in_=w1mt[:])
            nc.sync.dma_start(out=b1r4_t[:], in_=b1r4[:])
            nc.sync.dma_start(out=b2_t[:], in_=b2c[:])

            for _rep in range(repeat):
                # ---- precompute node tables (bank-interleaved U0,M0,U1,M1
                # so bucket (0,0) gathers can start early) ----
                gno = 0
                ZB = 4096            # z columns staged per DMA (8 groups)
                for bi in range(len(rows_u) if not dbg_no_precomp else 0):
                    for (zT, w1t, tabs, rows, addb1, goff) in (
                        (zuT, w1ut_t, tabs_u, rows_u, True, 0),
                        (zmT, w1mt_t, tabs_m, rows_m, False, 0),
                    ):
                        base = int(sum(rows[:bi]))
                        for z0 in range(0, rows[bi], ZB):
                            zn = min(ZB, rows[bi] - z0)
                            zbig = prepool.tile([P, ZB], bf16, tag="zst")
                            nc.sync.dma_start(
                                out=zbig[:, 0:zn],
                                in_=zT[:, base + z0:base + z0 + zn])
                            for s in range(zn // TILE):
                                so = z0 // TILE + s
                                pps = ppool.tile([P, GRP * H], f32, tag="ps")
                                if addb1:
                                    nc.scalar.copy(out=pps[:], in_=b1r4_t[:])
                                for t in range(GRP):
                                    nc.tensor.matmul(
                                        out=pps[:, t * H:(t + 1) * H],
                                        lhsT=zbig[:, s * TILE + t * P:
                                                  s * TILE + (t + 1) * P],
                                        rhs=w1t[:], start=not addb1,
                                        stop=True, skip_group_check=True)
                                ast = prepool.tile([P, GRP * H], bf16,
                                                   tag="ast")
                                if gno % 2 == 0:
                                    nc.scalar.copy(out=ast[:], in_=pps[:])
                                else:
                                    nc.vector.tensor_copy(out=ast[:],
                                                          in_=pps[:])
                                gno += 1
                                nc.sync.dma_start(
                                    out=tabs[bi][so * TILE:(so + 1) * TILE, :]
                                    .rearrange("(t p) h -> p t h", p=P),
                                    in_=ast[:].rearrange("p (t h) -> p t h",
                                                         h=H))

                # ---- edge phase ----
                col = 0              # global tile-column index
                lg_blk = None

                def drain_block(c0):
                    nco = c0 % BCOLS if c0 % BCOLS else BCOLS
                    blk = (c0 - 1) // BCOLS
                    ot = opool.tile([P, BCOLS], f32, tag="ot")
                    nc.scalar.activation(
                        out=ot[:, 0:nco], in_=lg_blk[:, 0:nco],
                        func=mybir.ActivationFunctionType.Sigmoid,
                        bias=b2_t[:, 0:1], scale=1.0)
                    nc.scalar.mul(out=ot[:, 0:nco], in_=ot[:, 0:nco], mul=5.0)
                    nc.sync.dma_start(out=out_d[blk, :, 0:nco], in_=ot[:, 0:nco])

                chunk_no = 0
                for b, cap in enumerate(caps):
                    bu, bm = divmod(b, nbM)
                    base = int(sum(caps[:b]))
                    pos = 0
                    while pos < cap:
                        cur = int(min(ch, cap - pos))
                        o16 = (base + pos) // 16
                        na = cur // P        # tile-cols in this chunk
                        iu_t = ipool.tile([P, ch // 16], i16, tag="iu")
                        im_t = ipool.tile([P, ch // 16], i16, tag="im")
                        nc.sync.dma_start(out=iu_t[:, 0:cur // 16],
                                          in_=idxU[:, o16:o16 + cur // 16])
                        nc.sync.dma_start(out=im_t[:, 0:cur // 16],
                                          in_=idxM[:, o16:o16 + cur // 16])
                        ut = gpool.tile([P, ch], bf16, tag="ut")
                        mt = gpool.tile([P, ch], bf16, tag="mt")
                        if not dbg_no_gather:
                            nc.gpsimd.dma_gather(
                                out_ap=ut[:, 0:cur].rearrange(
                                    "p (a n) -> p a n", a=na),
                                in_ap=tabs_u[bu][:],
                                idxs_ap=iu_t[:, 0:cur // 16],
                                num_idxs=cur, num_idxs_reg=cur, elem_size=H,
                                transpose=False,
                                queue_num=(2 * chunk_no) % 4,
                                single_packet=False)
                            nc.gpsimd.dma_gather(
                                out_ap=mt[:, 0:cur].rearrange(
                                    "p (a n) -> p a n", a=na),
                                in_ap=tabs_m[bm][:],
                                idxs_ap=im_t[:, 0:cur // 16],
                                num_idxs=cur, num_idxs_reg=cur, elem_size=H,
                                transpose=False,
                                queue_num=(2 * chunk_no + 1) % 4,
                                single_packet=False)
                        chunk_no += 1
                        if dbg_no_compute:
                            col += na
                            if col % BCOLS == 0 or col >= ncols:
                                pass
                            pos += cur
                            continue
                        yt = rpool.tile([P, ch], bf16, tag="yt")
                        nc.vector.tensor_add(out=yt[:, 0:cur], in0=ut[:, 0:cur],
                                             in1=mt[:, 0:cur])
                        rl = rpool.tile([P, ch], bf16, tag="rl")
                        nc.scalar.activation(
                            out=rl[:, 0:cur], in_=yt[:, 0:cur],
                            func=mybir.ActivationFunctionType.Relu)
                        rv = rl[:, 0:cur].rearrange("p (a h) -> p a h", h=H)
                        # pos/neg reduces -> logits, split at block boundaries
                        a0 = 0
                        while a0 < na:
                            if col % BCOLS == 0:
                                lg_blk = lpool.tile([P, BCOLS], f32, tag="lg")
                            c4 = col % BCOLS
                            seg = int(min(na - a0, BCOLS - c4))
                            lgp = spool.tile([P, ch // P], f32, tag="lgp")
                            lgn = spool.tile([P, ch // P], f32, tag="lgn")
                            nc.vector.tensor_reduce(
                                out=lgp[:, 0:seg], in_=rv[:, a0:a0 + seg, 0:Hp],
                                axis=mybir.AxisListType.X,
                                op=mybir.AluOpType.add)
                            nc.vector.tensor_reduce(
                                out=lgn[:, 0:seg], in_=rv[:, a0:a0 + seg, Hp:H],
                                axis=mybir.AxisListType.X,
                                op=mybir.AluOpType.add)
                            nc.vector.tensor_sub(
                                out=lg_blk[:, c4:c4 + seg],
                                in0=lgp[:, 0:seg], in1=lgn[:, 0:seg])
                            col += seg
                            a0 += seg
                            if col % BCOLS == 0 or col == ncols:
                                drain_block(col)
                        pos += cur
    nc.finalize()
    return nc


def _prepare_v6(z_user, z_movie, edge_index, W1, b1, W2, b2,
                n_cores=N_CORES, bank=BANK):
    import ml_dtypes
    bf16 = ml_dtypes.bfloat16
    z_user = np.asarray(z_user, dtype=np.float32)
    z_movie = np.asarray(z_movie, dtype=np.float32)
    edge_index = np.asarray(edge_index)
    W1 = np.asarray(W1, dtype=np.float32)
    b1 = np.asarray(b1, dtype=np.float32)
    W2 = np.asarray(W2, dtype=np.float32)
    b2 = np.asarray(b2, dtype=np.float32)

    E = edge_index.shape[1]
    rows = edge_index[0].astype(np.int64)
    cols = edge_index[1].astype(np.int64)
    if E and (rows.max() >= NT or cols.max() >= NT):
        raise ValueError("edge index out of v6 table range")
    nbU = nbM = -(-NT // bank)
    nbkt = nbU * nbM
    Epc = -(-E // n_cores)

    per_core = []
    cnts = np.zeros((n_cores, nbkt), dtype=np.int64)
    for c in range(n_cores):
        sl = slice(c * Epc, min((c + 1) * Epc, E))
        r, co = rows[sl], cols[sl]
        bkt = (r // bank) * nbM + (co // bank)
        order = np.argsort(bkt, kind="stable")
        cnts[c] = np.bincount(bkt, minlength=nbkt)
        per_core.append((sl, order, r, co, bkt))

    caps = np.maximum(_roundup(cnts.max(axis=0), TILE), TILE)
    offs = np.concatenate([[0], np.cumsum(caps)])
    L = int(offs[-1])

    # permute hidden units w2>=0 first; fold |w2| into W1 rows and b1.
    w2 = W2.reshape(-1)
    perm = np.argsort(w2 < 0, kind="stable")
    Hp = int((w2 >= 0).sum())
    w2sc = np.abs(w2[perm])
    W1p = W1[perm] * w2sc[:, None]          # [h', 2H]
    b1p = b1[perm] * w2sc                   # [h']

    nuse_u = min(z_user.shape[0], NT)
    nuse_m = min(z_movie.shape[0], NT)
    zuT = np.zeros((P, NT), dtype=bf16)
    zuT[:, :nuse_u] = z_user[:nuse_u].T.astype(bf16)
    zmT = np.zeros((P, NT), dtype=bf16)
    zmT[:, :nuse_m] = z_movie[:nuse_m].T.astype(bf16)
    shared = {"zuT": zuT, "zmT": zmT,
              "w1ut": np.ascontiguousarray(W1p[:, :H].T).astype(bf16),
              "w1mt": np.ascontiguousarray(W1p[:, H:].T).astype(bf16),
              "b1r4": np.ascontiguousarray(
                  np.tile(b1p, (P, GRP)).astype(np.float32)),
              "b2c": np.full((P, 1), float(b2.reshape(-1)[0]), np.float32)}
    rows_u = [min(bank, NT - i * bank) for i in range(nbU)]
    rows_m = [min(bank, NT - i * bank) for i in range(nbM)]

    in_maps, backmaps = [], []
    for c in range(n_cores):
        sl, order, r, co, bkt = per_core[c]
        n = len(r)
        starts = np.concatenate([[0], np.cumsum(cnts[c])])
        sorted_bkt = bkt[order]
        k = np.arange(n) - starts[sorted_bkt]
        spos = offs[sorted_bkt] + k          # slot of edge order[i]
        iu = np.zeros(L, np.int16)
        im = np.zeros(L, np.int16)
        iu[spos] = (r[order] % bank).astype(np.int16)
        im[spos] = (co[order] % bank).astype(np.int16)
        slot = np.empty(n, np.int64)
        slot[order] = spos
        iu_w = np.ascontiguousarray(np.tile(iu.reshape(L // 16, 16).T, (8, 1)))
        im_w = np.ascontiguousarray(np.tile(im.reshape(L // 16, 16).T, (8, 1)))
        in_maps.append({**shared, "idxU": iu_w, "idxM": im_w})
        backmaps.append((sl, slot))
    return in_maps, dict(rows_u=rows_u, rows_m=rows_m,
                         caps=[int(x) for x in caps], L=L, E=E, Hp=Hp,
                         backmaps=backmaps)


# ---------------------------------------------------------------------------
# v7: user-range sharding. A-side (user) via PE one-hot expansion: edges
# sorted by (movie-bank, local user window); host streams bf16 one-hot masks
# (index-derived only); window tiles of the per-core A table feed PE as rhs.
# B-side (movie) via non-transpose dma_gather over all 4 SWDGE queues.
# Tables precomputed on device (A per-core slice w/ b1+|w2| fold; B full).
# Slot mapping identical to v3/v6 (slot s -> partition s%128, col s//128).
# ---------------------------------------------------------------------------

UPC7 = 6272      # users per core (50176/8); window = 128 users, 49/core


def _v7_schedule(caps):
    """caps: [2][49] window slot capacities (each mult of 16; run totals mult
    of 512). Returns (sched, naux): sched = per 512-slot group the list of
    window ids (global: mb*49 + w); naux = total aux mask tiles."""
    nwin = len(caps[0])
    sched = []
    base = 0
    for mb in range(2):
        run = int(sum(caps[mb]))
        assert run % 512 == 0
        starts = np.concatenate([[0], np.cumsum(caps[mb])])
        for g0 in range(run // 512):
            lo, hi = g0 * 512, (g0 + 1) * 512
            w_lo = int(np.searchsorted(starts, lo, side="right") - 1)
            w_hi = int(np.searchsorted(starts, hi - 1, side="right") - 1)
            sched.append([mb * nwin + w for w in range(w_lo, w_hi + 1)])
        base += run
    naux = sum(len(ws) - 1 for ws in sched)
    return sched, naux


def _build_nc_v7(caps, Hp, sched, naux, ch=CH, repeat=1):
    """caps: [2][nwin] window capacities. sched/naux: from _v7_schedule."""
    f32 = mybir.dt.float32
    bf16 = mybir.dt.bfloat16
    i16 = mybir.dt.int16
    nwin = len(caps[0])
    run_len = [int(sum(caps[mb])) for mb in range(2)]
    L = sum(run_len)
    ncols = L // P
    nblk = -(-ncols // BCOLS)
    NTU = nwin * P               # per-core A rows (6272)
    rows_m = [BANK, NT - BANK]
    assert 0 < Hp < H and L % 512 == 0

    nc = bacc.Bacc(None, target_bir_lowering=False,
                   dynamic_dma_scratch_size=32768, num_swdge_queues=4)

    zuTc = nc.dram_tensor("zuTc", [P, NTU], bf16, kind="ExternalInput")
    zmT = nc.dram_tensor("zmT", [P, NT], bf16, kind="ExternalInput")
    w1ut = nc.dram_tensor("w1ut", [H, H], bf16, kind="ExternalInput")
    w1mt = nc.dram_tensor("w1mt", [H, H], bf16, kind="ExternalInput")
    b1r4 = nc.dram_tensor("b1r4", [P, GRP * H], f32, kind="ExternalInput")
    b2c = nc.dram_tensor("b2c", [P, 1], f32, kind="ExternalInput")
    idxM = nc.dram_tensor("idxM", [P, L // 16], i16, kind="ExternalInput")
    mask0 = nc.dram_tensor("mask0", [P, L], bf16, kind="ExternalInput")
    maskx = nc.dram_tensor("maskx", [P, max(naux, 1) * 512], bf16,
                           kind="ExternalInput")
    out_d = nc.dram_tensor("out", [nblk, P, BCOLS], f32, kind="ExternalOutput")

    tabU = nc.dram_tensor("tabU", [NTU, H], bf16)
    tabs_m = [nc.dram_tensor(f"tabM{i}", [r, H], bf16)
              for i, r in enumerate(rows_m)]

    with tile.TileContext(nc) as tc:
        with (
            tc.tile_pool(name="const", bufs=1) as cpool,
            tc.tile_pool(name="pre", bufs=3) as prepool,
            tc.tile_pool(name="gat", bufs=4) as gpool,
            tc.tile_pool(name="msk", bufs=3) as mpool,
            tc.tile_pool(name="idx", bufs=4) as ipool,
            tc.tile_pool(name="win", bufs=4) as wpool,
            tc.tile_pool(name="aux", bufs=3) as xpool,
            tc.tile_pool(name="rel", bufs=4) as rpool,
            tc.tile_pool(name="lgs", bufs=4) as spool,
            tc.tile_pool(name="lgb", bufs=2) as lpool,
            tc.tile_pool(name="obuf", bufs=2) as opool,
            tc.tile_pool(name="psum", bufs=4, space="PSUM") as ppool,
        ):
            nc.gpsimd.load_library(library_config.mlp)
            w1ut_t = cpool.tile([H, H], bf16)
            w1mt_t = cpool.tile([H, H], bf16)
            b1r4_t = cpool.tile([P, GRP * H], f32)
            b2_t = cpool.tile([P, 1], f32)
            nc.sync.dma_start(out=w1ut_t[:], in_=w1ut[:])
            nc.sync.dma_start(out=w1mt_t[:], in_=w1mt[:])
            nc.sync.dma_start(out=b1r4_t[:], in_=b1r4[:])
            nc.sync.dma_start(out=b2_t[:], in_=b2c[:])

            for _rep in range(repeat):
                # ---- precompute: tabM bank0, tabU slice, tabM bank1 ----
                gno = 0

                ZB = 4096            # z columns staged per DMA (8 groups)

                def pre_groups(zT, w1t, tab, zoff, n512, addb1):
                    nonlocal gno
                    for z0 in range(0, n512 * TILE, ZB):
                        zn = min(ZB, n512 * TILE - z0)
                        zbig = prepool.tile([P, ZB], bf16, tag="zst")
                        nc.sync.dma_start(
                            out=zbig[:, 0:zn],
                            in_=zT[:, zoff + z0:zoff + z0 + zn])
                        for s in range(zn // TILE):
                            so = z0 // TILE + s
                            pps = ppool.tile([P, GRP * H], f32, tag="ps")
                            if addb1:
                                nc.scalar.copy(out=pps[:], in_=b1r4_t[:])
                            for t in range(GRP):
                                nc.tensor.matmul(
                                    out=pps[:, t * H:(t + 1) * H],
                                    lhsT=zbig[:, s * TILE + t * P:
                                              s * TILE + (t + 1) * P],
                                    rhs=w1t[:], start=not addb1,
                                    stop=True, skip_group_check=True)
                            ast = prepool.tile([P, GRP * H], bf16, tag="ast")
                            if gno % 2 == 0:
                                nc.scalar.copy(out=ast[:], in_=pps[:])
                            else:
                                nc.vector.tensor_copy(out=ast[:], in_=pps[:])
                            gno += 1
                            nc.sync.dma_start(
                                out=tab[so * TILE:(so + 1) * TILE, :]
                                .rearrange("(t p) h -> p t h", p=P),
                                in_=ast[:].rearrange("p (t h) -> p t h", h=H))

                pre_groups(zmT, w1mt_t, tabs_m[0], 0, BANK // TILE, False)
                pre_groups(zuTc, w1ut_t, tabU, 0, NTU // TILE, True)
                pre_groups(zmT, w1mt_t, tabs_m[1], BANK,
                           (NT - BANK) // TILE, False)

                # ---- edge phase ----
                col = 0
                lg_blk = None
                aux_no = 0
                g_global = 0
                wt_cache = {}            # window id -> (handle, load_ordinal)
                wt_loads = 0

                def get_window(w):
                    nonlocal wt_loads
                    ent = wt_cache.get(w)
                    if ent is not None and wt_loads - ent[1] < 4:
                        return ent[0]
                    wt = wpool.tile([P, H], bf16, tag="wt")
                    r0 = (w % nwin) * P
                    nc.sync.dma_start(out=wt[:], in_=tabU[r0:r0 + P, :])
                    wt_cache[w] = (wt, wt_loads)
                    wt_loads += 1
                    return wt

                def drain_block(c0):
                    nco = c0 % BCOLS if c0 % BCOLS else BCOLS
                    blk = (c0 - 1) // BCOLS
                    ot = opool.tile([P, BCOLS], f32, tag="ot")
                    nc.scalar.activation(
                        out=ot[:, 0:nco], in_=lg_blk[:, 0:nco],
                        func=mybir.ActivationFunctionType.Sigmoid,
                        bias=b2_t[:, 0:1], scale=1.0)
                    nc.scalar.mul(out=ot[:, 0:nco], in_=ot[:, 0:nco], mul=5.0)
                    nc.sync.dma_start(out=out_d[blk, :, 0:nco], in_=ot[:, 0:nco])

                chunk_no = 0
                for mb in range(2):
                    base = sum(run_len[:mb])
                    cap = run_len[mb]
                    pos = 0
                    while pos < cap:
                        cur = int(min(ch, cap - pos))
                        s0 = base + pos
                        im_t = ipool.tile([P, ch // 16], i16, tag="im")
                        nc.sync.dma_start(
                            out=im_t[:, 0:cur // 16],
                            in_=idxM[:, s0 // 16:(s0 + cur) // 16])
                        bt = gpool.tile([P, ch], bf16, tag="bt")
                        nc.gpsimd.dma_gather(
                            out_ap=bt[:, 0:cur].rearrange(
                                "p (a n) -> p a n", a=cur // P),
                            in_ap=tabs_m[mb][:],
                            idxs_ap=im_t[:, 0:cur // 16],
                            num_idxs=cur, num_idxs_reg=cur, elem_size=H,
                            transpose=False, queue_num=chunk_no % 4,
                            single_packet=False)
                        chunk_no += 1
                        mk0 = mpool.tile([P, ch], bf16, tag="mk0")
                        nc.sync.dma_start(out=mk0[:, 0:cur],
                                          in_=mask0[:, s0:s0 + cur])
                        for gi in range(cur // 512):
                            wins = sched[g_global]
                            ps = ppool.tile([P, GRP * H], f32, tag="eps")
                            for ki, w in enumerate(wins):
                                if ki == 0:
                                    mk_t, moff = mk0, gi * 512
                                else:
                                    mk_t = xpool.tile([P, 512], bf16, tag="mx")
                                    nc.sync.dma_start(
                                        out=mk_t[:],
                                        in_=maskx[:, aux_no * 512:
                                                  (aux_no + 1) * 512])
                                    moff = 0
                                    aux_no += 1
                                wt = get_window(w)
                                for t in range(GRP):
                                    nc.tensor.matmul(
                                        out=ps[:, t * H:(t + 1) * H],
                                        lhsT=mk_t[:, moff + t * P:
                                                  moff + (t + 1) * P],
                                        rhs=wt[:],
                                        start=(ki == 0),
                                        stop=(ki == len(wins) - 1),
                                        skip_group_check=True)
                            g_global += 1
                            yt = rpool.tile([P, GRP * H], bf16, tag="yt")
                            nc.vector.tensor_add(
                                out=yt[:], in0=ps[:],
                                in1=bt[:, gi * GRP * H:(gi + 1) * GRP * H])
                            nc.scalar.activation(
                                out=yt[:], in_=yt[:],
                                func=mybir.ActivationFunctionType.Relu)
                            rv = yt[:].rearrange("p (a h) -> p a h", h=H)
                            if col % BCOLS == 0:
                                lg_blk = lpool.tile([P, BCOLS], f32, tag="lg")
                            c4 = col % BCOLS
                            lgp = spool.tile([P, GRP], f32, tag="lgp")
                            lgn = spool.tile([P, GRP], f32, tag="lgn")
                            nc.vector.tensor_reduce(
                                out=lgp[:], in_=rv[:, :, 0:Hp],
                                axis=mybir.AxisListType.X,
                                op=mybir.AluOpType.add)
                            nc.vector.tensor_reduce(
                                out=lgn[:], in_=rv[:, :, Hp:H],
                                axis=mybir.AxisListType.X,
                                op=mybir.AluOpType.add)
                            nc.vector.tensor_sub(
                                out=lg_blk[:, c4:c4 + GRP],
                                in0=lgp[:], in1=lgn[:])
                            col += GRP
                            if col % BCOLS == 0 or col == ncols:
                                drain_block(col)
                        pos += cur
    nc.finalize()
    return nc


def _prepare_v7(z_user, z_movie, edge_index, W1, b1, W2, b2,
                n_cores=N_CORES):
    import ml_dtypes
    bf16 = ml_dtypes.bfloat16
    z_user = np.asarray(z_user, dtype=np.float32)
    z_movie = np.asarray(z_movie, dtype=np.float32)
    edge_index = np.asarray(edge_index)
    W1 = np.asarray(W1, dtype=np.float32)
    b1 = np.asarray(b1, dtype=np.float32)
    W2 = np.asarray(W2, dtype=np.float32)
    b2 = np.asarray(b2, dtype=np.float32)

    E = edge_index.shape[1]
    rows = edge_index[0].astype(np.int64)
    cols = edge_index[1].astype(np.int64)
    if E and (rows.max() >= NT or cols.max() >= NT):
        raise ValueError("edge index out of v7 table range")
    nwin = UPC7 // P

    # per-core split (by user range), then by movie bank, then by window
    core_of = rows // UPC7
    per_core = []
    wcnt = np.zeros((n_cores, 2, nwin), dtype=np.int64)
    for c in range(n_cores):
        eids = np.nonzero(core_of == c)[0]
        r, co = rows[eids], cols[eids]
        u = r - c * UPC7
        mb = co // BANK
        w = u // P
        order = np.lexsort((w, mb))
        eids, u, co, mb, w = eids[order], u[order], co[order], mb[order], w[order]
        for b in range(2):
            wcnt[c, b] = np.bincount(w[mb == b], minlength=nwin)
        per_core.append((eids, u, co, mb, w))

    # shared window capacities: max over cores, round to 16; run mult of 512
    caps = np.maximum(_roundup(wcnt.max(axis=0), 16), 16)
    for b in range(2):
        tot = int(caps[b].sum())
        caps[b][-1] += _roundup(tot, 512) - tot
    run_len = [int(caps[b].sum()) for b in range(2)]
    L = sum(run_len)
    starts = np.zeros((2, nwin), dtype=np.int64)
    for b in range(2):
        starts[b] = sum(run_len[:b]) + np.concatenate(
            [[0], np.cumsum(caps[b])[:-1]])

    sched, naux = _v7_schedule([list(map(int, caps[0])),
                                list(map(int, caps[1]))])
    # aux ordinal lookup: (group, window) -> ordinal for non-first windows
    aux_of = {}
    k = 0
    for g, ws in enumerate(sched):
        for wi in ws[1:]:
            aux_of[(g, wi)] = k
            k += 1
    assert k == naux

    w2v = W2.reshape(-1)
    perm = np.argsort(w2v < 0, kind="stable")
    Hp = int((w2v >= 0).sum())
    w2sc = np.abs(w2v[perm])
    W1p = W1[perm] * w2sc[:, None]
    b1p = b1[perm] * w2sc

    nuse_m = min(z_movie.shape[0], NT)
    zmT = np.zeros((P, NT), dtype=bf16)
    zmT[:, :nuse_m] = z_movie[:nuse_m].T.astype(bf16)
    shared = {"zmT": zmT,
              "w1ut": np.ascontiguousarray(W1p[:, :H].T).astype(bf16),
              "w1mt": np.ascontiguousarray(W1p[:, H:].T).astype(bf16),
              "b1r4": np.ascontiguousarray(
                  np.tile(b1p, (P, GRP)).astype(np.float32)),
              "b2c": np.full((P, 1), float(b2.reshape(-1)[0]), np.float32)}

    sched_w0 = np.array([ws[0] for ws in sched], dtype=np.int64)
    in_maps, backmaps = [], []
    for c in range(n_cores):
        eids, u, co, mb, w = per_core[c]
        # slot: within-window rank
        wk = mb * nwin + w
        ordr = np.argsort(wk, kind="stable")   # already sorted; rank within
        kk = np.arange(len(u)) - np.concatenate(
            [[0], np.cumsum(np.bincount(wk, minlength=2 * nwin))])[wk]
        slot = starts[mb, w] + kk
        g = slot // 512
        wg = mb * nwin + w                     # global window id of each edge
        is_first = wg == sched_w0[g]
        urow = (u % P).astype(np.int64)
        m0 = np.zeros((P, L), dtype=bf16)
        m0[urow[is_first], slot[is_first]] = 1
        mx = np.zeros((P, max(naux, 1) * 512), dtype=bf16)
        nf = np.nonzero(~is_first)[0]
        if len(nf):
            aux_idx = np.array([aux_of[(int(g[i]), int(wg[i]))] for i in nf],
                               dtype=np.int64)
            mx[urow[nf], aux_idx * 512 + (slot[nf] % 512)] = 1
        im = np.zeros(L, np.int16)
        im[slot] = (co % BANK).astype(np.int16)
        im_w = np.ascontiguousarray(np.tile(im.reshape(L // 16, 16).T, (8, 1)))
        zuTc = np.zeros((P, UPC7), dtype=bf16)
        lo = c * UPC7
        hi = min((c + 1) * UPC7, z_user.shape[0])
        if hi > lo:
            zuTc[:, :hi - lo] = z_user[lo:hi].T.astype(bf16)
        in_maps.append({**shared, "zuTc": zuTc, "idxM": im_w,
                        "mask0": m0, "maskx": mx})
        backmaps.append((eids, slot))
    caps_py = [list(map(int, caps[0])), list(map(int, caps[1]))]
    return in_maps, dict(caps=caps_py, sched=sched, naux=naux, L=L, E=E,
                         Hp=Hp, backmaps=backmaps)


def _unpack_v7(res, meta):
    out = np.empty(meta["E"], dtype=np.float32)
    for c, (eids, slot) in enumerate(meta["backmaps"]):
        flat = np.asarray(res.results[c]["out"], dtype=np.float32).reshape(-1)
        tc_ = slot // P
        p = slot % P
        fidx = (tc_ // BCOLS) * (P * BCOLS) + p * BCOLS + (tc_ % BCOLS)
        out[eids] = flat[fidx]
    return out


# ---------------------------------------------------------------------------
# v1 (fallback): precomputed tables + per-column indirect DMA gathers
# ---------------------------------------------------------------------------

def _build_nc(C, NA, NB, Hp, repeat=1, repeat_pre=None, repeat_gather=None):
    """C: edge cols per core (edges = 128*C). NA/NB: padded table rows. Hp: # pos-w2 units.
    repeat>1 re-runs the compute phases (identical results) for slope-based timing."""
    f32 = mybir.dt.float32
    i32 = mybir.dt.int32
    nc = bacc.Bacc(None, target_bir_lowering=False)

    zTu = nc.dram_tensor("zTu", [P, NA], f32, kind="ExternalInput")
    zTm = nc.dram_tensor("zTm", [P, NB], f32, kind="ExternalInput")
    w1ut = nc.dram_tensor("w1ut", [P, H], f32, kind="ExternalInput")
    w1mt = nc.dram_tensor("w1mt", [P, H], f32, kind="ExternalInput")
    b1rep = nc.dram_tensor("b1rep", [P, H], f32, kind="ExternalInput")
    b2rep = nc.dram_tensor("b2rep", [P, 1], f32, kind="ExternalInput")
    idxA = nc.dram_tensor("idxA", [P, C], i32, kind="ExternalInput")
    idxB = nc.dram_tensor("idxB", [P, C], i32, kind="ExternalInput")
    out_d = nc.dram_tensor("out", [P, C], f32, kind="ExternalOutput")

    tabA = nc.dram_tensor("tabA", [NA, H], f32)
    tabB = nc.dram_tensor("tabB", [NB, H], f32)
    # tile-linearized write view: table row (p*(N/128) + m) <-> partition p, col block m
    tabA_v = tabA[:].rearrange("(p m) d -> p (m d)", p=P)
    tabB_v = tabB[:].rearrange("(p m) d -> p (m d)", p=P)

    with tile.TileContext(nc) as tc:
        with (
            tc.tile_pool(name="const", bufs=1) as cpool,
            tc.tile_pool(name="work", bufs=3) as wpool,
            tc.tile_pool(name="psum", bufs=4, space="PSUM") as ppool,
        ):
            w1ut_t = cpool.tile([P, H], f32)
            w1mt_t = cpool.tile([P, H], f32)
            b1rep_t = cpool.tile([P, H], f32)
            b2rep_t = cpool.tile([P, 1], f32)
            idxA_t = cpool.tile([P, C], i32)
            idxB_t = cpool.tile([P, C], i32)
            logits = cpool.tile([P, C], f32)
            nc.sync.dma_start(out=w1ut_t[:], in_=w1ut[:])
            nc.sync.dma_start(out=w1mt_t[:], in_=w1mt[:])
            nc.sync.dma_start(out=b1rep_t[:], in_=b1rep[:])
            nc.sync.dma_start(out=b2rep_t[:], in_=b2rep[:])
            nc.sync.dma_start(out=idxA_t[:], in_=idxA[:])
            nc.sync.dma_start(out=idxB_t[:], in_=idxB[:])

            # ---- precompute tables ----
            for (zT, w1t, tab_v, npad, addb1) in (
                (zTu, w1ut_t, tabA_v, NA, True),
                (zTm, w1mt_t, tabB_v, NB, False),
            ) * (repeat_pre if repeat_pre is not None else repeat):
                with tc.For_i(0, npad, ZBODY) as iv:
                    zstage = wpool.tile([P, ZBODY], f32, tag="zstage")
                    nc.sync.dma_start(out=zstage[:], in_=zT[:, bass.ds(iv, ZBODY)])
                    astage = wpool.tile([P, ZBODY], f32, tag="astage")
                    for k in range(ZBODY // P):
                        ps = ppool.tile([P, H], f32, tag="ps")
                        nc.tensor.matmul(
                            out=ps[:],
                            lhsT=zstage[:, k * P:(k + 1) * P],
                            rhs=w1t[:],
                            start=True, stop=True,
                        )
                        sl = astage[:, k * H:(k + 1) * H]
                        if addb1:
                            nc.vector.tensor_add(out=sl, in0=ps[:], in1=b1rep_t[:])
                        else:
                            nc.scalar.copy(out=sl, in_=ps[:])
                    nc.sync.dma_start(out=tab_v[:, bass.ds(iv, ZBODY)], in_=astage[:])

            # ---- edge gather + MLP ----
            def gather_body(iv):
                rstage = wpool.tile([P, G], i32, tag="rstage")
                cstage = wpool.tile([P, G], i32, tag="cstage")
                nc.vector.tensor_copy(out=rstage[:], in_=idxA_t[:, bass.ds(iv, G)])
                nc.vector.tensor_copy(out=cstage[:], in_=idxB_t[:, bass.ds(iv, G)])
                ct = wpool.tile([P, G * H], f32, tag="ct")
                for j in range(G):
                    sl = ct[:, j * H:(j + 1) * H]
                    nc.gpsimd.indirect_dma_start(
                        out=sl, out_offset=None, in_=tabA[:],
                        in_offset=bass.IndirectOffsetOnAxis(ap=rstage[:, j:j + 1], axis=0),
                    )
                    nc.gpsimd.indirect_dma_start(
                        out=sl, out_offset=None, in_=tabB[:],
                        in_offset=bass.IndirectOffsetOnAxis(ap=cstage[:, j:j + 1], axis=0),
                        compute_op=mybir.AluOpType.add,
                    )
                cc = ct[:].rearrange("p (g h) -> p g h", h=H)
                if Hp > 0:
                    nc.vector.tensor_scalar_max(out=cc[:, :, 0:Hp], in0=cc[:, :, 0:Hp], scalar1=0.0)
                if Hp < H:
                    nc.vector.tensor_scalar_min(out=cc[:, :, Hp:H], in0=cc[:, :, Hp:H], scalar1=0.0)
                lsl = logits[:, bass.ds(iv, G)]
                if Hp == H or Hp == 0:
                    nc.vector.tensor_reduce(out=lsl, in_=cc[:, :, :], axis=mybir.AxisListType.X,
                                            op=mybir.AluOpType.add)
                else:
                    pos = wpool.tile([P, G], f32, tag="pos")
                    nc.vector.tensor_reduce(out=pos[:], in_=cc[:, :, 0:Hp],
                                            axis=mybir.AxisListType.X, op=mybir.AluOpType.add)
                    neg = wpool.tile([P, G], f32, tag="neg")
                    nc.vector.tensor_reduce(out=neg[:], in_=cc[:, :, Hp:H],
                                            axis=mybir.AxisListType.X, op=mybir.AluOpType.add)
                    nc.vector.tensor_add(out=lsl, in0=pos[:], in1=neg[:])

            for _rep in range(repeat_gather if repeat_gather is not None else repeat):
                with tc.For_i(0, C, G) as iv:
                    gather_body(iv)

            # ---- sigmoid tail ----
            sig = cpool.tile([P, C], f32)
            nc.scalar.activation(out=sig[:], in_=logits[:],
                                 func=mybir.ActivationFunctionType.Sigmoid,
                                 bias=b2rep_t[:, 0:1], scale=1.0)
            nc.scalar.mul(out=sig[:], in_=sig[:], mul=5.0)
            nc.sync.dma_start(out=out_d[:], in_=sig[:])
    nc.finalize()
    return nc


def _pad_cols(n, mult):
    return ((n + mult - 1) // mult) * mult


def _prepare(z_user, z_movie, edge_index, W1, b1, W2, b2, n_cores=N_CORES):
    z_user = np.asarray(z_user, dtype=np.float32)
    z_movie = np.asarray(z_movie, dtype=np.float32)
    edge_index = np.asarray(edge_index)
    W1 = np.asarray(W1, dtype=np.float32)
    b1 = np.asarray(b1, dtype=np.float32)
    W2 = np.asarray(W2, dtype=np.float32)
    b2 = np.asarray(b2, dtype=np.float32)

    E = edge_index.shape[1]
    rows = edge_index[0].astype(np.int64)
    cols = edge_index[1].astype(np.int64)

    NAr = int(rows.max()) + 1 if E else 1          # referenced user rows
    NBr = z_movie.shape[0]
    NA = _pad_cols(max(NAr, ZBODY), ZBODY)
    NB = _pad_cols(max(NBr, ZBODY), ZBODY)

    # hidden permutation: positive-w2 units first; fold signed w2 and b1 into tables
    w2 = W2.reshape(-1)
    perm = np.argsort(w2 < 0, kind="stable")       # stable: positives (False) first
    Hp = int((w2 >= 0).sum())
    W1p = W1[perm]                                  # [H, 2H]
    b1p = b1[perm]
    scale = w2[perm]  # signed: w2*relu(x) = max0(w2*x) for w2>0, min0(w2*x) for w2<0
    w1ut = np.ascontiguousarray((W1p[:, :H] * scale[:, None]).T)   # [in, h]
    w1mt = np.ascontiguousarray((W1p[:, H:] * scale[:, None]).T)
    b1rep = np.tile(b1p * scale, (P, 1)).astype(np.float32)
    b2rep = np.full((P, 1), float(b2.reshape(-1)[0]), dtype=np.float32)

    # transposed, padded node features
    zTu = np.zeros((P, NA), dtype=np.float32)
    zTu[:, :NAr] = z_user[:NAr].T
    zTm = np.zeros((P, NB), dtype=np.float32)
    zTm[:, :NBr] = z_movie.T

    # tile-linearized table row index: u -> (u%128)*(N/128) + u//128
    mA, mB = NA // P, NB // P
    idxA_full = ((rows % P) * mA + rows // P).astype(np.int32)
    idxB_full = ((cols % P) * mB + cols // P).astype(np.int32)

    # shard edges: per core 128*C edges, C divisible by G
    C = _pad_cols(-(-E // (n_cores * P)), G)
    Epc = P * C
    Etot = n_cores * Epc
    idxA_pad = np.zeros(Etot, dtype=np.int32)
    idxA_pad[:E] = idxA_full
    idxB_pad = np.zeros(Etot, dtype=np.int32)
    idxB_pad[:E] = idxB_full

    in_maps = []
    for c in range(n_cores):
        sl = slice(c * Epc, (c + 1) * Epc)
        in_maps.append({
            "zTu": zTu, "zTm": zTm, "w1ut": w1ut, "w1mt": w1mt,
            "b1rep": b1rep, "b2rep": b2rep,
            "idxA": idxA_pad[sl].reshape(P, C),
            "idxB": idxB_pad[sl].reshape(P, C),
        })
    return in_maps, dict(C=C, NA=NA, NB=NB, Hp=Hp, E=E)


def kernel(z_user, z_movie, edge_index, W1, b1, W2, b2):
    import os
    if os.environ.get("EDGE_KERNEL_V4") == "1":  # correct but ~5x slower on HW than v3
        try:
            in_maps, meta = _prepare_v4(z_user, z_movie, edge_index, W1, b1, W2, b2)
            nc = _build_nc_v4(meta["nwin"], meta["lsp_caps"], meta["Hp"])
            res = run_bass_kernel_spmd(nc, in_maps, core_ids=list(range(N_CORES)))
            out = _unpack_v4(res, meta)
            _LAST_STATS.update(exec_time_ns=res.exec_time_ns, nc=nc,
                               in_maps=in_maps, meta=meta, version="v4")
            return out
        except Exception as e:
            import traceback
            traceback.print_exc()
            print(f"[kernel] v4 path failed ({type(e).__name__}: {e}); falling back to v3",
                  file=sys.stderr)
    if os.environ.get("EDGE_KERNEL_V7") == "1":
        try:
            in_maps, meta = _prepare_v7(z_user, z_movie, edge_index, W1, b1, W2, b2)
            nc = _build_nc_v7(meta["caps"], meta["Hp"], meta["sched"],
                              meta["naux"])
            res = run_bass_kernel_spmd(nc, in_maps, core_ids=list(range(N_CORES)))
            out = _unpack_v7(res, meta)
            _LAST_STATS.update(exec_time_ns=res.exec_time_ns, nc=nc,
                               in_maps=in_maps, meta=meta, version="v7")
            return out
        except Exception as e:
            import traceback
            traceback.print_exc()
            print(f"[kernel] v7 path failed ({type(e).__name__}: {e}); falling back",
                  file=sys.stderr)
    if os.environ.get("EDGE_KERNEL_V6", "1") == "1":
        try:
            in_maps, meta = _prepare_v6(z_user, z_movie, edge_index, W1, b1, W2, b2)
            nc = _build_nc_v6(meta["rows_u"], meta["rows_m"], meta["caps"],
                              meta["Hp"])
            res = run_bass_kernel_spmd(nc, in_maps, core_ids=list(range(N_CORES)))
            out = _unpack_v3(res, meta)
            _LAST_STATS.update(exec_time_ns=res.exec_time_ns, nc=nc,
                               in_maps=in_maps, meta=meta, version="v6")
            return out
        except Exception as e:
            import traceback
            traceback.print_exc()
            print(f"[kernel] v6 path failed ({type(e).__name__}: {e}); falling back to v3",
                  file=sys.stderr)
    if os.environ.get("EDGE_KERNEL_V1") != "1":
        try:
            in_maps, meta = _prepare_v3(z_user, z_movie, edge_index, W1, b1, W2, b2)
            nc = _build_nc_v3(meta["rows_u"], meta["rows_m"], meta["caps"], meta["Hp"])
            res = run_bass_kernel_spmd(nc, in_maps, core_ids=list(range(N_CORES)))
            out = _unpack_v3(res, meta)
            _LAST_STATS.update(exec_time_ns=res.exec_time_ns, nc=nc,
                               in_maps=in_maps, meta=meta, version="v3")
            return out
        except Exception as e:
            import traceback
            traceback.print_exc()
            print(f"[kernel] v3 path failed ({type(e).__name__}: {e}); falling back to v1",
                  file=sys.stderr)
    in_maps, meta = _prepare(z_user, z_movie, edge_index, W1, b1, W2, b2)
    nc = _build_nc(meta["C"], meta["NA"], meta["NB"], meta["Hp"])
    res = run_bass_kernel_spmd(nc, in_maps, core_ids=list(range(N_CORES)))
    out = np.concatenate([res.results[c]["out"].reshape(-1) for c in range(N_CORES)])
    _LAST_STATS.update(exec_time_ns=res.exec_time_ns, nc=nc,
                       in_maps=in_maps, meta=meta, version="v1")
    return out[:meta["E"]].astype(np.float32)


# ---------------------------------------------------------------------------
# v4: user-range sharding + movie-sorted windows; tabB streamed and expanded
# on PE via on-chip one-hot (colrep broadcast + DVE is_equal vs iota), tabA
# gathered per edge (non-transpose). Spill edges (window overflow) gather both
# tables. Tables precomputed on device in bf16 with w2/b1 folded.
# ---------------------------------------------------------------------------

WCAP = 640       # edge slots per 128-movie window (uniform across cores)
UPC = 12500      # users per core (100000 / 8)
NAC = 12800      # padded per-core tabA rows
NBP = 50176      # padded tabB rows (392 windows)


def _build_nc_v4(nwin, lsp_caps, Hp, ch=CH, repeat=1):
    """nwin: movie windows. lsp_caps: spill caps per movie-bank bucket
    (multiples of TILE). Hp: pos-w2 unit count."""
    f32 = mybir.dt.float32
    bf16 = mybir.dt.bfloat16
    i16 = mybir.dt.int16
    Lw = nwin * WCAP
    assert Lw % TILE == 0
    Lsp = int(sum(lsp_caps))
    L = Lw + Lsp
    ncols = L // P
    nblk = -(-ncols // BCOLS)
    ngrp = Lw // TILE

    nc = bacc.Bacc(None, target_bir_lowering=False,
                   dynamic_dma_scratch_size=32768)

    zTuc = nc.dram_tensor("zTuc", [P, NAC], bf16, kind="ExternalInput")
    zTmf = nc.dram_tensor("zTmf", [P, NBP], bf16, kind="ExternalInput")
    w1utF = nc.dram_tensor("w1utF", [H, H], bf16, kind="ExternalInput")
    w1mtF = nc.dram_tensor("w1mtF", [H, H], bf16, kind="ExternalInput")
    b1pre = nc.dram_tensor("b1pre", [P, GRP * H], f32, kind="ExternalInput")
    b2c = nc.dram_tensor("b2c", [P, 1], f32, kind="ExternalInput")
    iotas = nc.dram_tensor("iotas", [P, 2 * TILE], f32, kind="ExternalInput")
    ones1 = nc.dram_tensor("ones1", [1, P], bf16, kind="ExternalInput")
    colloc = nc.dram_tensor("colloc", [1, Lw], bf16, kind="ExternalInput")
    idxU = nc.dram_tensor("idxU", [P, L // 16], i16, kind="ExternalInput")
    idxMsp = nc.dram_tensor("idxMsp", [P, max(Lsp, 16) // 16], i16,
                            kind="ExternalInput")
    out_d = nc.dram_tensor("out", [nblk, P, BCOLS], f32, kind="ExternalOutput")

    tabA = nc.dram_tensor("tabA", [NAC, H], bf16)
    tabB = nc.dram_tensor("tabB", [NBP, H], bf16)
    # tabA is gather-only: store tile-linearized (row p*(NAC//P)+m <-> strip
    # node s*512+t*128+p at m = s*4+t); host linearizes gather indices.
    tabA_v = tabA[:].rearrange("(p m) h -> p (m h)", p=P)

    with tile.TileContext(nc) as tc:
        with (
            tc.tile_pool(name="const", bufs=1) as cpool,
            tc.tile_pool(name="pre", bufs=3) as prepool,
            tc.tile_pool(name="gat", bufs=2) as gpool,
            tc.tile_pool(name="idx", bufs=3) as ipool,
            tc.tile_pool(name="win", bufs=4) as wpool,
            tc.tile_pool(name="rel", bufs=4) as rpool,
            tc.tile_pool(name="lgb", bufs=2) as lpool,
            tc.tile_pool(name="obuf", bufs=2) as opool,
            tc.tile_pool(name="psum", bufs=3, space="PSUM") as ppool,
            tc.tile_pool(name="crps", bufs=2, space="PSUM") as crpool,
        ):
            nc.gpsimd.load_library(library_config.mlp)
            w1ut_t = cpool.tile([H, H], bf16)
            w1mt_t = cpool.tile([H, H], bf16)
            b1p_t = cpool.tile([P, GRP * H], f32)
            b2_t = cpool.tile([P, 1], f32)
            iota_t = cpool.tile([P, 2 * TILE], f32)
            ones_t = cpool.tile([1, P], bf16)
            nc.sync.dma_start(out=w1ut_t[:], in_=w1utF[:])
            nc.sync.dma_start(out=w1mt_t[:], in_=w1mtF[:])
            nc.sync.dma_start(out=b1p_t[:], in_=b1pre[:])
            nc.sync.dma_start(out=b2_t[:], in_=b2c[:])
            nc.sync.dma_start(out=iota_t[:], in_=iotas[:])
            nc.sync.dma_start(out=ones_t[:], in_=ones1[:])

            # ---- precompute tabA (b1 folded) and tabB ----
            for (zT, w1t, natural, npad, addb1) in (
                (zTuc, w1ut_t, False, NAC, True),
                (zTmf, w1mt_t, True, NBP, False),
            ):
                for s in range(npad // TILE):
                    zst = prepool.tile([P, TILE], bf16, tag="zst")
                    nc.sync.dma_start(out=zst[:],
                                      in_=zT[:, s * TILE:(s + 1) * TILE])
                    pps = ppool.tile([P, GRP * H], f32, tag="ps")
                    if addb1:
                        nc.scalar.copy(out=pps[:], in_=b1p_t[:])
                    for t in range(GRP):
                        nc.tensor.matmul(out=pps[:, t * H:(t + 1) * H],
                                         lhsT=zst[:, t * P:(t + 1) * P],
                                         rhs=w1t[:], start=not addb1,
                                         stop=True, skip_group_check=True)
                    ast = prepool.tile([P, GRP * H], bf16, tag="ast")
                    nc.scalar.copy(out=ast[:], in_=pps[:])
                    if natural:
                        nc.sync.dma_start(
                            out=tabB[s * TILE:(s + 1) * TILE, :].rearrange(
                                "(t p) h -> p t h", p=P),
                            in_=ast[:].rearrange("p (t h) -> p t h", h=H))
                    else:
                        nc.sync.dma_start(
                            out=tabA_v[:, s * GRP * H:(s + 1) * GRP * H],
                            in_=ast[:])

            for _rep in range(repeat):
                col = 0
                lg_blk = None

                def drain_block(c0):
                    nco = c0 % BCOLS if c0 % BCOLS else BCOLS
                    blk = (c0 - 1) // BCOLS
                    ot = opool.tile([P, BCOLS], f32, tag="ot")
                    nc.scalar.activation(
                        out=ot[:, 0:nco], in_=lg_blk[:, 0:nco],
                        func=mybir.ActivationFunctionType.Sigmoid,
                        bias=b2_t[:, 0:1], scale=1.0)
                    nc.scalar.mul(out=ot[:, 0:nco], in_=ot[:, 0:nco], mul=5.0)
                    nc.sync.dma_start(out=out_d[blk, :, 0:nco], in_=ot[:, 0:nco])

                def reduce_emit(yv, c4):
                    # yv: [P, GRP, H] bf16 view; write logits to lg_blk cols
                    if Hp > 0:
                        nc.vector.tensor_scalar_max(out=yv[:, :, 0:Hp],
                                                    in0=yv[:, :, 0:Hp],
                                                    scalar1=0.0)
                    if Hp < H:
                        nc.vector.tensor_scalar_min(out=yv[:, :, Hp:H],
                                                    in0=yv[:, :, Hp:H],
                                                    scalar1=0.0)
                    lgp = rpool.tile([P, GRP], f32, tag="lgp")
                    lgn = rpool.tile([P, GRP], f32, tag="lgn")
                    if Hp > 0:
                        nc.vector.tensor_reduce(out=lgp[:], in_=yv[:, :, 0:Hp],
                                                axis=mybir.AxisListType.X,
                                                op=mybir.AluOpType.add)
                    if Hp < H:
                        nc.vector.tensor_reduce(out=lgn[:], in_=yv[:, :, Hp:H],
                                                axis=mybir.AxisListType.X,
                                                op=mybir.AluOpType.add)
                    if Hp == H:
                        nc.vector.tensor_copy(out=lg_blk[:, c4:c4 + GRP], in_=lgp[:])
                    elif Hp == 0:
                        nc.vector.tensor_copy(out=lg_blk[:, c4:c4 + GRP], in_=lgn[:])
                    else:
                        nc.vector.tensor_add(out=lg_blk[:, c4:c4 + GRP],
                                             in0=lgp[:], in1=lgn[:])

                # ---- window region ----
                for gbase in range(0, ngrp, ch // TILE):
                    gend = min(gbase + ch // TILE, ngrp)
                    nsl = (gend - gbase) * TILE
                    s0 = gbase * TILE
                    iu_t = ipool.tile([P, ch // 16], i16, tag="iu")
                    nc.sync.dma_start(out=iu_t[:, 0:nsl // 16],
                                      in_=idxU[:, s0 // 16:(s0 + nsl) // 16])
                    at = gpool.tile([P, nsl], bf16, tag="at")
                    nc.gpsimd.dma_gather(
                        out_ap=at[:].rearrange("p (a n) -> p a n", a=nsl // P),
                        in_ap=tabA[:], idxs_ap=iu_t[:, 0:nsl // 16],
                        num_idxs=nsl, num_idxs_reg=nsl, elem_size=H,
                        transpose=False, queue_num=0, single_packet=False)
                    cl_t = ipool.tile([1, ch], bf16, tag="cl")
                    nc.sync.dma_start(out=cl_t[0:1, 0:nsl],
                                      in_=colloc[0:1, s0:s0 + nsl])
                    for g in range(gbase, gend):
                        w0 = (g * TILE) // WCAP
                        straddle = (g * TILE + TILE - 1) // WCAP > w0
                        wins = [w0, w0 + 1] if straddle and w0 + 1 < nwin else [w0]
                        go = (g - gbase) * TILE
                        crp = crpool.tile([P, TILE], f32, tag="cr")
                        nc.tensor.matmul(out=crp[:], lhsT=ones_t[:],
                                         rhs=cl_t[0:1, go:go + TILE],
                                         start=True, stop=True)
                        bps = ppool.tile([P, GRP * H], f32, tag="ps")
                        sks, tbws = [], []
                        for ki, w in enumerate(wins):
                            sk = wpool.tile([P, TILE], bf16, tag="sk")
                            nc.vector.tensor_tensor(
                                out=sk[:], in0=crp[:],
                                in1=iota_t[:, ki * TILE:(ki + 1) * TILE],
                                op=mybir.AluOpType.is_equal)
                            tbw = wpool.tile([P, H], bf16, tag="tbw")
                            nc.sync.dma_start(out=tbw[:],
                                              in_=tabB[w * P:(w + 1) * P, :])
                            sks.append(sk)
                            tbws.append(tbw)
                        for t in range(GRP):
                            for ki in range(len(wins)):
                                nc.tensor.matmul(
                                    out=bps[:, t * H:(t + 1) * H],
                                    lhsT=sks[ki][:, t * P:(t + 1) * P],
                                    rhs=tbws[ki][:],
                                    start=(ki == 0), stop=(ki == len(wins) - 1),
                                    skip_group_check=True)
                        y = rpool.tile([P, GRP * H], bf16, tag="y")
                        ab = (g - gbase) * GRP * H
                        nc.vector.tensor_add(out=y[:], in0=bps[:],
                                             in1=at[:, ab:ab + GRP * H])
                        if col % BCOLS == 0:
                            lg_blk = lpool.tile([P, BCOLS], f32, tag="lg")
                        reduce_emit(y[:].rearrange("p (g h) -> p g h", h=H),
                                    col % BCOLS)
                        col += GRP
                        if col % BCOLS == 0 or col == ncols:
                            drain_block(col)

                # ---- spill region: gather both tables ----
                for b, cap in enumerate(lsp_caps):
                    sbase = Lw + int(sum(lsp_caps[:b]))
                    pos = 0
                    while pos < cap:
                        cur = int(min(ch, cap - pos))
                        s0 = sbase + pos
                        iu_t = ipool.tile([P, ch // 16], i16, tag="iu")
                        nc.sync.dma_start(out=iu_t[:, 0:cur // 16],
                                          in_=idxU[:, s0 // 16:(s0 + cur) // 16])
                        im_t = ipool.tile([P, ch // 16], i16, tag="im")
                        nc.sync.dma_start(
                            out=im_t[:, 0:cur // 16],
                            in_=idxMsp[:, (s0 - Lw) // 16:(s0 - Lw + cur) // 16])
                        at = gpool.tile([P, cur], bf16, tag="at")
                        nc.gpsimd.dma_gather(
                            out_ap=at[:].rearrange("p (a n) -> p a n", a=cur // P),
                            in_ap=tabA[:], idxs_ap=iu_t[:, 0:cur // 16],
                            num_idxs=cur, num_idxs_reg=cur, elem_size=H,
                            transpose=False, queue_num=0, single_packet=False)
                        bt = gpool.tile([P, cur], bf16, tag="bt")
                        nc.gpsimd.dma_gather(
                            out_ap=bt[:].rearrange("p (a n) -> p a n", a=cur // P),
                            in_ap=tabB[min(b * BANK, NBP - P):min((b + 1) * BANK, NBP), :],
                            idxs_ap=im_t[:, 0:cur // 16],
                            num_idxs=cur, num_idxs_reg=cur, elem_size=H,
                            transpose=False, queue_num=0, single_packet=False)
                        for g in range(cur // TILE):
                            go = g * TILE
                            y = rpool.tile([P, GRP * H], bf16, tag="y")
                            ab = g * GRP * H
                            nc.vector.tensor_add(out=y[:],
                                                 in0=at[:, ab:ab + GRP * H],
                                                 in1=bt[:, ab:ab + GRP * H])
                            if col % BCOLS == 0:
                                lg_blk = lpool.tile([P, BCOLS], f32, tag="lg")
                            reduce_emit(y[:].rearrange("p (g h) -> p g h", h=H),
                                        col % BCOLS)
                            col += GRP
                            if col % BCOLS == 0 or col == ncols:
                                drain_block(col)
                        pos += cur
    nc.finalize()
    return nc


def _prepare_v4(z_user, z_movie, edge_index, W1, b1, W2, b2,
                n_cores=N_CORES, upc=UPC, wcap=WCAP):
    import ml_dtypes
    bf16 = ml_dtypes.bfloat16
    z_user = np.asarray(z_user, dtype=np.float32)
    z_movie = np.asarray(z_movie, dtype=np.float32)
    edge_index = np.asarray(edge_index)
    W1 = np.asarray(W1, dtype=np.float32)
    b1 = np.asarray(b1, dtype=np.float32)
    W2 = np.asarray(W2, dtype=np.float32)
    b2 = np.asarray(b2, dtype=np.float32)
    E = edge_index.shape[1]
    rows = edge_index[0].astype(np.int64)
    cols = edge_index[1].astype(np.int64)
    NM = z_movie.shape[0]
    nwin = NBP // P
    assert NM <= NBP and z_user.shape[0] <= n_cores * upc

    w2 = W2.reshape(-1)
    perm = np.argsort(w2 < 0, kind="stable")
    Hp = int((w2 >= 0).sum())
    w2sc = w2[perm]                  # signed: max0 pos-range, min0 neg-range
    W1p = W1[perm] * w2sc[:, None]
    b1p = b1[perm] * w2sc

    zmT = np.zeros((P, NBP), dtype=bf16)
    zmT[:, :NM] = z_movie.T.astype(bf16)
    shared = {"zTmf": zmT,
              "w1utF": np.ascontiguousarray(W1p[:, :H].T).astype(bf16),
              "w1mtF": np.ascontiguousarray(W1p[:, H:].T).astype(bf16),
              "b1pre": np.ascontiguousarray(np.tile(b1p, (P, GRP)).astype(np.float32)),
              "b2c": np.full((P, 1), float(b2.reshape(-1)[0]), np.float32),
              "iotas": np.ascontiguousarray(np.concatenate(
                  [np.tile(np.arange(P, dtype=np.float32)[:, None], (1, TILE)),
                   np.tile(np.arange(P, 2 * P, dtype=np.float32)[:, None], (1, TILE))],
                  axis=1)),
              "ones1": np.ones((1, P), dtype=bf16)}

    core_ids = rows // upc
    Lw = nwin * wcap
    per_core = []
    spill_cnt = np.zeros((n_cores, 2), dtype=np.int64)
    for c in range(n_cores):
        m = core_ids == c
        eids = np.nonzero(m)[0]
        r, co = rows[eids], cols[eids]
        order = np.argsort(co, kind="stable")
        eids, r, co = eids[order], r[order], co[order]
        win = co // P
        wstart = np.searchsorted(win, np.arange(nwin))
        wend = np.searchsorted(win, np.arange(nwin), side="right")
        k = np.arange(len(co)) - wstart[win]
        in_window = k < wcap
        spill_bank = (co // BANK).astype(np.int64)
        for bk in range(2):
            spill_cnt[c, bk] = int(np.count_nonzero(~in_window & (spill_bank == bk)))
        per_core.append((eids, r, co, win, k, in_window, spill_bank))

    lsp_caps = [int(_roundup(max(int(spill_cnt[:, bk].max()), 1), TILE))
                for bk in range(2)]
    Lsp = sum(lsp_caps)
    L = Lw + Lsp

    # static group->w0 for collocal encoding
    slot_arr = np.arange(Lw)
    grp_w0 = (slot_arr // TILE * TILE) // wcap     # w0 of each slot's group

    in_maps, backmaps = [], []
    for c in range(n_cores):
        eids, r, co, win, k, in_window, spill_bank = per_core[c]
        iu = np.zeros(L, np.int16)
        clv = np.full(Lw, 512.0, np.float32)
        imsp = np.zeros(max(Lsp, 16), np.int16)
        slot = np.empty(len(eids), np.int64)
        # window slots
        mA = NAC // P
        def lin(u):
            return ((u % P) * mA + u // P).astype(np.int16)
        wi = np.nonzero(in_window)[0]
        ws = win[wi] * wcap + k[wi]
        slot[wi] = ws
        iu[ws] = lin(r[wi] - c * upc)
        clv[ws] = (co[wi] - grp_w0[ws] * P).astype(np.float32)
        # spill slots
        off = 0
        for bk in range(2):
            si = np.nonzero(~in_window & (spill_bank == bk))[0]
            ss = Lw + off + np.arange(len(si))
            slot[si] = ss
            iu[ss] = lin(r[si] - c * upc)
            imsp[ss - Lw] = (co[si] % BANK).astype(np.int16)
            off += lsp_caps[bk]
        zuT = np.zeros((P, NAC), dtype=bf16)
        ncr = min((c + 1) * upc, z_user.shape[0]) - c * upc
        zuT[:, :ncr] = z_user[c * upc:c * upc + ncr].T.astype(bf16)
        iu_w = np.ascontiguousarray(np.tile(iu.reshape(L // 16, 16).T, (8, 1)))
        im_w = np.ascontiguousarray(
            np.tile(imsp.reshape(len(imsp) // 16, 16).T, (8, 1)))
        in_maps.append({**shared, "zTuc": zuT,
                        "colloc": np.ascontiguousarray(clv[None, :]).astype(bf16),
                        "idxU": iu_w, "idxMsp": im_w})
        backmaps.append((eids, slot))
    return in_maps, dict(nwin=nwin, lsp_caps=lsp_caps, L=L, E=E, Hp=Hp,
                         backmaps=backmaps)


def _unpack_v4(res, meta):
    out = np.empty(meta["E"], dtype=np.float32)
    for c, (eids, slot) in enumerate(meta["backmaps"]):
        flat = np.asarray(res.results[c]["out"], dtype=np.float32).reshape(-1)
        tc_ = slot // P
        p = slot % P
        fidx = (tc_ // BCOLS) * (P * BCOLS) + p * BCOLS + (tc_ % BCOLS)
        out[eids] = flat[fidx]
    return out



# revision 29
# speedup vs baseline: 2.2353x; 2.2353x over previous
"""EdgeDecoder Trainium2 kernel: out = 5*sigmoid(w2 . relu([z_u[row]; z_m[col]] @ W1.T + b1) + b2).

v3 strategy (8 NeuronCores, data-parallel over edges):
  No precomputed node tables. Per edge, gather the raw bf16 z_user[row] and
  z_movie[col] rows straight from HBM with batched dma_gather(transpose=True)
  (one SWDGE instruction per 2048 edges instead of one indirect DMA per 128
  edges), which lands z-components on partitions. The gathered tiles feed the
  PE as the *stationary* operand so edges land on PSUM partitions: per
  512-edge group (4 tiles of 128 edges sharing one PSUM bank), ACT preloads
  b1*w2 into the bank, 8 matmuls (4 tiles x {W1u', W1m'}) accumulate on top
  (W1 columns pre-scaled by |w2| with positive-w2 hidden units permuted
  first), one ACT relu drains the bank to SBUF, and DVE does two free-dim
  tensor_reduces (pos / neg ranges) + subtract -> per-edge logits in an SBUF
  block. Every 512 tile-columns: ACT sigmoid(+b2)*5 and one DMA out.

  dma_gather indices are int16, so node tables are split into <=32768-row
  banks and edges are bucketed by (user-bank, movie-bank) on the host; bucket
  capacities are padded to the max across cores so all 8 cores share one
  compiled program.

v1 (fallback): precomputed A/B tables + per-128-edge indirect DMA gathers.
"""
import sys
import numpy as np

sys.path.insert(0, '/opt/trn_rl_repo')

import concourse.bass as bass
import concourse.bacc as bacc
import concourse.mybir as mybir
import concourse.tile as tile
from concourse import library_config
from concourse.bass_utils import run_bass_kernel_spmd

N_CORES = 8
P = 128
H = 128          # hidden
BANK = 32768     # rows per gather bank (int16 index limit)
CH = 8192        # edges per dma_gather call
TILE = 512       # edges per PE tile (psum bank = 512 f32)
G = 32           # v1: gather-loop cols per iteration
ZBODY = 1024     # v1: precompute rows per loop body

_LAST_STATS = {}


# ---------------------------------------------------------------------------
# v3
# ---------------------------------------------------------------------------

GRP = 4          # 128-edge tiles per PSUM group (group = 512 edges = 1 bank)
BCOLS = 512      # logit-block tile-columns (block = 65536 edges)


def _build_nc_v3(rows_u, rows_m, caps, Hp, ch=CH, repeat=1):
    """rows_u/rows_m: rows per user/movie bank. caps: per-bucket edge capacity
    (each a multiple of TILE; bucket b = ubank*len(rows_m) + mbank).
    Hp: # hidden units with w2 >= 0 (after the pos-first permutation)."""
    f32 = mybir.dt.float32
    bf16 = mybir.dt.bfloat16
    i16 = mybir.dt.int16
    nbM = len(rows_m)
    L = int(sum(caps))
    ncols = L // P                   # total tile-columns
    nblk = -(-ncols // BCOLS)

    import os
    dbg_no_gather = os.environ.get("EDGE_V3_NO_GATHER") == "1"
    dbg_no_preload = os.environ.get("EDGE_V3_NO_PRELOAD") == "1"
    dbg_no_compute = os.environ.get("EDGE_V3_NO_COMPUTE") == "1"
    dbg_two_queue = os.environ.get("EDGE_V3_TWO_QUEUE") == "1"

    nc = bacc.Bacc(None, target_bir_lowering=False,
                   dynamic_dma_scratch_size=32768, num_swdge_queues=4)

    if dbg_no_gather:
        zdummy = nc.dram_tensor("zdummy", [P, ch], bf16, kind="ExternalInput")
    zu_b = [nc.dram_tensor(f"zu{i}", [r, H], bf16, kind="ExternalInput")
            for i, r in enumerate(rows_u)]
    zm_b = [nc.dram_tensor(f"zm{i}", [r, H], bf16, kind="ExternalInput")
            for i, r in enumerate(rows_m)]
    w1ut = nc.dram_tensor("w1ut", [H, H], bf16, kind="ExternalInput")
    w1mt = nc.dram_tensor("w1mt", [H, H], bf16, kind="ExternalInput")
    b1r4 = nc.dram_tensor("b1r4", [P, GRP * H], f32, kind="ExternalInput")
    b2c = nc.dram_tensor("b2c", [P, 1], f32, kind="ExternalInput")
    idxU = nc.dram_tensor("idxU", [P, L // 16], i16, kind="ExternalInput")
    idxM = nc.dram_tensor("idxM", [P, L // 16], i16, kind="ExternalInput")
    out_d = nc.dram_tensor("out", [nblk, P, BCOLS], f32, kind="ExternalOutput")

    with tile.TileContext(nc) as tc:
        with (
            tc.tile_pool(name="const", bufs=1) as cpool,
            tc.tile_pool(name="gat", bufs=4) as gpool,
            tc.tile_pool(name="idx", bufs=4) as ipool,
            tc.tile_pool(name="rel", bufs=4) as rpool,
            tc.tile_pool(name="lgb", bufs=2) as lpool,
            tc.tile_pool(name="obuf", bufs=2) as opool,
            tc.tile_pool(name="psum", bufs=4, space="PSUM") as ppool,
        ):
            nc.gpsimd.load_library(library_config.mlp)
            w1ut_t = cpool.tile([H, H], bf16)
            w1mt_t = cpool.tile([H, H], bf16)
            b1r4_t = cpool.tile([P, GRP * H], f32)
            b2_t = cpool.tile([P, 1], f32)
            nc.sync.dma_start(out=w1ut_t[:], in_=w1ut[:])
            nc.sync.dma_start(out=w1mt_t[:], in_=w1mt[:])
            nc.sync.dma_start(out=b1r4_t[:], in_=b1r4[:])
            nc.sync.dma_start(out=b2_t[:], in_=b2c[:])

            for _rep in range(repeat):
                col = 0              # global tile-column index
                lg_blk = None

                def drain_block(c0):
                    nco = c0 % BCOLS if c0 % BCOLS else BCOLS
                    blk = (c0 - 1) // BCOLS
                    ot = opool.tile([P, BCOLS], f32, tag="ot")
                    nc.scalar.activation(
                        out=ot[:, 0:nco], in_=lg_blk[:, 0:nco],
                        func=mybir.ActivationFunctionType.Sigmoid,
                        bias=b2_t[:, 0:1], scale=1.0)
                    nc.scalar.mul(out=ot[:, 0:nco], in_=ot[:, 0:nco], mul=5.0)
                    nc.sync.dma_start(out=out_d[blk, :, 0:nco], in_=ot[:, 0:nco])

                chunk_no = 0
                for b, cap in enumerate(caps):
                    bu, bm = divmod(b, nbM)
                    base = int(sum(caps[:b]))
                    pos = 0
                    while pos < cap:
                        cur = int(min(ch, cap - pos))
                        o16 = (base + pos) // 16
                        iu_t = ipool.tile([P, ch // 16], i16, tag="iu")
                        im_t = ipool.tile([P, ch // 16], i16, tag="im")
                        nc.sync.dma_start(out=iu_t[:, 0:cur // 16],
                                          in_=idxU[:, o16:o16 + cur // 16])
                        nc.sync.dma_start(out=im_t[:, 0:cur // 16],
                                          in_=idxM[:, o16:o16 + cur // 16])
                        ut = gpool.tile([P, cur], bf16, tag="ut")
                        mt = gpool.tile([P, cur], bf16, tag="mt")
                        if dbg_no_gather:
                            nc.sync.dma_start(out=ut[:], in_=zdummy[:, 0:cur])
                            nc.sync.dma_start(out=mt[:], in_=zdummy[:, 0:cur])
                        else:
                            # NOTE: transpose gathers corrupt data when run
                            # concurrently on multiple queues (shared xbar
                            # scratch) - keep both on queue 0.
                            nc.gpsimd.dma_gather(
                                out_ap=ut[:].rearrange("p (a n) -> p a n", a=1),
                                in_ap=zu_b[bu][:],
                                idxs_ap=iu_t[:, 0:cur // 16],
                                num_idxs=cur, num_idxs_reg=cur, elem_size=H,
                                transpose=True, queue_num=0,
                                single_packet=False)
                            nc.gpsimd.dma_gather(
                                out_ap=mt[:].rearrange("p (a n) -> p a n", a=1),
                                in_ap=zm_b[bm][:],
                                idxs_ap=im_t[:, 0:cur // 16],
                                num_idxs=cur, num_idxs_reg=cur, elem_size=H,
                                transpose=True, queue_num=0,
                                single_packet=False)
                        chunk_no += 1
                        for g in range(cur // TILE):
                            if col % BCOLS == 0:
                                lg_blk = lpool.tile([P, BCOLS], f32, tag="lg")
                            if dbg_no_compute:
                                col += GRP
                                continue
                            ps = ppool.tile([P, GRP * H], f32, tag="ps")
                            if not dbg_no_preload:
                                nc.scalar.copy(out=ps[:], in_=b1r4_t[:])
                            for t in range(GRP):
                                e0 = (g * GRP + t) * P
                                nc.tensor.matmul(
                                    out=ps[:, t * H:(t + 1) * H],
                                    lhsT=ut[:, e0:e0 + P], rhs=w1ut_t[:],
                                    start=dbg_no_preload, stop=False,
                                    skip_group_check=True)
                                nc.tensor.matmul(
                                    out=ps[:, t * H:(t + 1) * H],
                                    lhsT=mt[:, e0:e0 + P], rhs=w1mt_t[:],
                                    start=False, stop=True,
                                    skip_group_check=True)
                            rl = rpool.tile([P, GRP * H], bf16, tag="rl")
                            nc.scalar.activation(
                                out=rl[:], in_=ps[:],
                                func=mybir.ActivationFunctionType.Relu)
                            rv = rl[:].rearrange("p (g h) -> p g h", h=H)
                            c4 = col % BCOLS
                            if Hp == H:
                                nc.vector.tensor_reduce(
                                    out=lg_blk[:, c4:c4 + GRP], in_=rv[:, :, :],
                                    axis=mybir.AxisListType.X,
                                    op=mybir.AluOpType.add)
                            else:
                                lgp = rpool.tile([P, GRP], f32, tag="lgp")
                                lgn = rpool.tile([P, GRP], f32, tag="lgn")
                                if Hp > 0:
                                    nc.vector.tensor_reduce(
                                        out=lgp[:], in_=rv[:, :, 0:Hp],
                                        axis=mybir.AxisListType.X,
                                        op=mybir.AluOpType.add)
                                nc.vector.tensor_reduce(
                                    out=lgn[:], in_=rv[:, :, Hp:H],
                                    axis=mybir.AxisListType.X,
                                    op=mybir.AluOpType.add)
                                if Hp > 0:
                                    nc.vector.tensor_sub(
                                        out=lg_blk[:, c4:c4 + GRP],
                                        in0=lgp[:], in1=lgn[:])
                                else:
                                    nc.vector.tensor_scalar_mul(
                                        out=lg_blk[:, c4:c4 + GRP],
                                        in0=lgn[:], scalar1=-1.0)
                            col += GRP
                            if col % BCOLS == 0 or col == ncols:
                                drain_block(col)
                        pos += cur
    nc.finalize()
    return nc


def _roundup(n, m):
    return ((n + m - 1) // m) * m


def _prepare_v3(z_user, z_movie, edge_index, W1, b1, W2, b2,
                n_cores=N_CORES, bank=BANK):
    import ml_dtypes
    bf16 = ml_dtypes.bfloat16
    z_user = np.asarray(z_user, dtype=np.float32)
    z_movie = np.asarray(z_movie, dtype=np.float32)
    edge_index = np.asarray(edge_index)
    W1 = np.asarray(W1, dtype=np.float32)
    b1 = np.asarray(b1, dtype=np.float32)
    W2 = np.asarray(W2, dtype=np.float32)
    b2 = np.asarray(b2, dtype=np.float32)

    E = edge_index.shape[1]
    rows = edge_index[0].astype(np.int64)
    cols = edge_index[1].astype(np.int64)
    NU, NM = z_user.shape[0], z_movie.shape[0]
    nbU, nbM = -(-NU // bank), -(-NM // bank)
    nbkt = nbU * nbM
    Epc = -(-E // n_cores)

    per_core = []
    cnts = np.zeros((n_cores, nbkt), dtype=np.int64)
    for c in range(n_cores):
        sl = slice(c * Epc, min((c + 1) * Epc, E))
        r, co = rows[sl], cols[sl]
        bkt = (r // bank) * nbM + (co // bank)
        order = np.argsort(bkt, kind="stable")
        cnts[c] = np.bincount(bkt, minlength=nbkt)
        per_core.append((sl, order, r, co, bkt))

    caps = np.maximum(_roundup(cnts.max(axis=0), TILE), TILE)
    offs = np.concatenate([[0], np.cumsum(caps)])
    L = int(offs[-1])

    # permute hidden units w2>=0 first; fold |w2| into W1 rows and b1.
    # logit = sum_pos relu(|w2|y) - sum_neg relu(|w2|y)
    w2 = W2.reshape(-1)
    perm = np.argsort(w2 < 0, kind="stable")
    Hp = int((w2 >= 0).sum())
    w2sc = np.abs(w2[perm])
    W1p = W1[perm] * w2sc[:, None]          # [h', 2H]
    b1p = b1[perm] * w2sc                   # [h']

    zu16 = np.ascontiguousarray(z_user.astype(bf16))
    zm16 = np.ascontiguousarray(z_movie.astype(bf16))
    shared = {"w1ut": np.ascontiguousarray(W1p[:, :H].T).astype(bf16),
              "w1mt": np.ascontiguousarray(W1p[:, H:].T).astype(bf16),
              "b1r4": np.ascontiguousarray(
                  np.tile(b1p, (P, GRP)).astype(np.float32)),
              "b2c": np.full((P, 1), float(b2.reshape(-1)[0]), np.float32)}
    rows_u, rows_m = [], []
    for i in range(nbU):
        bk = np.ascontiguousarray(zu16[i * bank:(i + 1) * bank])
        shared[f"zu{i}"] = bk
        rows_u.append(bk.shape[0])
    for i in range(nbM):
        bk = np.ascontiguousarray(zm16[i * bank:(i + 1) * bank])
        shared[f"zm{i}"] = bk
        rows_m.append(bk.shape[0])

    in_maps, backmaps = [], []
    for c in range(n_cores):
        sl, order, r, co, bkt = per_core[c]
        n = len(r)
        starts = np.concatenate([[0], np.cumsum(cnts[c])])
        sorted_bkt = bkt[order]
        k = np.arange(n) - starts[sorted_bkt]
        spos = offs[sorted_bkt] + k          # slot of edge order[i]
        iu = np.zeros(L, np.int16)
        im = np.zeros(L, np.int16)
        iu[spos] = (r[order] % bank).astype(np.int16)
        im[spos] = (co[order] % bank).astype(np.int16)
        slot = np.empty(n, np.int64)
        slot[order] = spos
        iu_w = np.ascontiguousarray(np.tile(iu.reshape(L // 16, 16).T, (8, 1)))
        im_w = np.ascontiguousarray(np.tile(im.reshape(L // 16, 16).T, (8, 1)))
        in_maps.append({**shared, "idxU": iu_w, "idxM": im_w})
        backmaps.append((sl, slot))
    return in_maps, dict(rows_u=rows_u, rows_m=rows_m,
                         caps=[int(x) for x in caps], L=L, E=E, Hp=Hp,
                         backmaps=backmaps)


def _unpack_v3(res, meta):
    out = np.empty(meta["E"], dtype=np.float32)
    for c, (sl, slot) in enumerate(meta["backmaps"]):
        flat = np.asarray(res.results[c]["out"], dtype=np.float32).reshape(-1)
        # edge at stream slot s -> tile-column s//128, partition s%128;
        # out tensor is [nblk, 128, BCOLS]
        tc_ = slot // P
        p = slot % P
        fidx = (tc_ // BCOLS) * (P * BCOLS) + p * BCOLS + (tc_ % BCOLS)
        out[sl] = flat[fidx]
    return out


# ---------------------------------------------------------------------------
# v6: device-precomputed A/B node tables (A=W1u z_u + b1, B=W1m z_m, |w2|
# folded, pos-w2-first permutation) + per-edge dual NON-transpose dma_gather
# spread over all 4 SWDGE queues (measured: 1 queue = 7.9 ns/row, 4 queues =
# 1.79 ns/row). Edges land on partitions, H on free dim: DVE add, ACT relu,
# DVE pos/neg reduces -> logits. No per-edge PE work.
# Slot mapping identical to v3 (slot s -> partition s%128, tile-col s//128).
# ---------------------------------------------------------------------------

NT = 50176       # referenced node rows padded to 98*512 (indices < 50000)


def _build_nc_v6(rows_u, rows_m, caps, Hp, ch=CH, repeat=1):
    """rows_u/rows_m: rows per user/movie table bank (sum = NT each).
    caps: per-bucket edge capacity (multiples of TILE; bucket b =
    ubank*len(rows_m) + mbank). Hp: # hidden units with w2 >= 0."""
    f32 = mybir.dt.float32
    bf16 = mybir.dt.bfloat16
    i16 = mybir.dt.int16
    nbM = len(rows_m)
    L = int(sum(caps))
    ncols = L // P                   # total tile-columns
    nblk = -(-ncols // BCOLS)
    assert 0 < Hp < H

    import os
    dbg_no_gather = os.environ.get("EDGE_V6_NO_GATHER") == "1"
    dbg_no_precomp = os.environ.get("EDGE_V6_NO_PRECOMP") == "1"
    dbg_no_compute = os.environ.get("EDGE_V6_NO_COMPUTE") == "1"

    nc = bacc.Bacc(None, target_bir_lowering=False,
                   dynamic_dma_scratch_size=32768, num_swdge_queues=4)

    zuT = nc.dram_tensor("zuT", [P, NT], bf16, kind="ExternalInput")
    zmT = nc.dram_tensor("zmT", [P, NT], bf16, kind="ExternalInput")
    w1ut = nc.dram_tensor("w1ut", [H, H], bf16, kind="ExternalInput")
    w1mt = nc.dram_tensor("w1mt", [H, H], bf16, kind="ExternalInput")
    b1r4 = nc.dram_tensor("b1r4", [P, GRP * H], f32, kind="ExternalInput")
    b2c = nc.dram_tensor("b2c", [P, 1], f32, kind="ExternalInput")
    idxU = nc.dram_tensor("idxU", [P, L // 16], i16, kind="ExternalInput")
    idxM = nc.dram_tensor("idxM", [P, L // 16], i16, kind="ExternalInput")
    out_d = nc.dram_tensor("out", [nblk, P, BCOLS], f32, kind="ExternalOutput")

    tabs_u = [nc.dram_tensor(f"tabU{i}", [r, H], bf16) for i, r in enumerate(rows_u)]
    tabs_m = [nc.dram_tensor(f"tabM{i}", [r, H], bf16) for i, r in enumerate(rows_m)]

    with tile.TileContext(nc) as tc:
        with (
            tc.tile_pool(name="const", bufs=1) as cpool,
            tc.tile_pool(name="pre", bufs=2) as prepool,
            tc.tile_pool(name="gat", bufs=3) as gpool,
            tc.tile_pool(name="idx", bufs=4) as ipool,
            tc.tile_pool(name="rel", bufs=2) as rpool,
            tc.tile_pool(name="lgs", bufs=3) as spool,
            tc.tile_pool(name="lgb", bufs=2) as lpool,
            tc.tile_pool(name="obuf", bufs=2) as opool,
            tc.tile_pool(name="psum", bufs=4, space="PSUM") as ppool,
        ):
            nc.gpsimd.load_library(library_config.mlp)
            w1ut_t = cpool.tile([H, H], bf16)
            w1mt_t = cpool.tile([H, H], bf16)
            b1r4_t = cpool.tile([P, GRP * H], f32)
            b2_t = cpool.tile([P, 1], f32)
            nc.sync.dma_start(out=w1ut_t[:], in_=w1ut[:])
            nc.sync.dma_start(out=w1mt_t[:], in_=w1mt[:])
            nc.sync.dma_start(out=b1r4_t[:], in_=b1r4[:])
            nc.sync.dma_start(out=b2_t[:], in_=b2c[:])

            for _rep in range(repeat):
                # ---- precompute node tables (bank-interleaved U0,M0,U1,M1
                # so bucket (0,0) gathers can start early) ----
                gno = 0
                ZB = 4096            # z columns staged per DMA (8 groups)
                for bi in range(len(rows_u) if not dbg_no_precomp else 0):
                    for (zT, w1t, tabs, rows, addb1, goff) in (
                        (zuT, w1ut_t, tabs_u, rows_u, True, 0),
                        (zmT, w1mt_t, tabs_m, rows_m, False, 0),
                    ):
                        base = int(sum(rows[:bi]))
                        for z0 in range(0, rows[bi], ZB):
                            zn = min(ZB, rows[bi] - z0)
                            zbig = prepool.tile([P, ZB], bf16, tag="zst")
                            nc.sync.dma_start(
                                out=zbig[:, 0:zn],
                                in_=zT[:, base + z0:base + z0 + zn])
                            for s in range(zn // TILE):
                                so = z0 // TILE + s
                                pps = ppool.tile([P, GRP * H], f32, tag="ps")
                                if addb1:
                                    nc.scalar.copy(out=pps[:], in_=b1r4_t[:])
                                for t in range(GRP):
                                    nc.tensor.matmul(
                                        out=pps[:, t * H:(t + 1) * H],
                                        lhsT=zbig[:, s * TILE + t * P:
                                                  s * TILE + (t + 1) * P],
                                        rhs=w1t[:], start=not addb1,
                                        stop=True, skip_group_check=True)
                                ast = prepool.tile([P, GRP * H], bf16,
                                                   tag="ast")
                                if gno % 2 == 0:
                                    nc.scalar.copy(out=ast[:], in_=pps[:])
                                else:
                                    nc.vector.tensor_copy(out=ast[:],
                                                          in_=pps[:])
                                gno += 1
                                nc.sync.dma_start(
                                    out=tabs[bi][so * TILE:(so + 1) * TILE, :]
                                    .rearrange("(t p) h -> p t h", p=P),
                                    in_=ast[:].rearrange("p (t h) -> p t h",
                                                         h=H))

                # ---- edge phase ----
                col = 0              # global tile-column index
                lg_blk = None

                def drain_block(c0):
                    nco = c0 % BCOLS if c0 % BCOLS else BCOLS
                    blk = (c0 - 1) // BCOLS
                    ot = opool.tile([P, BCOLS], f32, tag="ot")
                    nc.scalar.activation(
                        out=ot[:, 0:nco], in_=lg_blk[:, 0:nco],
                        func=mybir.ActivationFunctionType.Sigmoid,
                        bias=b2_t[:, 0:1], scale=1.0)
                    nc.scalar.mul(out=ot[:, 0:nco], in_=ot[:, 0:nco], mul=5.0)
                    nc.sync.dma_start(out=out_d[blk, :, 0:nco], in_=ot[:, 0:nco])

                chunk_no = 0
                for b, cap in enumerate(caps):
                    bu, bm = divmod(b, nbM)
                    base = int(sum(caps[:b]))
                    pos = 0
                    while pos < cap:
                        cur = int(min(ch, cap - pos))
                        o16 = (base + pos) // 16
                        na = cur // P        # tile-cols in this chunk
                        iu_t = ipool.tile([P, ch // 16], i16, tag="iu")
                        im_t = ipool.tile([P, ch // 16], i16, tag="im")
                        nc.sync.dma_start(out=iu_t[:, 0:cur // 16],
                                          in_=idxU[:, o16:o16 + cur // 16])
                        nc.sync.dma_start(out=im_t[:, 0:cur // 16],
                                          in_=idxM[:, o16:o16 + cur // 16])
                        ut = gpool.tile([P, ch], bf16, tag="ut")
                        mt = gpool.tile([P, ch], bf16, tag="mt")
                        if not dbg_no_gather:
                            nc.gpsimd.dma_gather(
                                out_ap=ut[:, 0:cur].rearrange(
                                    "p (a n) -> p a n", a=na),
                                in_ap=tabs_u[bu][:],
                                idxs_ap=iu_t[:, 0:cur // 16],
                                num_idxs=cur, num_idxs_reg=cur, elem_size=H,
                                transpose=False,
                                queue_num=(2 * chunk_no) % 4,
                                single_packet=False)
                            nc.gpsimd.dma_gather(
                                out_ap=mt[:, 0:cur].rearrange(
                                    "p (a n) -> p a n", a=na),
                                in_ap=tabs_m[bm][:],
                                idxs_ap=im_t[:, 0:cur // 16],
                                num_idxs=cur, num_idxs_reg=cur, elem_size=H,
                                transpose=False,
                                queue_num=(2 * chunk_no + 1) % 4,
                                single_packet=False)
                        chunk_no += 1
                        if dbg_no_compute:
                            col += na
                            if col % BCOLS == 0 or col >= ncols:
                                pass
                            pos += cur
                            continue
                        yt = rpool.tile([P, ch], bf16, tag="yt")
                        nc.vector.tensor_add(out=yt[:, 0:cur], in0=ut[:, 0:cur],
                                             in1=mt[:, 0:cur])
                        nc.scalar.activation(
                            out=yt[:, 0:cur], in_=yt[:, 0:cur],
                            func=mybir.ActivationFunctionType.Relu)
                        rv = yt[:, 0:cur].rearrange("p (a h) -> p a h", h=H)
                        # pos/neg reduces -> logits, split at block boundaries
                        a0 = 0
                        while a0 < na:
                            if col % BCOLS == 0:
                                lg_blk = lpool.tile([P, BCOLS], f32, tag="lg")
                            c4 = col % BCOLS
                            seg = int(min(na - a0, BCOLS - c4))
                            lgp = spool.tile([P, ch // P], f32, tag="lgp")
                            lgn = spool.tile([P, ch // P], f32, tag="lgn")
                            nc.vector.tensor_reduce(
                                out=lgp[:, 0:seg], in_=rv[:, a0:a0 + seg, 0:Hp],
                                axis=mybir.AxisListType.X,
                                op=mybir.AluOpType.add)
                            nc.vector.tensor_reduce(
                                out=lgn[:, 0:seg], in_=rv[:, a0:a0 + seg, Hp:H],
                                axis=mybir.AxisListType.X,
                                op=mybir.AluOpType.add)
                            nc.vector.tensor_sub(
                                out=lg_blk[:, c4:c4 + seg],
                                in0=lgp[:, 0:seg], in1=lgn[:, 0:seg])
                            col += seg
                            a0 += seg
                            if col % BCOLS == 0 or col == ncols:
                                drain_block(col)
                        pos += cur
    nc.finalize()
    return nc


def _prepare_v6(z_user, z_movie, edge_index, W1, b1, W2, b2,
                n_cores=N_CORES, bank=BANK):
    import ml_dtypes
    bf16 = ml_dtypes.bfloat16
    z_user = np.asarray(z_user, dtype=np.float32)
    z_movie = np.asarray(z_movie, dtype=np.float32)
    edge_index = np.asarray(edge_index)
    W1 = np.asarray(W1, dtype=np.float32)
    b1 = np.asarray(b1, dtype=np.float32)
    W2 = np.asarray(W2, dtype=np.float32)
    b2 = np.asarray(b2, dtype=np.float32)

    E = edge_index.shape[1]
    rows = edge_index[0].astype(np.int64)
    cols = edge_index[1].astype(np.int64)
    if E and (rows.max() >= NT or cols.max() >= NT):
        raise ValueError("edge index out of v6 table range")
    nbU = nbM = -(-NT // bank)
    nbkt = nbU * nbM
    Epc = -(-E // n_cores)

    per_core = []
    cnts = np.zeros((n_cores, nbkt), dtype=np.int64)
    for c in range(n_cores):
        sl = slice(c * Epc, min((c + 1) * Epc, E))
        r, co = rows[sl], cols[sl]
        bkt = (r // bank) * nbM + (co // bank)
        order = np.argsort(bkt, kind="stable")
        cnts[c] = np.bincount(bkt, minlength=nbkt)
        per_core.append((sl, order, r, co, bkt))

    caps = np.maximum(_roundup(cnts.max(axis=0), TILE), TILE)
    offs = np.concatenate([[0], np.cumsum(caps)])
    L = int(offs[-1])

    # permute hidden units w2>=0 first; fold |w2| into W1 rows and b1.
    w2 = W2.reshape(-1)
    perm = np.argsort(w2 < 0, kind="stable")
    Hp = int((w2 >= 0).sum())
    w2sc = np.abs(w2[perm])
    W1p = W1[perm] * w2sc[:, None]          # [h', 2H]
    b1p = b1[perm] * w2sc                   # [h']

    nuse_u = min(z_user.shape[0], NT)
    nuse_m = min(z_movie.shape[0], NT)
    zuT = np.zeros((P, NT), dtype=bf16)
    zuT[:, :nuse_u] = z_user[:nuse_u].T.astype(bf16)
    zmT = np.zeros((P, NT), dtype=bf16)
    zmT[:, :nuse_m] = z_movie[:nuse_m].T.astype(bf16)
    shared = {"zuT": zuT, "zmT": zmT,
              "w1ut": np.ascontiguousarray(W1p[:, :H].T).astype(bf16),
              "w1mt": np.ascontiguousarray(W1p[:, H:].T).astype(bf16),
              "b1r4": np.ascontiguousarray(
                  np.tile(b1p, (P, GRP)).astype(np.float32)),
              "b2c": np.full((P, 1), float(b2.reshape(-1)[0]), np.float32)}
    rows_u = [min(bank, NT - i * bank) for i in range(nbU)]
    rows_m = [min(bank, NT - i * bank) for i in range(nbM)]

    in_maps, backmaps = [], []
    for c in range(n_cores):
        sl, order, r, co, bkt = per_core[c]
        n = len(r)
        starts = np.concatenate([[0], np.cumsum(cnts[c])])
        sorted_bkt = bkt[order]
        k = np.arange(n) - starts[sorted_bkt]
        spos = offs[sorted_bkt] + k          # slot of edge order[i]
        iu = np.zeros(L, np.int16)
        im = np.zeros(L, np.int16)
        iu[spos] = (r[order] % bank).astype(np.int16)
        im[spos] = (co[order] % bank).astype(np.int16)
        slot = np.empty(n, np.int64)
        slot[order] = spos
        iu_w = np.ascontiguousarray(np.tile(iu.reshape(L // 16, 16).T, (8, 1)))
        im_w = np.ascontiguousarray(np.tile(im.reshape(L // 16, 16).T, (8, 1)))
        in_maps.append({**shared, "idxU": iu_w, "idxM": im_w})
        backmaps.append((sl, slot))
    return in_maps, dict(rows_u=rows_u, rows_m=rows_m,
                         caps=[int(x) for x in caps], L=L, E=E, Hp=Hp,
                         backmaps=backmaps)


# ---------------------------------------------------------------------------
# v7: user-range sharding. A-side (user) via PE one-hot expansion: edges
# sorted by (movie-bank, local user window); host streams bf16 one-hot masks
# (index-derived only); window tiles of the per-core A table feed PE as rhs.
# B-side (movie) via non-transpose dma_gather over all 4 SWDGE queues.
# Tables precomputed on device (A per-core slice w/ b1+|w2| fold; B full).
# Slot mapping identical to v3/v6 (slot s -> partition s%128, col s//128).
# ---------------------------------------------------------------------------

UPC7 = 6272      # users per core (50176/8); window = 128 users, 49/core
UPC7P = 6656     # padded to 13*512 for the 512-row precompute groups


def _v7_schedule(caps):
    """caps: [2][49] window slot capacities (each mult of 16; run totals mult
    of 512). Returns (sched, naux): sched = per 512-slot group the list of
    window ids (global: mb*49 + w); naux = total aux mask tiles."""
    nwin = len(caps[0])
    sched = []
    base = 0
    for mb in range(2):
        run = int(sum(caps[mb]))
        assert run % 512 == 0
        starts = np.concatenate([[0], np.cumsum(caps[mb])])
        for g0 in range(run // 512):
            lo, hi = g0 * 512, (g0 + 1) * 512
            w_lo = int(np.searchsorted(starts, lo, side="right") - 1)
            w_hi = int(np.searchsorted(starts, hi - 1, side="right") - 1)
            sched.append([mb * nwin + w for w in range(w_lo, w_hi + 1)])
        base += run
    naux = sum(len(ws) - 1 for ws in sched)
    return sched, naux


def _build_nc_v7(caps, Hp, sched, naux, ch=CH, repeat=1):
    """caps: [2][nwin] window capacities. sched/naux: from _v7_schedule."""
    f32 = mybir.dt.float32
    bf16 = mybir.dt.bfloat16
    i16 = mybir.dt.int16
    nwin = len(caps[0])
    run_len = [int(sum(caps[mb])) for mb in range(2)]
    L = sum(run_len)
    ncols = L // P
    nblk = -(-ncols // BCOLS)
    NTU = nwin * P               # per-core A rows (6272)
    rows_m = [BANK, NT - BANK]
    assert 0 < Hp < H and L % 512 == 0

    import os
    dbg_no_gather = os.environ.get("EDGE_V7_NO_GATHER") == "1"
    dbg_no_onehot = os.environ.get("EDGE_V7_NO_ONEHOT") == "1"
    dbg_no_mask0 = os.environ.get("EDGE_V7_NO_MASK0") == "1"
    dbg_no_compute = os.environ.get("EDGE_V7_NO_COMPUTE") == "1"
    dbg_no_precomp = os.environ.get("EDGE_V7_NO_PRECOMP") == "1"

    nc = bacc.Bacc(None, target_bir_lowering=False,
                   dynamic_dma_scratch_size=32768, num_swdge_queues=4)

    zuTc = nc.dram_tensor("zuTc", [P, UPC7P], bf16, kind="ExternalInput")
    zmT = nc.dram_tensor("zmT", [P, NT], bf16, kind="ExternalInput")
    w1ut = nc.dram_tensor("w1ut", [H, H], bf16, kind="ExternalInput")
    w1mt = nc.dram_tensor("w1mt", [H, H], bf16, kind="ExternalInput")
    b1r4 = nc.dram_tensor("b1r4", [P, GRP * H], f32, kind="ExternalInput")
    b2c = nc.dram_tensor("b2c", [P, 1], f32, kind="ExternalInput")
    idxM = nc.dram_tensor("idxM", [P, L // 16], i16, kind="ExternalInput")
    mask0 = nc.dram_tensor("mask0", [P, L], bf16, kind="ExternalInput")
    maskx = nc.dram_tensor("maskx", [P, max(naux, 1) * 512], bf16,
                           kind="ExternalInput")
    out_d = nc.dram_tensor("out", [nblk, P, BCOLS], f32, kind="ExternalOutput")

    tabU = nc.dram_tensor("tabU", [UPC7P, H], bf16)
    tabs_m = [nc.dram_tensor(f"tabM{i}", [r, H], bf16)
              for i, r in enumerate(rows_m)]

    with tile.TileContext(nc) as tc:
        with (
            tc.tile_pool(name="const", bufs=1) as cpool,
            tc.tile_pool(name="pre", bufs=2) as prepool,
            tc.tile_pool(name="gat", bufs=4) as gpool,
            tc.tile_pool(name="msk", bufs=2) as mpool,
            tc.tile_pool(name="idx", bufs=4) as ipool,
            tc.tile_pool(name="win", bufs=4) as wpool,
            tc.tile_pool(name="aux", bufs=3) as xpool,
            tc.tile_pool(name="rel", bufs=2) as rpool,
            tc.tile_pool(name="lgs", bufs=4) as spool,
            tc.tile_pool(name="lgb", bufs=2) as lpool,
            tc.tile_pool(name="obuf", bufs=2) as opool,
            tc.tile_pool(name="psum", bufs=4, space="PSUM") as ppool,
        ):
            nc.gpsimd.load_library(library_config.mlp)
            w1ut_t = cpool.tile([H, H], bf16)
            w1mt_t = cpool.tile([H, H], bf16)
            b1r4_t = cpool.tile([P, GRP * H], f32)
            b2_t = cpool.tile([P, 1], f32)
            nc.sync.dma_start(out=w1ut_t[:], in_=w1ut[:])
            nc.sync.dma_start(out=w1mt_t[:], in_=w1mt[:])
            nc.sync.dma_start(out=b1r4_t[:], in_=b1r4[:])
            nc.sync.dma_start(out=b2_t[:], in_=b2c[:])

            for _rep in range(repeat):
                # ---- precompute: tabM bank0, tabU slice, tabM bank1 ----
                gno = 0

                ZB = 4096            # z columns staged per DMA (8 groups)

                def pre_groups(zT, w1t, tab, zoff, n512, addb1):
                    nonlocal gno
                    for z0 in range(0, n512 * TILE, ZB):
                        zn = min(ZB, n512 * TILE - z0)
                        zbig = prepool.tile([P, ZB], bf16, tag="zst")
                        nc.sync.dma_start(
                            out=zbig[:, 0:zn],
                            in_=zT[:, zoff + z0:zoff + z0 + zn])
                        for s in range(zn // TILE):
                            so = z0 // TILE + s
                            pps = ppool.tile([P, GRP * H], f32, tag="ps")
                            if addb1:
                                nc.scalar.copy(out=pps[:], in_=b1r4_t[:])
                            for t in range(GRP):
                                nc.tensor.matmul(
                                    out=pps[:, t * H:(t + 1) * H],
                                    lhsT=zbig[:, s * TILE + t * P:
                                              s * TILE + (t + 1) * P],
                                    rhs=w1t[:], start=not addb1,
                                    stop=True, skip_group_check=True)
                            ast = prepool.tile([P, GRP * H], bf16, tag="ast")
                            if gno % 2 == 0:
                                nc.scalar.copy(out=ast[:], in_=pps[:])
                            else:
                                nc.vector.tensor_copy(out=ast[:], in_=pps[:])
                            gno += 1
                            nc.sync.dma_start(
                                out=tab[so * TILE:(so + 1) * TILE, :]
                                .rearrange("(t p) h -> p t h", p=P),
                                in_=ast[:].rearrange("p (t h) -> p t h", h=H))

                if not dbg_no_precomp:
                    pre_groups(zmT, w1mt_t, tabs_m[0], 0, BANK // TILE, False)
                    pre_groups(zuTc, w1ut_t, tabU, 0, UPC7P // TILE, True)
                    pre_groups(zmT, w1mt_t, tabs_m[1], BANK,
                               (NT - BANK) // TILE, False)

                # ---- edge phase ----
                col = 0
                lg_blk = None
                aux_no = 0
                g_global = 0
                wt_cache = {}            # window id -> (handle, load_ordinal)
                wt_loads = 0

                def get_window(w):
                    nonlocal wt_loads
                    ent = wt_cache.get(w)
                    if ent is not None and wt_loads - ent[1] < 4:
                        return ent[0]
                    wt = wpool.tile([P, H], bf16, tag="wt")
                    r0 = (w % nwin) * P
                    nc.sync.dma_start(out=wt[:], in_=tabU[r0:r0 + P, :])
                    wt_cache[w] = (wt, wt_loads)
                    wt_loads += 1
                    return wt

                def drain_block(c0):
                    nco = c0 % BCOLS if c0 % BCOLS else BCOLS
                    blk = (c0 - 1) // BCOLS
                    ot = opool.tile([P, BCOLS], f32, tag="ot")
                    nc.scalar.activation(
                        out=ot[:, 0:nco], in_=lg_blk[:, 0:nco],
                        func=mybir.ActivationFunctionType.Sigmoid,
                        bias=b2_t[:, 0:1], scale=1.0)
                    nc.scalar.mul(out=ot[:, 0:nco], in_=ot[:, 0:nco], mul=5.0)
                    nc.sync.dma_start(out=out_d[blk, :, 0:nco], in_=ot[:, 0:nco])

                chunk_no = 0
                for mb in range(2):
                    base = sum(run_len[:mb])
                    cap = run_len[mb]
                    pos = 0
                    while pos < cap:
                        cur = int(min(ch, cap - pos))
                        s0 = base + pos
                        im_t = ipool.tile([P, ch // 16], i16, tag="im")
                        nc.sync.dma_start(
                            out=im_t[:, 0:cur // 16],
                            in_=idxM[:, s0 // 16:(s0 + cur) // 16])
                        bt = gpool.tile([P, ch], bf16, tag="bt")
                        if not dbg_no_gather:
                            nc.gpsimd.dma_gather(
                                out_ap=bt[:, 0:cur].rearrange(
                                    "p (a n) -> p a n", a=cur // P),
                                in_ap=tabs_m[mb][:],
                                idxs_ap=im_t[:, 0:cur // 16],
                                num_idxs=cur, num_idxs_reg=cur, elem_size=H,
                                transpose=False, queue_num=chunk_no % 4,
                                single_packet=False)
                        chunk_no += 1
                        mk0 = mpool.tile([P, ch], bf16, tag="mk0")
                        if not dbg_no_mask0:
                            nc.sync.dma_start(out=mk0[:, 0:cur],
                                              in_=mask0[:, s0:s0 + cur])
                        if dbg_no_compute:
                            g_global += cur // 512
                            col += cur // P
                            pos += cur
                            continue
                        yt = rpool.tile([P, ch], bf16, tag="yt")
                        for gi in range(cur // 512):
                            if dbg_no_onehot:
                                g_global += 1
                                nc.vector.tensor_copy(
                                    out=yt[:, gi * GRP * H:(gi + 1) * GRP * H],
                                    in_=bt[:, gi * GRP * H:(gi + 1) * GRP * H])
                                continue
                            wins = sched[g_global]
                            ps = ppool.tile([P, GRP * H], f32, tag="eps")
                            parts = []
                            for ki, w in enumerate(wins):
                                if ki == 0:
                                    mk_t, moff = mk0, gi * 512
                                else:
                                    mk_t = xpool.tile([P, 512], bf16, tag="mx")
                                    nc.sync.dma_start(
                                        out=mk_t[:],
                                        in_=maskx[:, aux_no * 512:
                                                  (aux_no + 1) * 512])
                                    moff = 0
                                    aux_no += 1
                                parts.append((mk_t, moff, get_window(w)))
                            # per PSUM segment, open and close the PE
                            # accumulation group before moving on (groups
                            # must not interleave across segments)
                            for t in range(GRP):
                                for ki, (mk_t, moff, wt) in enumerate(parts):
                                    nc.tensor.matmul(
                                        out=ps[:, t * H:(t + 1) * H],
                                        lhsT=mk_t[:, moff + t * P:
                                                  moff + (t + 1) * P],
                                        rhs=wt[:],
                                        start=(ki == 0),
                                        stop=(ki == len(parts) - 1),
                                        skip_group_check=True)
                            g_global += 1
                            nc.vector.tensor_add(
                                out=yt[:, gi * GRP * H:(gi + 1) * GRP * H],
                                in0=ps[:],
                                in1=bt[:, gi * GRP * H:(gi + 1) * GRP * H])
                        nc.scalar.activation(
                            out=yt[:, 0:cur], in_=yt[:, 0:cur],
                            func=mybir.ActivationFunctionType.Relu)
                        rv = yt[:, 0:cur].rearrange("p (a h) -> p a h", h=H)
                        na = cur // P
                        a0 = 0
                        while a0 < na:
                            if col % BCOLS == 0:
                                lg_blk = lpool.tile([P, BCOLS], f32, tag="lg")
                            c4 = col % BCOLS
                            seg = int(min(na - a0, BCOLS - c4))
                            lgp = spool.tile([P, ch // P], f32, tag="lgp")
                            lgn = spool.tile([P, ch // P], f32, tag="lgn")
                            nc.vector.tensor_reduce(
                                out=lgp[:, 0:seg], in_=rv[:, a0:a0 + seg, 0:Hp],
                                axis=mybir.AxisListType.X,
                                op=mybir.AluOpType.add)
                            nc.vector.tensor_reduce(
                                out=lgn[:, 0:seg], in_=rv[:, a0:a0 + seg, Hp:H],
                                axis=mybir.AxisListType.X,
                                op=mybir.AluOpType.add)
                            nc.vector.tensor_sub(
                                out=lg_blk[:, c4:c4 + seg],
                                in0=lgp[:, 0:seg], in1=lgn[:, 0:seg])
                            col += seg
                            a0 += seg
                            if col % BCOLS == 0 or col == ncols:
                                drain_block(col)
                        pos += cur
    nc.finalize()
    return nc


def _prepare_v7(z_user, z_movie, edge_index, W1, b1, W2, b2,
                n_cores=N_CORES):
    import ml_dtypes
    bf16 = ml_dtypes.bfloat16
    z_user = np.asarray(z_user, dtype=np.float32)
    z_movie = np.asarray(z_movie, dtype=np.float32)
    edge_index = np.asarray(edge_index)
    W1 = np.asarray(W1, dtype=np.float32)
    b1 = np.asarray(b1, dtype=np.float32)
    W2 = np.asarray(W2, dtype=np.float32)
    b2 = np.asarray(b2, dtype=np.float32)

    E = edge_index.shape[1]
    rows = edge_index[0].astype(np.int64)
    cols = edge_index[1].astype(np.int64)
    if E and (rows.max() >= NT or cols.max() >= NT):
        raise ValueError("edge index out of v7 table range")
    nwin = UPC7 // P

    # per-core split (by user range), then by movie bank, then by window
    core_of = rows // UPC7
    per_core = []
    wcnt = np.zeros((n_cores, 2, nwin), dtype=np.int64)
    for c in range(n_cores):
        eids = np.nonzero(core_of == c)[0]
        r, co = rows[eids], cols[eids]
        u = r - c * UPC7
        mb = co // BANK
        w = u // P
        order = np.lexsort((w, mb))
        eids, u, co, mb, w = eids[order], u[order], co[order], mb[order], w[order]
        for b in range(2):
            wcnt[c, b] = np.bincount(w[mb == b], minlength=nwin)
        per_core.append((eids, u, co, mb, w))

    # shared window capacities: max over cores, round to 16; run mult of 512
    caps = np.maximum(_roundup(wcnt.max(axis=0), 16), 16)
    for b in range(2):
        tot = int(caps[b].sum())
        caps[b][-1] += _roundup(tot, 512) - tot
    run_len = [int(caps[b].sum()) for b in range(2)]
    L = sum(run_len)
    starts = np.zeros((2, nwin), dtype=np.int64)
    for b in range(2):
        starts[b] = sum(run_len[:b]) + np.concatenate(
            [[0], np.cumsum(caps[b])[:-1]])

    sched, naux = _v7_schedule([list(map(int, caps[0])),
                                list(map(int, caps[1]))])
    # aux ordinal lookup: (group, window) -> ordinal for non-first windows
    aux_of = {}
    k = 0
    for g, ws in enumerate(sched):
        for wi in ws[1:]:
            aux_of[(g, wi)] = k
            k += 1
    assert k == naux

    w2v = W2.reshape(-1)
    perm = np.argsort(w2v < 0, kind="stable")
    Hp = int((w2v >= 0).sum())
    w2sc = np.abs(w2v[perm])
    W1p = W1[perm] * w2sc[:, None]
    b1p = b1[perm] * w2sc

    nuse_m = min(z_movie.shape[0], NT)
    zmT = np.zeros((P, NT), dtype=bf16)
    zmT[:, :nuse_m] = z_movie[:nuse_m].T.astype(bf16)
    shared = {"zmT": zmT,
              "w1ut": np.ascontiguousarray(W1p[:, :H].T).astype(bf16),
              "w1mt": np.ascontiguousarray(W1p[:, H:].T).astype(bf16),
              "b1r4": np.ascontiguousarray(
                  np.tile(b1p, (P, GRP)).astype(np.float32)),
              "b2c": np.full((P, 1), float(b2.reshape(-1)[0]), np.float32)}

    sched_w0 = np.array([ws[0] for ws in sched], dtype=np.int64)
    in_maps, backmaps = [], []
    for c in range(n_cores):
        eids, u, co, mb, w = per_core[c]
        # slot: within-window rank
        wk = mb * nwin + w
        ordr = np.argsort(wk, kind="stable")   # already sorted; rank within
        kk = np.arange(len(u)) - np.concatenate(
            [[0], np.cumsum(np.bincount(wk, minlength=2 * nwin))])[wk]
        slot = starts[mb, w] + kk
        g = slot // 512
        wg = mb * nwin + w                     # global window id of each edge
        is_first = wg == sched_w0[g]
        urow = (u % P).astype(np.int64)
        m0 = np.zeros((P, L), dtype=bf16)
        m0[urow[is_first], slot[is_first]] = 1
        mx = np.zeros((P, max(naux, 1) * 512), dtype=bf16)
        nf = np.nonzero(~is_first)[0]
        if len(nf):
            aux_idx = np.array([aux_of[(int(g[i]), int(wg[i]))] for i in nf],
                               dtype=np.int64)
            mx[urow[nf], aux_idx * 512 + (slot[nf] % 512)] = 1
        im = np.zeros(L, np.int16)
        im[slot] = (co % BANK).astype(np.int16)
        im_w = np.ascontiguousarray(np.tile(im.reshape(L // 16, 16).T, (8, 1)))
        zuTc = np.zeros((P, UPC7P), dtype=bf16)
        lo = c * UPC7
        hi = min((c + 1) * UPC7, z_user.shape[0])
        if hi > lo:
            zuTc[:, :hi - lo] = z_user[lo:hi].T.astype(bf16)
        in_maps.append({**shared, "zuTc": zuTc, "idxM": im_w,
                        "mask0": m0, "maskx": mx})
        backmaps.append((eids, slot))
    caps_py = [list(map(int, caps[0])), list(map(int, caps[1]))]
    return in_maps, dict(caps=caps_py, sched=sched, naux=naux, L=L, E=E,
                         Hp=Hp, backmaps=backmaps)


def _unpack_v7(res, meta):
    out = np.empty(meta["E"], dtype=np.float32)
    for c, (eids, slot) in enumerate(meta["backmaps"]):
        flat = np.asarray(res.results[c]["out"], dtype=np.float32).reshape(-1)
        tc_ = slot // P
        p = slot % P
        fidx = (tc_ // BCOLS) * (P * BCOLS) + p * BCOLS + (tc_ % BCOLS)
        out[eids] = flat[fidx]
    return out


# ---------------------------------------------------------------------------
# v1 (fallback): precomputed tables + per-column indirect DMA gathers
# ---------------------------------------------------------------------------

def _build_nc(C, NA, NB, Hp, repeat=1, repeat_pre=None, repeat_gather=None):
    """C: edge cols per core (edges = 128*C). NA/NB: padded table rows. Hp: # pos-w2 units.
    repeat>1 re-runs the compute phases (identical results) for slope-based timing."""
    f32 = mybir.dt.float32
    i32 = mybir.dt.int32
    nc = bacc.Bacc(None, target_bir_lowering=False)

    zTu = nc.dram_tensor("zTu", [P, NA], f32, kind="ExternalInput")
    zTm = nc.dram_tensor("zTm", [P, NB], f32, kind="ExternalInput")
    w1ut = nc.dram_tensor("w1ut", [P, H], f32, kind="ExternalInput")
    w1mt = nc.dram_tensor("w1mt", [P, H], f32, kind="ExternalInput")
    b1rep = nc.dram_tensor("b1rep", [P, H], f32, kind="ExternalInput")
    b2rep = nc.dram_tensor("b2rep", [P, 1], f32, kind="ExternalInput")
    idxA = nc.dram_tensor("idxA", [P, C], i32, kind="ExternalInput")
    idxB = nc.dram_tensor("idxB", [P, C], i32, kind="ExternalInput")
    out_d = nc.dram_tensor("out", [P, C], f32, kind="ExternalOutput")

    tabA = nc.dram_tensor("tabA", [NA, H], f32)
    tabB = nc.dram_tensor("tabB", [NB, H], f32)
    # tile-linearized write view: table row (p*(N/128) + m) <-> partition p, col block m
    tabA_v = tabA[:].rearrange("(p m) d -> p (m d)", p=P)
    tabB_v = tabB[:].rearrange("(p m) d -> p (m d)", p=P)

    with tile.TileContext(nc) as tc:
        with (
            tc.tile_pool(name="const", bufs=1) as cpool,
            tc.tile_pool(name="work", bufs=3) as wpool,
            tc.tile_pool(name="psum", bufs=4, space="PSUM") as ppool,
        ):
            w1ut_t = cpool.tile([P, H], f32)
            w1mt_t = cpool.tile([P, H], f32)
            b1rep_t = cpool.tile([P, H], f32)
            b2rep_t = cpool.tile([P, 1], f32)
            idxA_t = cpool.tile([P, C], i32)
            idxB_t = cpool.tile([P, C], i32)
            logits = cpool.tile([P, C], f32)
            nc.sync.dma_start(out=w1ut_t[:], in_=w1ut[:])
            nc.sync.dma_start(out=w1mt_t[:], in_=w1mt[:])
            nc.sync.dma_start(out=b1rep_t[:], in_=b1rep[:])
            nc.sync.dma_start(out=b2rep_t[:], in_=b2rep[:])
            nc.sync.dma_start(out=idxA_t[:], in_=idxA[:])
            nc.sync.dma_start(out=idxB_t[:], in_=idxB[:])

            # ---- precompute tables ----
            for (zT, w1t, tab_v, npad, addb1) in (
                (zTu, w1ut_t, tabA_v, NA, True),
                (zTm, w1mt_t, tabB_v, NB, False),
            ) * (repeat_pre if repeat_pre is not None else repeat):
                with tc.For_i(0, npad, ZBODY) as iv:
                    zstage = wpool.tile([P, ZBODY], f32, tag="zstage")
                    nc.sync.dma_start(out=zstage[:], in_=zT[:, bass.ds(iv, ZBODY)])
                    astage = wpool.tile([P, ZBODY], f32, tag="astage")
                    for k in range(ZBODY // P):
                        ps = ppool.tile([P, H], f32, tag="ps")
                        nc.tensor.matmul(
                            out=ps[:],
                            lhsT=zstage[:, k * P:(k + 1) * P],
                            rhs=w1t[:],
                            start=True, stop=True,
                        )
                        sl = astage[:, k * H:(k + 1) * H]
                        if addb1:
                            nc.vector.tensor_add(out=sl, in0=ps[:], in1=b1rep_t[:])
                        else:
                            nc.scalar.copy(out=sl, in_=ps[:])
                    nc.sync.dma_start(out=tab_v[:, bass.ds(iv, ZBODY)], in_=astage[:])

            # ---- edge gather + MLP ----
            def gather_body(iv):
                rstage = wpool.tile([P, G], i32, tag="rstage")
                cstage = wpool.tile([P, G], i32, tag="cstage")
                nc.vector.tensor_copy(out=rstage[:], in_=idxA_t[:, bass.ds(iv, G)])
                nc.vector.tensor_copy(out=cstage[:], in_=idxB_t[:, bass.ds(iv, G)])
                ct = wpool.tile([P, G * H], f32, tag="ct")
                for j in range(G):
                    sl = ct[:, j * H:(j + 1) * H]
                    nc.gpsimd.indirect_dma_start(
                        out=sl, out_offset=None, in_=tabA[:],
                        in_offset=bass.IndirectOffsetOnAxis(ap=rstage[:, j:j + 1], axis=0),
                    )
                    nc.gpsimd.indirect_dma_start(
                        out=sl, out_offset=None, in_=tabB[:],
                        in_offset=bass.IndirectOffsetOnAxis(ap=cstage[:, j:j + 1], axis=0),
                        compute_op=mybir.AluOpType.add,
                    )
                cc = ct[:].rearrange("p (g h) -> p g h", h=H)
                if Hp > 0:
                    nc.vector.tensor_scalar_max(out=cc[:, :, 0:Hp], in0=cc[:, :, 0:Hp], scalar1=0.0)
                if Hp < H:
                    nc.vector.tensor_scalar_min(out=cc[:, :, Hp:H], in0=cc[:, :, Hp:H], scalar1=0.0)
                lsl = logits[:, bass.ds(iv, G)]
                if Hp == H or Hp == 0:
                    nc.vector.tensor_reduce(out=lsl, in_=cc[:, :, :], axis=mybir.AxisListType.X,
                                            op=mybir.AluOpType.add)
                else:
                    pos = wpool.tile([P, G], f32, tag="pos")
                    nc.vector.tensor_reduce(out=pos[:], in_=cc[:, :, 0:Hp],
                                            axis=mybir.AxisListType.X, op=mybir.AluOpType.add)
                    neg = wpool.tile([P, G], f32, tag="neg")
                    nc.vector.tensor_reduce(out=neg[:], in_=cc[:, :, Hp:H],
                                            axis=mybir.AxisListType.X, op=mybir.AluOpType.add)
                    nc.vector.tensor_add(out=lsl, in0=pos[:], in1=neg[:])

            for _rep in range(repeat_gather if repeat_gather is not None else repeat):
                with tc.For_i(0, C, G) as iv:
                    gather_body(iv)

            # ---- sigmoid tail ----
            sig = cpool.tile([P, C], f32)
            nc.scalar.activation(out=sig[:], in_=logits[:],
                                 func=mybir.ActivationFunctionType.Sigmoid,
                                 bias=b2rep_t[:, 0:1], scale=1.0)
            nc.scalar.mul(out=sig[:], in_=sig[:], mul=5.0)
            nc.sync.dma_start(out=out_d[:], in_=sig[:])
    nc.finalize()
    return nc


def _pad_cols(n, mult):
    return ((n + mult - 1) // mult) * mult


def _prepare(z_user, z_movie, edge_index, W1, b1, W2, b2, n_cores=N_CORES):
    z_user = np.asarray(z_user, dtype=np.float32)
    z_movie = np.asarray(z_movie, dtype=np.float32)
    edge_index = np.asarray(edge_index)
    W1 = np.asarray(W1, dtype=np.float32)
    b1 = np.asarray(b1, dtype=np.float32)
    W2 = np.asarray(W2, dtype=np.float32)
    b2 = np.asarray(b2, dtype=np.float32)

    E = edge_index.shape[1]
    rows = edge_index[0].astype(np.int64)
    cols = edge_index[1].astype(np.int64)

    NAr = int(rows.max()) + 1 if E else 1          # referenced user rows
    NBr = z_movie.shape[0]
    NA = _pad_cols(max(NAr, ZBODY), ZBODY)
    NB = _pad_cols(max(NBr, ZBODY), ZBODY)

    # hidden permutation: positive-w2 units first; fold signed w2 and b1 into tables
    w2 = W2.reshape(-1)
    perm = np.argsort(w2 < 0, kind="stable")       # stable: positives (False) first
    Hp = int((w2 >= 0).sum())
    W1p = W1[perm]                                  # [H, 2H]
    b1p = b1[perm]
    scale = w2[perm]  # signed: w2*relu(x) = max0(w2*x) for w2>0, min0(w2*x) for w2<0
    w1ut = np.ascontiguousarray((W1p[:, :H] * scale[:, None]).T)   # [in, h]
    w1mt = np.ascontiguousarray((W1p[:, H:] * scale[:, None]).T)
    b1rep = np.tile(b1p * scale, (P, 1)).astype(np.float32)
    b2rep = np.full((P, 1), float(b2.reshape(-1)[0]), dtype=np.float32)

    # transposed, padded node features
    zTu = np.zeros((P, NA), dtype=np.float32)
    zTu[:, :NAr] = z_user[:NAr].T
    zTm = np.zeros((P, NB), dtype=np.float32)
    zTm[:, :NBr] = z_movie.T

    # tile-linearized table row index: u -> (u%128)*(N/128) + u//128
    mA, mB = NA // P, NB // P
    idxA_full = ((rows % P) * mA + rows // P).astype(np.int32)
    idxB_full = ((cols % P) * mB + cols // P).astype(np.int32)

    # shard edges: per core 128*C edges, C divisible by G
    C = _pad_cols(-(-E // (n_cores * P)), G)
    Epc = P * C
    Etot = n_cores * Epc
    idxA_pad = np.zeros(Etot, dtype=np.int32)
    idxA_pad[:E] = idxA_full
    idxB_pad = np.zeros(Etot, dtype=np.int32)
    idxB_pad[:E] = idxB_full

    in_maps = []
    for c in range(n_cores):
        sl = slice(c * Epc, (c + 1) * Epc)
        in_maps.append({
            "zTu": zTu, "zTm": zTm, "w1ut": w1ut, "w1mt": w1mt,
            "b1rep": b1rep, "b2rep": b2rep,
            "idxA": idxA_pad[sl].reshape(P, C),
            "idxB": idxB_pad[sl].reshape(P, C),
        })
    return in_maps, dict(C=C, NA=NA, NB=NB, Hp=Hp, E=E)


def kernel(z_user, z_movie, edge_index, W1, b1, W2, b2):
    import os
    if os.environ.get("EDGE_KERNEL_V4") == "1":  # correct but ~5x slower on HW than v3
        try:
            in_maps, meta = _prepare_v4(z_user, z_movie, edge_index, W1, b1, W2, b2)
            nc = _build_nc_v4(meta["nwin"], meta["lsp_caps"], meta["Hp"])
            res = run_bass_kernel_spmd(nc, in_maps, core_ids=list(range(N_CORES)))
            out = _unpack_v4(res, meta)
            _LAST_STATS.update(exec_time_ns=res.exec_time_ns, nc=nc,
                               in_maps=in_maps, meta=meta, version="v4")
            return out
        except Exception as e:
            import traceback
            traceback.print_exc()
            print(f"[kernel] v4 path failed ({type(e).__name__}: {e}); falling back to v3",
                  file=sys.stderr)
    if os.environ.get("EDGE_KERNEL_V7") == "1":
        try:
            in_maps, meta = _prepare_v7(z_user, z_movie, edge_index, W1, b1, W2, b2)
            nc = _build_nc_v7(meta["caps"], meta["Hp"], meta["sched"],
                              meta["naux"])
            res = run_bass_kernel_spmd(nc, in_maps, core_ids=list(range(N_CORES)))
            out = _unpack_v7(res, meta)
            _LAST_STATS.update(exec_time_ns=res.exec_time_ns, nc=nc,
                               in_maps=in_maps, meta=meta, version="v7")
            return out
        except Exception as e:
            import traceback
            traceback.print_exc()
            print(f"[kernel] v7 path failed ({type(e).__name__}: {e}); falling back",
                  file=sys.stderr)
    if os.environ.get("EDGE_KERNEL_V6", "1") == "1":
        try:
            in_maps, meta = _prepare_v6(z_user, z_movie, edge_index, W1, b1, W2, b2)
            nc = _build_nc_v6(meta["rows_u"], meta["rows_m"], meta["caps"],
                              meta["Hp"])
            res = run_bass_kernel_spmd(nc, in_maps, core_ids=list(range(N_CORES)))
            out = _unpack_v3(res, meta)
            _LAST_STATS.update(exec_time_ns=res.exec_time_ns, nc=nc,
                               in_maps=in_maps, meta=meta, version="v6")
            return out
        except Exception as e:
            import traceback
            traceback.print_exc()
            print(f"[kernel] v6 path failed ({type(e).__name__}: {e}); falling back to v3",
                  file=sys.stderr)
    if os.environ.get("EDGE_KERNEL_V1") != "1":
        try:
            in_maps, meta = _prepare_v3(z_user, z_movie, edge_index, W1, b1, W2, b2)
            nc = _build_nc_v3(meta["rows_u"], meta["rows_m"], meta["caps"], meta["Hp"])
            res = run_bass_kernel_spmd(nc, in_maps, core_ids=list(range(N_CORES)))
            out = _unpack_v3(res, meta)
            _LAST_STATS.update(exec_time_ns=res.exec_time_ns, nc=nc,
                               in_maps=in_maps, meta=meta, version="v3")
            return out
        except Exception as e:
            import traceback
            traceback.print_exc()
            print(f"[kernel] v3 path failed ({type(e).__name__}: {e}); falling back to v1",
                  file=sys.stderr)
    in_maps, meta = _prepare(z_user, z_movie, edge_index, W1, b1, W2, b2)
    nc = _build_nc(meta["C"], meta["NA"], meta["NB"], meta["Hp"])
    res = run_bass_kernel_spmd(nc, in_maps, core_ids=list(range(N_CORES)))
    out = np.concatenate([res.results[c]["out"].reshape(-1) for c in range(N_CORES)])
    _LAST_STATS.update(exec_time_ns=res.exec_time_ns, nc=nc,
                       in_maps=in_maps, meta=meta, version="v1")
    return out[:meta["E"]].astype(np.float32)


# ---------------------------------------------------------------------------
# v4: user-range sharding + movie-sorted windows; tabB streamed and expanded
# on PE via on-chip one-hot (colrep broadcast + DVE is_equal vs iota), tabA
# gathered per edge (non-transpose). Spill edges (window overflow) gather both
# tables. Tables precomputed on device in bf16 with w2/b1 folded.
# ---------------------------------------------------------------------------

WCAP = 640       # edge slots per 128-movie window (uniform across cores)
UPC = 12500      # users per core (100000 / 8)
NAC = 12800      # padded per-core tabA rows
NBP = 50176      # padded tabB rows (392 windows)


def _build_nc_v4(nwin, lsp_caps, Hp, ch=CH, repeat=1):
    """nwin: movie windows. lsp_caps: spill caps per movie-bank bucket
    (multiples of TILE). Hp: pos-w2 unit count."""
    f32 = mybir.dt.float32
    bf16 = mybir.dt.bfloat16
    i16 = mybir.dt.int16
    Lw = nwin * WCAP
    assert Lw % TILE == 0
    Lsp = int(sum(lsp_caps))
    L = Lw + Lsp
    ncols = L // P
    nblk = -(-ncols // BCOLS)
    ngrp = Lw // TILE

    nc = bacc.Bacc(None, target_bir_lowering=False,
                   dynamic_dma_scratch_size=32768)

    zTuc = nc.dram_tensor("zTuc", [P, NAC], bf16, kind="ExternalInput")
    zTmf = nc.dram_tensor("zTmf", [P, NBP], bf16, kind="ExternalInput")
    w1utF = nc.dram_tensor("w1utF", [H, H], bf16, kind="ExternalInput")
    w1mtF = nc.dram_tensor("w1mtF", [H, H], bf16, kind="ExternalInput")
    b1pre = nc.dram_tensor("b1pre", [P, GRP * H], f32, kind="ExternalInput")
    b2c = nc.dram_tensor("b2c", [P, 1], f32, kind="ExternalInput")
    iotas = nc.dram_tensor("iotas", [P, 2 * TILE], f32, kind="ExternalInput")
    ones1 = nc.dram_tensor("ones1", [1, P], bf16, kind="ExternalInput")
    colloc = nc.dram_tensor("colloc", [1, Lw], bf16, kind="ExternalInput")
    idxU = nc.dram_tensor("idxU", [P, L // 16], i16, kind="ExternalInput")
    idxMsp = nc.dram_tensor("idxMsp", [P, max(Lsp, 16) // 16], i16,
                            kind="ExternalInput")
    out_d = nc.dram_tensor("out", [nblk, P, BCOLS], f32, kind="ExternalOutput")

    tabA = nc.dram_tensor("tabA", [NAC, H], bf16)
    tabB = nc.dram_tensor("tabB", [NBP, H], bf16)
    # tabA is gather-only: store tile-linearized (row p*(NAC//P)+m <-> strip
    # node s*512+t*128+p at m = s*4+t); host linearizes gather indices.
    tabA_v = tabA[:].rearrange("(p m) h -> p (m h)", p=P)

    with tile.TileContext(nc) as tc:
        with (
            tc.tile_pool(name="const", bufs=1) as cpool,
            tc.tile_pool(name="pre", bufs=3) as prepool,
            tc.tile_pool(name="gat", bufs=2) as gpool,
            tc.tile_pool(name="idx", bufs=3) as ipool,
            tc.tile_pool(name="win", bufs=4) as wpool,
            tc.tile_pool(name="rel", bufs=4) as rpool,
            tc.tile_pool(name="lgb", bufs=2) as lpool,
            tc.tile_pool(name="obuf", bufs=2) as opool,
            tc.tile_pool(name="psum", bufs=3, space="PSUM") as ppool,
            tc.tile_pool(name="crps", bufs=2, space="PSUM") as crpool,
        ):
            nc.gpsimd.load_library(library_config.mlp)
            w1ut_t = cpool.tile([H, H], bf16)
            w1mt_t = cpool.tile([H, H], bf16)
            b1p_t = cpool.tile([P, GRP * H], f32)
            b2_t = cpool.tile([P, 1], f32)
            iota_t = cpool.tile([P, 2 * TILE], f32)
            ones_t = cpool.tile([1, P], bf16)
            nc.sync.dma_start(out=w1ut_t[:], in_=w1utF[:])
            nc.sync.dma_start(out=w1mt_t[:], in_=w1mtF[:])
            nc.sync.dma_start(out=b1p_t[:], in_=b1pre[:])
            nc.sync.dma_start(out=b2_t[:], in_=b2c[:])
            nc.sync.dma_start(out=iota_t[:], in_=iotas[:])
            nc.sync.dma_start(out=ones_t[:], in_=ones1[:])

            # ---- precompute tabA (b1 folded) and tabB ----
            for (zT, w1t, natural, npad, addb1) in (
                (zTuc, w1ut_t, False, NAC, True),
                (zTmf, w1mt_t, True, NBP, False),
            ):
                for s in range(npad // TILE):
                    zst = prepool.tile([P, TILE], bf16, tag="zst")
                    nc.sync.dma_start(out=zst[:],
                                      in_=zT[:, s * TILE:(s + 1) * TILE])
                    pps = ppool.tile([P, GRP * H], f32, tag="ps")
                    if addb1:
                        nc.scalar.copy(out=pps[:], in_=b1p_t[:])
                    for t in range(GRP):
                        nc.tensor.matmul(out=pps[:, t * H:(t + 1) * H],
                                         lhsT=zst[:, t * P:(t + 1) * P],
                                         rhs=w1t[:], start=not addb1,
                                         stop=True, skip_group_check=True)
                    ast = prepool.tile([P, GRP * H], bf16, tag="ast")
                    nc.scalar.copy(out=ast[:], in_=pps[:])
                    if natural:
                        nc.sync.dma_start(
                            out=tabB[s * TILE:(s + 1) * TILE, :].rearrange(
                                "(t p) h -> p t h", p=P),
                            in_=ast[:].rearrange("p (t h) -> p t h", h=H))
                    else:
                        nc.sync.dma_start(
                            out=tabA_v[:, s * GRP * H:(s + 1) * GRP * H],
                            in_=ast[:])

            for _rep in range(repeat):
                col = 0
                lg_blk = None

                def drain_block(c0):
                    nco = c0 % BCOLS if c0 % BCOLS else BCOLS
                    blk = (c0 - 1) // BCOLS
                    ot = opool.tile([P, BCOLS], f32, tag="ot")
                    nc.scalar.activation(
                        out=ot[:, 0:nco], in_=lg_blk[:, 0:nco],
                        func=mybir.ActivationFunctionType.Sigmoid,
                        bias=b2_t[:, 0:1], scale=1.0)
                    nc.scalar.mul(out=ot[:, 0:nco], in_=ot[:, 0:nco], mul=5.0)
                    nc.sync.dma_start(out=out_d[blk, :, 0:nco], in_=ot[:, 0:nco])

                def reduce_emit(yv, c4):
                    # yv: [P, GRP, H] bf16 view; write logits to lg_blk cols
                    if Hp > 0:
                        nc.vector.tensor_scalar_max(out=yv[:, :, 0:Hp],
                                                    in0=yv[:, :, 0:Hp],
                                                    scalar1=0.0)
                    if Hp < H:
                        nc.vector.tensor_scalar_min(out=yv[:, :, Hp:H],
                                                    in0=yv[:, :, Hp:H],
                                                    scalar1=0.0)
                    lgp = rpool.tile([P, GRP], f32, tag="lgp")
                    lgn = rpool.tile([P, GRP], f32, tag="lgn")
                    if Hp > 0:
                        nc.vector.tensor_reduce(out=lgp[:], in_=yv[:, :, 0:Hp],
                                                axis=mybir.AxisListType.X,
                                                op=mybir.AluOpType.add)
                    if Hp < H:
                        nc.vector.tensor_reduce(out=lgn[:], in_=yv[:, :, Hp:H],
                                                axis=mybir.AxisListType.X,
                                                op=mybir.AluOpType.add)
                    if Hp == H:
                        nc.vector.tensor_copy(out=lg_blk[:, c4:c4 + GRP], in_=lgp[:])
                    elif Hp == 0:
                        nc.vector.tensor_copy(out=lg_blk[:, c4:c4 + GRP], in_=lgn[:])
                    else:
                        nc.vector.tensor_add(out=lg_blk[:, c4:c4 + GRP],
                                             in0=lgp[:], in1=lgn[:])

                # ---- window region ----
                for gbase in range(0, ngrp, ch // TILE):
                    gend = min(gbase + ch // TILE, ngrp)
                    nsl = (gend - gbase) * TILE
                    s0 = gbase * TILE
                    iu_t = ipool.tile([P, ch // 16], i16, tag="iu")
                    nc.sync.dma_start(out=iu_t[:, 0:nsl // 16],
                                      in_=idxU[:, s0 // 16:(s0 + nsl) // 16])
                    at = gpool.tile([P, nsl], bf16, tag="at")
                    nc.gpsimd.dma_gather(
                        out_ap=at[:].rearrange("p (a n) -> p a n", a=nsl // P),
                        in_ap=tabA[:], idxs_ap=iu_t[:, 0:nsl // 16],
                        num_idxs=nsl, num_idxs_reg=nsl, elem_size=H,
                        transpose=False, queue_num=0, single_packet=False)
                    cl_t = ipool.tile([1, ch], bf16, tag="cl")
                    nc.sync.dma_start(out=cl_t[0:1, 0:nsl],
                                      in_=colloc[0:1, s0:s0 + nsl])
                    for g in range(gbase, gend):
                        w0 = (g * TILE) // WCAP
                        straddle = (g * TILE + TILE - 1) // WCAP > w0
                        wins = [w0, w0 + 1] if straddle and w0 + 1 < nwin else [w0]
                        go = (g - gbase) * TILE
                        crp = crpool.tile([P, TILE], f32, tag="cr")
                        nc.tensor.matmul(out=crp[:], lhsT=ones_t[:],
                                         rhs=cl_t[0:1, go:go + TILE],
                                         start=True, stop=True)
                        bps = ppool.tile([P, GRP * H], f32, tag="ps")
                        sks, tbws = [], []
                        for ki, w in enumerate(wins):
                            sk = wpool.tile([P, TILE], bf16, tag="sk")
                            nc.vector.tensor_tensor(
                                out=sk[:], in0=crp[:],
                                in1=iota_t[:, ki * TILE:(ki + 1) * TILE],
                                op=mybir.AluOpType.is_equal)
                            tbw = wpool.tile([P, H], bf16, tag="tbw")
                            nc.sync.dma_start(out=tbw[:],
                                              in_=tabB[w * P:(w + 1) * P, :])
                            sks.append(sk)
                            tbws.append(tbw)
                        for t in range(GRP):
                            for ki in range(len(wins)):
                                nc.tensor.matmul(
                                    out=bps[:, t * H:(t + 1) * H],
                                    lhsT=sks[ki][:, t * P:(t + 1) * P],
                                    rhs=tbws[ki][:],
                                    start=(ki == 0), stop=(ki == len(wins) - 1),
                                    skip_group_check=True)
                        y = rpool.tile([P, GRP * H], bf16, tag="y")
                        ab = (g - gbase) * GRP * H
                        nc.vector.tensor_add(out=y[:], in0=bps[:],
                                             in1=at[:, ab:ab + GRP * H])
                        if col % BCOLS == 0:
                            lg_blk = lpool.tile([P, BCOLS], f32, tag="lg")
                        reduce_emit(y[:].rearrange("p (g h) -> p g h", h=H),
                                    col % BCOLS)
                        col += GRP
                        if col % BCOLS == 0 or col == ncols:
                            drain_block(col)

                # ---- spill region: gather both tables ----
                for b, cap in enumerate(lsp_caps):
                    sbase = Lw + int(sum(lsp_caps[:b]))
                    pos = 0
                    while pos < cap:
                        cur = int(min(ch, cap - pos))
                        s0 = sbase + pos
                        iu_t = ipool.tile([P, ch // 16], i16, tag="iu")
                        nc.sync.dma_start(out=iu_t[:, 0:cur // 16],
                                          in_=idxU[:, s0 // 16:(s0 + cur) // 16])
                        im_t = ipool.tile([P, ch // 16], i16, tag="im")
                        nc.sync.dma_start(
                            out=im_t[:, 0:cur // 16],
                            in_=idxMsp[:, (s0 - Lw) // 16:(s0 - Lw + cur) // 16])
                        at = gpool.tile([P, cur], bf16, tag="at")
                        nc.gpsimd.dma_gather(
                            out_ap=at[:].rearrange("p (a n) -> p a n", a=cur // P),
                            in_ap=tabA[:], idxs_ap=iu_t[:, 0:cur // 16],
                            num_idxs=cur, num_idxs_reg=cur, elem_size=H,
                            transpose=False, queue_num=0, single_packet=False)
                        bt = gpool.tile([P, cur], bf16, tag="bt")
                        nc.gpsimd.dma_gather(
                            out_ap=bt[:].rearrange("p (a n) -> p a n", a=cur // P),
                            in_ap=tabB[min(b * BANK, NBP - P):min((b + 1) * BANK, NBP), :],
                            idxs_ap=im_t[:, 0:cur // 16],
                            num_idxs=cur, num_idxs_reg=cur, elem_size=H,
                            transpose=False, queue_num=0, single_packet=False)
                        for g in range(cur // TILE):
                            go = g * TILE
                            y = rpool.tile([P, GRP * H], bf16, tag="y")
                            ab = g * GRP * H
                            nc.vector.tensor_add(out=y[:],
                                                 in0=at[:, ab:ab + GRP * H],
                                                 in1=bt[:, ab:ab + GRP * H])
                            if col % BCOLS == 0:
                                lg_blk = lpool.tile([P, BCOLS], f32, tag="lg")
                            reduce_emit(y[:].rearrange("p (g h) -> p g h", h=H),
                                        col % BCOLS)
                            col += GRP
                            if col % BCOLS == 0 or col == ncols:
                                drain_block(col)
                        pos += cur
    nc.finalize()
    return nc


def _prepare_v4(z_user, z_movie, edge_index, W1, b1, W2, b2,
                n_cores=N_CORES, upc=UPC, wcap=WCAP):
    import ml_dtypes
    bf16 = ml_dtypes.bfloat16
    z_user = np.asarray(z_user, dtype=np.float32)
    z_movie = np.asarray(z_movie, dtype=np.float32)
    edge_index = np.asarray(edge_index)
    W1 = np.asarray(W1, dtype=np.float32)
    b1 = np.asarray(b1, dtype=np.float32)
    W2 = np.asarray(W2, dtype=np.float32)
    b2 = np.asarray(b2, dtype=np.float32)
    E = edge_index.shape[1]
    rows = edge_index[0].astype(np.int64)
    cols = edge_index[1].astype(np.int64)
    NM = z_movie.shape[0]
    nwin = NBP // P
    assert NM <= NBP and z_user.shape[0] <= n_cores * upc

    w2 = W2.reshape(-1)
    perm = np.argsort(w2 < 0, kind="stable")
    Hp = int((w2 >= 0).sum())
    w2sc = w2[perm]                  # signed: max0 pos-range, min0 neg-range
    W1p = W1[perm] * w2sc[:, None]
    b1p = b1[perm] * w2sc

    zmT = np.zeros((P, NBP), dtype=bf16)
    zmT[:, :NM] = z_movie.T.astype(bf16)
    shared = {"zTmf": zmT,
              "w1utF": np.ascontiguousarray(W1p[:, :H].T).astype(bf16),
              "w1mtF": np.ascontiguousarray(W1p[:, H:].T).astype(bf16),
              "b1pre": np.ascontiguousarray(np.tile(b1p, (P, GRP)).astype(np.float32)),
              "b2c": np.full((P, 1), float(b2.reshape(-1)[0]), np.float32),
              "iotas": np.ascontiguousarray(np.concatenate(
                  [np.tile(np.arange(P, dtype=np.float32)[:, None], (1, TILE)),
                   np.tile(np.arange(P, 2 * P, dtype=np.float32)[:, None], (1, TILE))],
                  axis=1)),
              "ones1": np.ones((1, P), dtype=bf16)}

    core_ids = rows // upc
    Lw = nwin * wcap
    per_core = []
    spill_cnt = np.zeros((n_cores, 2), dtype=np.int64)
    for c in range(n_cores):
        m = core_ids == c
        eids = np.nonzero(m)[0]
        r, co = rows[eids], cols[eids]
        order = np.argsort(co, kind="stable")
        eids, r, co = eids[order], r[order], co[order]
        win = co // P
        wstart = np.searchsorted(win, np.arange(nwin))
        wend = np.searchsorted(win, np.arange(nwin), side="right")
        k = np.arange(len(co)) - wstart[win]
        in_window = k < wcap
        spill_bank = (co // BANK).astype(np.int64)
        for bk in range(2):
            spill_cnt[c, bk] = int(np.count_nonzero(~in_window & (spill_bank == bk)))
        per_core.append((eids, r, co, win, k, in_window, spill_bank))

    lsp_caps = [int(_roundup(max(int(spill_cnt[:, bk].max()), 1), TILE))
                for bk in range(2)]
    Lsp = sum(lsp_caps)
    L = Lw + Lsp

    # static group->w0 for collocal encoding
    slot_arr = np.arange(Lw)
    grp_w0 = (slot_arr // TILE * TILE) // wcap     # w0 of each slot's group

    in_maps, backmaps = [], []
    for c in range(n_cores):
        eids, r, co, win, k, in_window, spill_bank = per_core[c]
        iu = np.zeros(L, np.int16)
        clv = np.full(Lw, 512.0, np.float32)
        imsp = np.zeros(max(Lsp, 16), np.int16)
        slot = np.empty(len(eids), np.int64)
        # window slots
        mA = NAC // P
        def lin(u):
            return ((u % P) * mA + u // P).astype(np.int16)
        wi = np.nonzero(in_window)[0]
        ws = win[wi] * wcap + k[wi]
        slot[wi] = ws
        iu[ws] = lin(r[wi] - c * upc)
        clv[ws] = (co[wi] - grp_w0[ws] * P).astype(np.float32)
        # spill slots
        off = 0
        for bk in range(2):
            si = np.nonzero(~in_window & (spill_bank == bk))[0]
            ss = Lw + off + np.arange(len(si))
            slot[si] = ss
            iu[ss] = lin(r[si] - c * upc)
            imsp[ss - Lw] = (co[si] % BANK).astype(np.int16)
            off += lsp_caps[bk]
        zuT = np.zeros((P, NAC), dtype=bf16)
        ncr = min((c + 1) * upc, z_user.shape[0]) - c * upc
        zuT[:, :ncr] = z_user[c * upc:c * upc + ncr].T.astype(bf16)
        iu_w = np.ascontiguousarray(np.tile(iu.reshape(L // 16, 16).T, (8, 1)))
        im_w = np.ascontiguousarray(
            np.tile(imsp.reshape(len(imsp) // 16, 16).T, (8, 1)))
        in_maps.append({**shared, "zTuc": zuT,
                        "colloc": np.ascontiguousarray(clv[None, :]).astype(bf16),
                        "idxU": iu_w, "idxMsp": im_w})
        backmaps.append((eids, slot))
    return in_maps, dict(nwin=nwin, lsp_caps=lsp_caps, L=L, E=E, Hp=Hp,
                         backmaps=backmaps)


def _unpack_v4(res, meta):
    out = np.empty(meta["E"], dtype=np.float32)
    for c, (eids, slot) in enumerate(meta["backmaps"]):
        flat = np.asarray(res.results[c]["out"], dtype=np.float32).reshape(-1)
        tc_ = slot // P
        p = slot % P
        fidx = (tc_ // BCOLS) * (P * BCOLS) + p * BCOLS + (tc_ % BCOLS)
        out[eids] = flat[fidx]
    return out



# revision 33
# speedup vs baseline: 3.1211x; 1.3963x over previous
"""EdgeDecoder Trainium2 kernel: out = 5*sigmoid(w2 . relu([z_u[row]; z_m[col]] @ W1.T + b1) + b2).

v3 strategy (8 NeuronCores, data-parallel over edges):
  No precomputed node tables. Per edge, gather the raw bf16 z_user[row] and
  z_movie[col] rows straight from HBM with batched dma_gather(transpose=True)
  (one SWDGE instruction per 2048 edges instead of one indirect DMA per 128
  edges), which lands z-components on partitions. The gathered tiles feed the
  PE as the *stationary* operand so edges land on PSUM partitions: per
  512-edge group (4 tiles of 128 edges sharing one PSUM bank), ACT preloads
  b1*w2 into the bank, 8 matmuls (4 tiles x {W1u', W1m'}) accumulate on top
  (W1 columns pre-scaled by |w2| with positive-w2 hidden units permuted
  first), one ACT relu drains the bank to SBUF, and DVE does two free-dim
  tensor_reduces (pos / neg ranges) + subtract -> per-edge logits in an SBUF
  block. Every 512 tile-columns: ACT sigmoid(+b2)*5 and one DMA out.

  dma_gather indices are int16, so node tables are split into <=32768-row
  banks and edges are bucketed by (user-bank, movie-bank) on the host; bucket
  capacities are padded to the max across cores so all 8 cores share one
  compiled program.

v1 (fallback): precomputed A/B tables + per-128-edge indirect DMA gathers.
"""
import sys
import numpy as np

sys.path.insert(0, '/opt/trn_rl_repo')

import concourse.bass as bass
import concourse.bacc as bacc
import concourse.mybir as mybir
import concourse.tile as tile
from concourse import library_config
from concourse.bass_utils import run_bass_kernel_spmd

N_CORES = 8
P = 128
H = 128          # hidden
BANK = 32768     # rows per gather bank (int16 index limit)
CH = 8192        # edges per dma_gather call
TILE = 512       # edges per PE tile (psum bank = 512 f32)
G = 32           # v1: gather-loop cols per iteration
ZBODY = 1024     # v1: precompute rows per loop body

_LAST_STATS = {}


# ---------------------------------------------------------------------------
# v3
# ---------------------------------------------------------------------------

GRP = 4          # 128-edge tiles per PSUM group (group = 512 edges = 1 bank)
BCOLS = 512      # logit-block tile-columns (block = 65536 edges)


def _build_nc_v3(rows_u, rows_m, caps, Hp, ch=CH, repeat=1):
    """rows_u/rows_m: rows per user/movie bank. caps: per-bucket edge capacity
    (each a multiple of TILE; bucket b = ubank*len(rows_m) + mbank).
    Hp: # hidden units with w2 >= 0 (after the pos-first permutation)."""
    f32 = mybir.dt.float32
    bf16 = mybir.dt.bfloat16
    i16 = mybir.dt.int16
    nbM = len(rows_m)
    L = int(sum(caps))
    ncols = L // P                   # total tile-columns
    nblk = -(-ncols // BCOLS)

    import os
    dbg_no_gather = os.environ.get("EDGE_V3_NO_GATHER") == "1"
    dbg_no_preload = os.environ.get("EDGE_V3_NO_PRELOAD") == "1"
    dbg_no_compute = os.environ.get("EDGE_V3_NO_COMPUTE") == "1"
    dbg_two_queue = os.environ.get("EDGE_V3_TWO_QUEUE") == "1"

    nc = bacc.Bacc(None, target_bir_lowering=False,
                   dynamic_dma_scratch_size=32768, num_swdge_queues=4)

    if dbg_no_gather:
        zdummy = nc.dram_tensor("zdummy", [P, ch], bf16, kind="ExternalInput")
    zu_b = [nc.dram_tensor(f"zu{i}", [r, H], bf16, kind="ExternalInput")
            for i, r in enumerate(rows_u)]
    zm_b = [nc.dram_tensor(f"zm{i}", [r, H], bf16, kind="ExternalInput")
            for i, r in enumerate(rows_m)]
    w1ut = nc.dram_tensor("w1ut", [H, H], bf16, kind="ExternalInput")
    w1mt = nc.dram_tensor("w1mt", [H, H], bf16, kind="ExternalInput")
    b1r4 = nc.dram_tensor("b1r4", [P, GRP * H], f32, kind="ExternalInput")
    b2c = nc.dram_tensor("b2c", [P, 1], f32, kind="ExternalInput")
    idxU = nc.dram_tensor("idxU", [P, L // 16], i16, kind="ExternalInput")
    idxM = nc.dram_tensor("idxM", [P, L // 16], i16, kind="ExternalInput")
    out_d = nc.dram_tensor("out", [nblk, P, BCOLS], f32, kind="ExternalOutput")

    with tile.TileContext(nc) as tc:
        with (
            tc.tile_pool(name="const", bufs=1) as cpool,
            tc.tile_pool(name="gat", bufs=4) as gpool,
            tc.tile_pool(name="idx", bufs=4) as ipool,
            tc.tile_pool(name="rel", bufs=4) as rpool,
            tc.tile_pool(name="lgb", bufs=2) as lpool,
            tc.tile_pool(name="obuf", bufs=2) as opool,
            tc.tile_pool(name="psum", bufs=4, space="PSUM") as ppool,
        ):
            nc.gpsimd.load_library(library_config.mlp)
            w1ut_t = cpool.tile([H, H], bf16)
            w1mt_t = cpool.tile([H, H], bf16)
            b1r4_t = cpool.tile([P, GRP * H], f32)
            b2_t = cpool.tile([P, 1], f32)
            nc.sync.dma_start(out=w1ut_t[:], in_=w1ut[:])
            nc.sync.dma_start(out=w1mt_t[:], in_=w1mt[:])
            nc.sync.dma_start(out=b1r4_t[:], in_=b1r4[:])
            nc.sync.dma_start(out=b2_t[:], in_=b2c[:])

            for _rep in range(repeat):
                col = 0              # global tile-column index
                lg_blk = None

                def drain_block(c0):
                    nco = c0 % BCOLS if c0 % BCOLS else BCOLS
                    blk = (c0 - 1) // BCOLS
                    ot = opool.tile([P, BCOLS], f32, tag="ot")
                    nc.scalar.activation(
                        out=ot[:, 0:nco], in_=lg_blk[:, 0:nco],
                        func=mybir.ActivationFunctionType.Sigmoid,
                        bias=b2_t[:, 0:1], scale=1.0)
                    nc.scalar.mul(out=ot[:, 0:nco], in_=ot[:, 0:nco], mul=5.0)
                    nc.sync.dma_start(out=out_d[blk, :, 0:nco], in_=ot[:, 0:nco])

                chunk_no = 0
                for b, cap in enumerate(caps):
                    bu, bm = divmod(b, nbM)
                    base = int(sum(caps[:b]))
                    pos = 0
                    while pos < cap:
                        cur = int(min(ch, cap - pos))
                        o16 = (base + pos) // 16
                        iu_t = ipool.tile([P, ch // 16], i16, tag="iu")
                        im_t = ipool.tile([P, ch // 16], i16, tag="im")
                        nc.sync.dma_start(out=iu_t[:, 0:cur // 16],
                                          in_=idxU[:, o16:o16 + cur // 16])
                        nc.sync.dma_start(out=im_t[:, 0:cur // 16],
                                          in_=idxM[:, o16:o16 + cur // 16])
                        ut = gpool.tile([P, cur], bf16, tag="ut")
                        mt = gpool.tile([P, cur], bf16, tag="mt")
                        if dbg_no_gather:
                            nc.sync.dma_start(out=ut[:], in_=zdummy[:, 0:cur])
                            nc.sync.dma_start(out=mt[:], in_=zdummy[:, 0:cur])
                        else:
                            # NOTE: transpose gathers corrupt data when run
                            # concurrently on multiple queues (shared xbar
                            # scratch) - keep both on queue 0.
                            nc.gpsimd.dma_gather(
                                out_ap=ut[:].rearrange("p (a n) -> p a n", a=1),
                                in_ap=zu_b[bu][:],
                                idxs_ap=iu_t[:, 0:cur // 16],
                                num_idxs=cur, num_idxs_reg=cur, elem_size=H,
                                transpose=True, queue_num=0,
                                single_packet=False)
                            nc.gpsimd.dma_gather(
                                out_ap=mt[:].rearrange("p (a n) -> p a n", a=1),
                                in_ap=zm_b[bm][:],
                                idxs_ap=im_t[:, 0:cur // 16],
                                num_idxs=cur, num_idxs_reg=cur, elem_size=H,
                                transpose=True, queue_num=0,
                                single_packet=False)
                        chunk_no += 1
                        for g in range(cur // TILE):
                            if col % BCOLS == 0:
                                lg_blk = lpool.tile([P, BCOLS], f32, tag="lg")
                            if dbg_no_compute:
                                col += GRP
                                continue
                            ps = ppool.tile([P, GRP * H], f32, tag="ps")
                            if not dbg_no_preload:
                                nc.scalar.copy(out=ps[:], in_=b1r4_t[:])
                            for t in range(GRP):
                                e0 = (g * GRP + t) * P
                                nc.tensor.matmul(
                                    out=ps[:, t * H:(t + 1) * H],
                                    lhsT=ut[:, e0:e0 + P], rhs=w1ut_t[:],
                                    start=dbg_no_preload, stop=False,
                                    skip_group_check=True)
                                nc.tensor.matmul(
                                    out=ps[:, t * H:(t + 1) * H],
                                    lhsT=mt[:, e0:e0 + P], rhs=w1mt_t[:],
                                    start=False, stop=True,
                                    skip_group_check=True)
                            rl = rpool.tile([P, GRP * H], bf16, tag="rl")
                            nc.scalar.activation(
                                out=rl[:], in_=ps[:],
                                func=mybir.ActivationFunctionType.Relu)
                            rv = rl[:].rearrange("p (g h) -> p g h", h=H)
                            c4 = col % BCOLS
                            if Hp == H:
                                nc.vector.tensor_reduce(
                                    out=lg_blk[:, c4:c4 + GRP], in_=rv[:, :, :],
                                    axis=mybir.AxisListType.X,
                                    op=mybir.AluOpType.add)
                            else:
                                lgp = rpool.tile([P, GRP], f32, tag="lgp")
                                lgn = rpool.tile([P, GRP], f32, tag="lgn")
                                if Hp > 0:
                                    nc.vector.tensor_reduce(
                                        out=lgp[:], in_=rv[:, :, 0:Hp],
                                        axis=mybir.AxisListType.X,
                                        op=mybir.AluOpType.add)
                                nc.vector.tensor_reduce(
                                    out=lgn[:], in_=rv[:, :, Hp:H],
                                    axis=mybir.AxisListType.X,
                                    op=mybir.AluOpType.add)
                                if Hp > 0:
                                    nc.vector.tensor_sub(
                                        out=lg_blk[:, c4:c4 + GRP],
                                        in0=lgp[:], in1=lgn[:])
                                else:
                                    nc.vector.tensor_scalar_mul(
                                        out=lg_blk[:, c4:c4 + GRP],
                                        in0=lgn[:], scalar1=-1.0)
                            col += GRP
                            if col % BCOLS == 0 or col == ncols:
                                drain_block(col)
                        pos += cur
    nc.finalize()
    return nc


def _roundup(n, m):
    return ((n + m - 1) // m) * m


def _prepare_v3(z_user, z_movie, edge_index, W1, b1, W2, b2,
                n_cores=N_CORES, bank=BANK):
    import ml_dtypes
    bf16 = ml_dtypes.bfloat16
    z_user = np.asarray(z_user, dtype=np.float32)
    z_movie = np.asarray(z_movie, dtype=np.float32)
    edge_index = np.asarray(edge_index)
    W1 = np.asarray(W1, dtype=np.float32)
    b1 = np.asarray(b1, dtype=np.float32)
    W2 = np.asarray(W2, dtype=np.float32)
    b2 = np.asarray(b2, dtype=np.float32)

    E = edge_index.shape[1]
    rows = edge_index[0].astype(np.int64)
    cols = edge_index[1].astype(np.int64)
    NU, NM = z_user.shape[0], z_movie.shape[0]
    nbU, nbM = -(-NU // bank), -(-NM // bank)
    nbkt = nbU * nbM
    Epc = -(-E // n_cores)

    per_core = []
    cnts = np.zeros((n_cores, nbkt), dtype=np.int64)
    for c in range(n_cores):
        sl = slice(c * Epc, min((c + 1) * Epc, E))
        r, co = rows[sl], cols[sl]
        bkt = (r // bank) * nbM + (co // bank)
        order = np.argsort(bkt, kind="stable")
        cnts[c] = np.bincount(bkt, minlength=nbkt)
        per_core.append((sl, order, r, co, bkt))

    caps = np.maximum(_roundup(cnts.max(axis=0), TILE), TILE)
    offs = np.concatenate([[0], np.cumsum(caps)])
    L = int(offs[-1])

    # permute hidden units w2>=0 first; fold |w2| into W1 rows and b1.
    # logit = sum_pos relu(|w2|y) - sum_neg relu(|w2|y)
    w2 = W2.reshape(-1)
    perm = np.argsort(w2 < 0, kind="stable")
    Hp = int((w2 >= 0).sum())
    w2sc = np.abs(w2[perm])
    W1p = W1[perm] * w2sc[:, None]          # [h', 2H]
    b1p = b1[perm] * w2sc                   # [h']

    zu16 = np.ascontiguousarray(z_user.astype(bf16))
    zm16 = np.ascontiguousarray(z_movie.astype(bf16))
    shared = {"w1ut": np.ascontiguousarray(W1p[:, :H].T).astype(bf16),
              "w1mt": np.ascontiguousarray(W1p[:, H:].T).astype(bf16),
              "b1r4": np.ascontiguousarray(
                  np.tile(b1p, (P, GRP)).astype(np.float32)),
              "b2c": np.full((P, 1), float(b2.reshape(-1)[0]), np.float32)}
    rows_u, rows_m = [], []
    for i in range(nbU):
        bk = np.ascontiguousarray(zu16[i * bank:(i + 1) * bank])
        shared[f"zu{i}"] = bk
        rows_u.append(bk.shape[0])
    for i in range(nbM):
        bk = np.ascontiguousarray(zm16[i * bank:(i + 1) * bank])
        shared[f"zm{i}"] = bk
        rows_m.append(bk.shape[0])

    in_maps, backmaps = [], []
    for c in range(n_cores):
        sl, order, r, co, bkt = per_core[c]
        n = len(r)
        starts = np.concatenate([[0], np.cumsum(cnts[c])])
        sorted_bkt = bkt[order]
        k = np.arange(n) - starts[sorted_bkt]
        spos = offs[sorted_bkt] + k          # slot of edge order[i]
        iu = np.zeros(L, np.int16)
        im = np.zeros(L, np.int16)
        iu[spos] = (r[order] % bank).astype(np.int16)
        im[spos] = (co[order] % bank).astype(np.int16)
        slot = np.empty(n, np.int64)
        slot[order] = spos
        iu_w = np.ascontiguousarray(np.tile(iu.reshape(L // 16, 16).T, (8, 1)))
        im_w = np.ascontiguousarray(np.tile(im.reshape(L // 16, 16).T, (8, 1)))
        in_maps.append({**shared, "idxU": iu_w, "idxM": im_w})
        backmaps.append((sl, slot))
    return in_maps, dict(rows_u=rows_u, rows_m=rows_m,
                         caps=[int(x) for x in caps], L=L, E=E, Hp=Hp,
                         backmaps=backmaps)


def _unpack_v3(res, meta):
    out = np.empty(meta["E"], dtype=np.float32)
    for c, (sl, slot) in enumerate(meta["backmaps"]):
        flat = np.asarray(res.results[c]["out"], dtype=np.float32).reshape(-1)
        # edge at stream slot s -> tile-column s//128, partition s%128;
        # out tensor is [nblk, 128, BCOLS]
        tc_ = slot // P
        p = slot % P
        fidx = (tc_ // BCOLS) * (P * BCOLS) + p * BCOLS + (tc_ % BCOLS)
        out[sl] = flat[fidx]
    return out


# ---------------------------------------------------------------------------
# v6: device-precomputed A/B node tables (A=W1u z_u + b1, B=W1m z_m, |w2|
# folded, pos-w2-first permutation) + per-edge dual NON-transpose dma_gather
# spread over all 4 SWDGE queues (measured: 1 queue = 7.9 ns/row, 4 queues =
# 1.79 ns/row). Edges land on partitions, H on free dim: DVE add, ACT relu,
# DVE pos/neg reduces -> logits. No per-edge PE work.
# Slot mapping identical to v3 (slot s -> partition s%128, tile-col s//128).
# ---------------------------------------------------------------------------

NT = 50176       # referenced node rows padded to 98*512 (indices < 50000)


def _build_nc_v6(rows_u, rows_m, caps, Hp, ch=CH, repeat=1):
    """rows_u/rows_m: rows per user/movie table bank (sum = NT each).
    caps: per-bucket edge capacity (multiples of TILE; bucket b =
    ubank*len(rows_m) + mbank). Hp: # hidden units with w2 >= 0."""
    f32 = mybir.dt.float32
    bf16 = mybir.dt.bfloat16
    i16 = mybir.dt.int16
    nbM = len(rows_m)
    L = int(sum(caps))
    ncols = L // P                   # total tile-columns
    nblk = -(-ncols // BCOLS)
    assert 0 < Hp < H

    import os
    dbg_no_gather = os.environ.get("EDGE_V6_NO_GATHER") == "1"
    dbg_no_precomp = os.environ.get("EDGE_V6_NO_PRECOMP") == "1"
    dbg_no_compute = os.environ.get("EDGE_V6_NO_COMPUTE") == "1"

    nc = bacc.Bacc(None, target_bir_lowering=False,
                   dynamic_dma_scratch_size=32768, num_swdge_queues=4)

    zuT = nc.dram_tensor("zuT", [P, NT], bf16, kind="ExternalInput")
    zmT = nc.dram_tensor("zmT", [P, NT], bf16, kind="ExternalInput")
    w1ut = nc.dram_tensor("w1ut", [H, H], bf16, kind="ExternalInput")
    w1mt = nc.dram_tensor("w1mt", [H, H], bf16, kind="ExternalInput")
    b1r4 = nc.dram_tensor("b1r4", [P, GRP * H], f32, kind="ExternalInput")
    b2c = nc.dram_tensor("b2c", [P, 1], f32, kind="ExternalInput")
    idxU = nc.dram_tensor("idxU", [P, L // 16], i16, kind="ExternalInput")
    idxM = nc.dram_tensor("idxM", [P, L // 16], i16, kind="ExternalInput")
    out_d = nc.dram_tensor("out", [nblk, P, BCOLS], f32, kind="ExternalOutput")

    tabs_u = [nc.dram_tensor(f"tabU{i}", [r, H], bf16) for i, r in enumerate(rows_u)]
    tabs_m = [nc.dram_tensor(f"tabM{i}", [r, H], bf16) for i, r in enumerate(rows_m)]

    with tile.TileContext(nc) as tc:
        with (
            tc.tile_pool(name="const", bufs=1) as cpool,
            tc.tile_pool(name="pre", bufs=2) as prepool,
            tc.tile_pool(name="gat", bufs=3) as gpool,
            tc.tile_pool(name="idx", bufs=4) as ipool,
            tc.tile_pool(name="rel", bufs=2) as rpool,
            tc.tile_pool(name="lgs", bufs=3) as spool,
            tc.tile_pool(name="lgb", bufs=2) as lpool,
            tc.tile_pool(name="obuf", bufs=2) as opool,
            tc.tile_pool(name="psum", bufs=4, space="PSUM") as ppool,
        ):
            nc.gpsimd.load_library(library_config.mlp)
            w1ut_t = cpool.tile([H, H], bf16)
            w1mt_t = cpool.tile([H, H], bf16)
            b1r4_t = cpool.tile([P, GRP * H], f32)
            b2_t = cpool.tile([P, 1], f32)
            nc.sync.dma_start(out=w1ut_t[:], in_=w1ut[:])
            nc.sync.dma_start(out=w1mt_t[:], in_=w1mt[:])
            nc.sync.dma_start(out=b1r4_t[:], in_=b1r4[:])
            nc.sync.dma_start(out=b2_t[:], in_=b2c[:])

            for _rep in range(repeat):
                # ---- precompute node tables (bank-interleaved U0,M0,U1,M1
                # so bucket (0,0) gathers can start early) ----
                gno = 0
                ZB = 4096            # z columns staged per DMA (8 groups)
                for bi in range(len(rows_u) if not dbg_no_precomp else 0):
                    for (zT, w1t, tabs, rows, addb1, goff) in (
                        (zuT, w1ut_t, tabs_u, rows_u, True, 0),
                        (zmT, w1mt_t, tabs_m, rows_m, False, 0),
                    ):
                        base = int(sum(rows[:bi]))
                        for z0 in range(0, rows[bi], ZB):
                            zn = min(ZB, rows[bi] - z0)
                            zbig = prepool.tile([P, ZB], bf16, tag="zst")
                            nc.sync.dma_start(
                                out=zbig[:, 0:zn],
                                in_=zT[:, base + z0:base + z0 + zn])
                            for s in range(zn // TILE):
                                so = z0 // TILE + s
                                pps = ppool.tile([P, GRP * H], f32, tag="ps")
                                if addb1:
                                    nc.scalar.copy(out=pps[:], in_=b1r4_t[:])
                                for t in range(GRP):
                                    nc.tensor.matmul(
                                        out=pps[:, t * H:(t + 1) * H],
                                        lhsT=zbig[:, s * TILE + t * P:
                                                  s * TILE + (t + 1) * P],
                                        rhs=w1t[:], start=not addb1,
                                        stop=True, skip_group_check=True)
                                ast = prepool.tile([P, GRP * H], bf16,
                                                   tag="ast")
                                if gno % 2 == 0:
                                    nc.scalar.copy(out=ast[:], in_=pps[:])
                                else:
                                    nc.vector.tensor_copy(out=ast[:],
                                                          in_=pps[:])
                                gno += 1
                                nc.sync.dma_start(
                                    out=tabs[bi][so * TILE:(so + 1) * TILE, :]
                                    .rearrange("(t p) h -> p t h", p=P),
                                    in_=ast[:].rearrange("p (t h) -> p t h",
                                                         h=H))

                # ---- edge phase ----
                col = 0              # global tile-column index
                lg_blk = None

                def drain_block(c0):
                    nco = c0 % BCOLS if c0 % BCOLS else BCOLS
                    blk = (c0 - 1) // BCOLS
                    ot = opool.tile([P, BCOLS], f32, tag="ot")
                    nc.scalar.activation(
                        out=ot[:, 0:nco], in_=lg_blk[:, 0:nco],
                        func=mybir.ActivationFunctionType.Sigmoid,
                        bias=b2_t[:, 0:1], scale=1.0)
                    nc.scalar.mul(out=ot[:, 0:nco], in_=ot[:, 0:nco], mul=5.0)
                    nc.sync.dma_start(out=out_d[blk, :, 0:nco], in_=ot[:, 0:nco])

                chunk_no = 0
                for b, cap in enumerate(caps):
                    bu, bm = divmod(b, nbM)
                    base = int(sum(caps[:b]))
                    pos = 0
                    while pos < cap:
                        cur = int(min(ch, cap - pos))
                        o16 = (base + pos) // 16
                        na = cur // P        # tile-cols in this chunk
                        iu_t = ipool.tile([P, ch // 16], i16, tag="iu")
                        im_t = ipool.tile([P, ch // 16], i16, tag="im")
                        nc.sync.dma_start(out=iu_t[:, 0:cur // 16],
                                          in_=idxU[:, o16:o16 + cur // 16])
                        nc.sync.dma_start(out=im_t[:, 0:cur // 16],
                                          in_=idxM[:, o16:o16 + cur // 16])
                        ut = gpool.tile([P, ch], bf16, tag="ut")
                        mt = gpool.tile([P, ch], bf16, tag="mt")
                        if not dbg_no_gather:
                            nc.gpsimd.dma_gather(
                                out_ap=ut[:, 0:cur].rearrange(
                                    "p (a n) -> p a n", a=na),
                                in_ap=tabs_u[bu][:],
                                idxs_ap=iu_t[:, 0:cur // 16],
                                num_idxs=cur, num_idxs_reg=cur, elem_size=H,
                                transpose=False,
                                queue_num=(2 * chunk_no) % 4,
                                single_packet=False)
                            nc.gpsimd.dma_gather(
                                out_ap=mt[:, 0:cur].rearrange(
                                    "p (a n) -> p a n", a=na),
                                in_ap=tabs_m[bm][:],
                                idxs_ap=im_t[:, 0:cur // 16],
                                num_idxs=cur, num_idxs_reg=cur, elem_size=H,
                                transpose=False,
                                queue_num=(2 * chunk_no + 1) % 4,
                                single_packet=False)
                        chunk_no += 1
                        if dbg_no_compute:
                            col += na
                            if col % BCOLS == 0 or col >= ncols:
                                pass
                            pos += cur
                            continue
                        yt = rpool.tile([P, ch], bf16, tag="yt")
                        nc.vector.tensor_add(out=yt[:, 0:cur], in0=ut[:, 0:cur],
                                             in1=mt[:, 0:cur])
                        nc.scalar.activation(
                            out=yt[:, 0:cur], in_=yt[:, 0:cur],
                            func=mybir.ActivationFunctionType.Relu)
                        rv = yt[:, 0:cur].rearrange("p (a h) -> p a h", h=H)
                        # pos/neg reduces -> logits, split at block boundaries
                        a0 = 0
                        while a0 < na:
                            if col % BCOLS == 0:
                                lg_blk = lpool.tile([P, BCOLS], f32, tag="lg")
                            c4 = col % BCOLS
                            seg = int(min(na - a0, BCOLS - c4))
                            lgp = spool.tile([P, ch // P], f32, tag="lgp")
                            lgn = spool.tile([P, ch // P], f32, tag="lgn")
                            nc.vector.tensor_reduce(
                                out=lgp[:, 0:seg], in_=rv[:, a0:a0 + seg, 0:Hp],
                                axis=mybir.AxisListType.X,
                                op=mybir.AluOpType.add)
                            nc.vector.tensor_reduce(
                                out=lgn[:, 0:seg], in_=rv[:, a0:a0 + seg, Hp:H],
                                axis=mybir.AxisListType.X,
                                op=mybir.AluOpType.add)
                            nc.vector.tensor_sub(
                                out=lg_blk[:, c4:c4 + seg],
                                in0=lgp[:, 0:seg], in1=lgn[:, 0:seg])
                            col += seg
                            a0 += seg
                            if col % BCOLS == 0 or col == ncols:
                                drain_block(col)
                        pos += cur
    nc.finalize()
    return nc


def _prepare_v6(z_user, z_movie, edge_index, W1, b1, W2, b2,
                n_cores=N_CORES, bank=BANK):
    import ml_dtypes
    bf16 = ml_dtypes.bfloat16
    z_user = np.asarray(z_user, dtype=np.float32)
    z_movie = np.asarray(z_movie, dtype=np.float32)
    edge_index = np.asarray(edge_index)
    W1 = np.asarray(W1, dtype=np.float32)
    b1 = np.asarray(b1, dtype=np.float32)
    W2 = np.asarray(W2, dtype=np.float32)
    b2 = np.asarray(b2, dtype=np.float32)

    E = edge_index.shape[1]
    rows = edge_index[0].astype(np.int64)
    cols = edge_index[1].astype(np.int64)
    if E and (rows.max() >= NT or cols.max() >= NT):
        raise ValueError("edge index out of v6 table range")
    nbU = nbM = -(-NT // bank)
    nbkt = nbU * nbM
    Epc = -(-E // n_cores)

    per_core = []
    cnts = np.zeros((n_cores, nbkt), dtype=np.int64)
    for c in range(n_cores):
        sl = slice(c * Epc, min((c + 1) * Epc, E))
        r, co = rows[sl], cols[sl]
        bkt = (r // bank) * nbM + (co // bank)
        order = np.argsort(bkt, kind="stable")
        cnts[c] = np.bincount(bkt, minlength=nbkt)
        per_core.append((sl, order, r, co, bkt))

    caps = np.maximum(_roundup(cnts.max(axis=0), TILE), TILE)
    offs = np.concatenate([[0], np.cumsum(caps)])
    L = int(offs[-1])

    # permute hidden units w2>=0 first; fold |w2| into W1 rows and b1.
    w2 = W2.reshape(-1)
    perm = np.argsort(w2 < 0, kind="stable")
    Hp = int((w2 >= 0).sum())
    w2sc = np.abs(w2[perm])
    W1p = W1[perm] * w2sc[:, None]          # [h', 2H]
    b1p = b1[perm] * w2sc                   # [h']

    nuse_u = min(z_user.shape[0], NT)
    nuse_m = min(z_movie.shape[0], NT)
    zuT = np.zeros((P, NT), dtype=bf16)
    zuT[:, :nuse_u] = z_user[:nuse_u].T.astype(bf16)
    zmT = np.zeros((P, NT), dtype=bf16)
    zmT[:, :nuse_m] = z_movie[:nuse_m].T.astype(bf16)
    shared = {"zuT": zuT, "zmT": zmT,
              "w1ut": np.ascontiguousarray(W1p[:, :H].T).astype(bf16),
              "w1mt": np.ascontiguousarray(W1p[:, H:].T).astype(bf16),
              "b1r4": np.ascontiguousarray(
                  np.tile(b1p, (P, GRP)).astype(np.float32)),
              "b2c": np.full((P, 1), float(b2.reshape(-1)[0]), np.float32)}
    rows_u = [min(bank, NT - i * bank) for i in range(nbU)]
    rows_m = [min(bank, NT - i * bank) for i in range(nbM)]

    in_maps, backmaps = [], []
    for c in range(n_cores):
        sl, order, r, co, bkt = per_core[c]
        n = len(r)
        starts = np.concatenate([[0], np.cumsum(cnts[c])])
        sorted_bkt = bkt[order]
        k = np.arange(n) - starts[sorted_bkt]
        spos = offs[sorted_bkt] + k          # slot of edge order[i]
        iu = np.zeros(L, np.int16)
        im = np.zeros(L, np.int16)
        iu[spos] = (r[order] % bank).astype(np.int16)
        im[spos] = (co[order] % bank).astype(np.int16)
        slot = np.empty(n, np.int64)
        slot[order] = spos
        iu_w = np.ascontiguousarray(np.tile(iu.reshape(L // 16, 16).T, (8, 1)))
        im_w = np.ascontiguousarray(np.tile(im.reshape(L // 16, 16).T, (8, 1)))
        in_maps.append({**shared, "idxU": iu_w, "idxM": im_w})
        backmaps.append((sl, slot))
    return in_maps, dict(rows_u=rows_u, rows_m=rows_m,
                         caps=[int(x) for x in caps], L=L, E=E, Hp=Hp,
                         backmaps=backmaps)


# ---------------------------------------------------------------------------
# v7: user-range sharding. A-side (user) via PE one-hot expansion: edges
# sorted by (movie-bank, local user window); host streams bf16 one-hot masks
# (index-derived only); window tiles of the per-core A table feed PE as rhs.
# B-side (movie) via non-transpose dma_gather over all 4 SWDGE queues.
# Tables precomputed on device (A per-core slice w/ b1+|w2| fold; B full).
# Slot mapping identical to v3/v6 (slot s -> partition s%128, col s//128).
# ---------------------------------------------------------------------------

UPC7 = 6272      # users per core (50176/8); window = 128 users, 49/core
UPC7P = 6656     # padded to 13*512 for the 512-row precompute groups


def _v7_schedule(caps):
    """caps: [2][49] window slot capacities (each mult of 16; run totals mult
    of 512). Returns (sched, naux): sched = per 512-slot group the list of
    window ids (global: mb*49 + w); naux = total aux mask tiles."""
    nwin = len(caps[0])
    sched = []
    base = 0
    for mb in range(2):
        run = int(sum(caps[mb]))
        assert run % 512 == 0
        starts = np.concatenate([[0], np.cumsum(caps[mb])])
        for g0 in range(run // 512):
            lo, hi = g0 * 512, (g0 + 1) * 512
            w_lo = int(np.searchsorted(starts, lo, side="right") - 1)
            w_hi = int(np.searchsorted(starts, hi - 1, side="right") - 1)
            sched.append([mb * nwin + w for w in range(w_lo, w_hi + 1)])
        base += run
    naux = sum(len(ws) - 1 for ws in sched)
    return sched, naux


def _build_nc_v7(caps, Hp, sched, naux, ch=4096, repeat=1):
    """caps: [2][nwin] window capacities. sched/naux: from _v7_schedule."""
    f32 = mybir.dt.float32
    bf16 = mybir.dt.bfloat16
    i16 = mybir.dt.int16
    nwin = len(caps[0])
    run_len = [int(sum(caps[mb])) for mb in range(2)]
    L = sum(run_len)
    ncols = L // P
    nblk = -(-ncols // BCOLS)
    NTU = nwin * P               # per-core A rows (6272)
    rows_m = [BANK, NT - BANK]
    assert 0 < Hp < H and L % 512 == 0

    import os
    dbg_no_gather = os.environ.get("EDGE_V7_NO_GATHER") == "1"
    dbg_no_onehot = os.environ.get("EDGE_V7_NO_ONEHOT") == "1"
    dbg_no_mask0 = os.environ.get("EDGE_V7_NO_MASK0") == "1"
    dbg_no_compute = os.environ.get("EDGE_V7_NO_COMPUTE") == "1"
    dbg_no_precomp = os.environ.get("EDGE_V7_NO_PRECOMP") == "1"

    nc = bacc.Bacc(None, target_bir_lowering=False,
                   dynamic_dma_scratch_size=32768, num_swdge_queues=4)

    zuTc = nc.dram_tensor("zuTc", [P, UPC7P], bf16, kind="ExternalInput")
    zmT = nc.dram_tensor("zmT", [P, NT], bf16, kind="ExternalInput")
    w1ut = nc.dram_tensor("w1ut", [H, H], bf16, kind="ExternalInput")
    w1mt = nc.dram_tensor("w1mt", [H, H], bf16, kind="ExternalInput")
    b1r4 = nc.dram_tensor("b1r4", [P, GRP * H], f32, kind="ExternalInput")
    b2c = nc.dram_tensor("b2c", [P, 1], f32, kind="ExternalInput")
    idxM = nc.dram_tensor("idxM", [P, L // 16], i16, kind="ExternalInput")
    mask0 = nc.dram_tensor("mask0", [P, L], bf16, kind="ExternalInput")
    maskx = nc.dram_tensor("maskx", [P, max(naux, 1) * 512], bf16,
                           kind="ExternalInput")
    out_d = nc.dram_tensor("out", [nblk, P, BCOLS], f32, kind="ExternalOutput")

    tabU = nc.dram_tensor("tabU", [UPC7P, H], bf16)
    tabs_m = [nc.dram_tensor(f"tabM{i}", [r, H], bf16)
              for i, r in enumerate(rows_m)]

    with tile.TileContext(nc) as tc:
        with (
            tc.tile_pool(name="const", bufs=1) as cpool,
            tc.tile_pool(name="pre", bufs=2) as prepool,
            tc.tile_pool(name="gat", bufs=8) as gpool,
            tc.tile_pool(name="msk", bufs=4) as mpool,
            tc.tile_pool(name="idx", bufs=8) as ipool,
            tc.tile_pool(name="win", bufs=4) as wpool,
            tc.tile_pool(name="aux", bufs=3) as xpool,
            tc.tile_pool(name="rel", bufs=2) as rpool,
            tc.tile_pool(name="lgs", bufs=4) as spool,
            tc.tile_pool(name="lgb", bufs=2) as lpool,
            tc.tile_pool(name="obuf", bufs=2) as opool,
            tc.tile_pool(name="psum", bufs=2, space="PSUM") as ppool,
            tc.tile_pool(name="psue", bufs=3, space="PSUM") as ppool2,
        ):
            nc.gpsimd.load_library(library_config.mlp)
            w1ut_t = cpool.tile([H, H], bf16)
            w1mt_t = cpool.tile([H, H], bf16)
            b1r4_t = cpool.tile([P, GRP * H], f32)
            b2_t = cpool.tile([P, 1], f32)
            nc.sync.dma_start(out=w1ut_t[:], in_=w1ut[:])
            nc.sync.dma_start(out=w1mt_t[:], in_=w1mt[:])
            nc.sync.dma_start(out=b1r4_t[:], in_=b1r4[:])
            nc.sync.dma_start(out=b2_t[:], in_=b2c[:])

            for _rep in range(repeat):
                # ---- precompute: tabM bank0, tabU slice, tabM bank1 ----
                gno = 0

                ZB = 4096            # z columns staged per DMA (8 groups)

                def pre_groups(zT, w1t, tab, zoff, n512, addb1):
                    nonlocal gno
                    for z0 in range(0, n512 * TILE, ZB):
                        zn = min(ZB, n512 * TILE - z0)
                        zbig = prepool.tile([P, ZB], bf16, tag="zst")
                        nc.sync.dma_start(
                            out=zbig[:, 0:zn],
                            in_=zT[:, zoff + z0:zoff + z0 + zn])
                        for s in range(zn // TILE):
                            so = z0 // TILE + s
                            pps = ppool.tile([P, GRP * H], f32, tag="ps")
                            if addb1:
                                nc.scalar.copy(out=pps[:], in_=b1r4_t[:])
                            for t in range(GRP):
                                nc.tensor.matmul(
                                    out=pps[:, t * H:(t + 1) * H],
                                    lhsT=zbig[:, s * TILE + t * P:
                                              s * TILE + (t + 1) * P],
                                    rhs=w1t[:], start=not addb1,
                                    stop=True, skip_group_check=True)
                            ast = prepool.tile([P, GRP * H], bf16, tag="ast")
                            if gno % 2 == 0:
                                nc.scalar.copy(out=ast[:], in_=pps[:])
                            else:
                                nc.vector.tensor_copy(out=ast[:], in_=pps[:])
                            gno += 1
                            nc.sync.dma_start(
                                out=tab[so * TILE:(so + 1) * TILE, :]
                                .rearrange("(t p) h -> p t h", p=P),
                                in_=ast[:].rearrange("p (t h) -> p t h", h=H))

                if not dbg_no_precomp:
                    pre_groups(zmT, w1mt_t, tabs_m[0], 0, BANK // TILE, False)
                    pre_groups(zuTc, w1ut_t, tabU, 0, UPC7P // TILE, True)
                    pre_groups(zmT, w1mt_t, tabs_m[1], BANK,
                               (NT - BANK) // TILE, False)

                # ---- edge phase ----
                col = 0
                lg_blk = None
                aux_no = 0
                g_global = 0
                wt_cache = {}            # window id -> (handle, load_ordinal)
                wt_loads = 0

                def get_window(w):
                    nonlocal wt_loads
                    ent = wt_cache.get(w)
                    if ent is not None and wt_loads - ent[1] < 4:
                        return ent[0]
                    wt = wpool.tile([P, H], bf16, tag="wt")
                    r0 = (w % nwin) * P
                    nc.sync.dma_start(out=wt[:], in_=tabU[r0:r0 + P, :])
                    wt_cache[w] = (wt, wt_loads)
                    wt_loads += 1
                    return wt

                def drain_block(c0):
                    nco = c0 % BCOLS if c0 % BCOLS else BCOLS
                    blk = (c0 - 1) // BCOLS
                    ot = opool.tile([P, BCOLS], f32, tag="ot")
                    nc.scalar.activation(
                        out=ot[:, 0:nco], in_=lg_blk[:, 0:nco],
                        func=mybir.ActivationFunctionType.Sigmoid,
                        bias=b2_t[:, 0:1], scale=1.0)
                    nc.scalar.mul(out=ot[:, 0:nco], in_=ot[:, 0:nco], mul=5.0)
                    nc.sync.dma_start(out=out_d[blk, :, 0:nco], in_=ot[:, 0:nco])

                chunk_no = 0
                for mb in range(2):
                    base = sum(run_len[:mb])
                    cap = run_len[mb]
                    pos = 0
                    while pos < cap:
                        cur = int(min(ch, cap - pos))
                        s0 = base + pos
                        im_t = ipool.tile([P, ch // 16], i16, tag="im")
                        nc.sync.dma_start(
                            out=im_t[:, 0:cur // 16],
                            in_=idxM[:, s0 // 16:(s0 + cur) // 16])
                        bt = gpool.tile([P, ch], bf16, tag="bt")
                        if not dbg_no_gather:
                            nc.gpsimd.dma_gather(
                                out_ap=bt[:, 0:cur].rearrange(
                                    "p (a n) -> p a n", a=cur // P),
                                in_ap=tabs_m[mb][:],
                                idxs_ap=im_t[:, 0:cur // 16],
                                num_idxs=cur, num_idxs_reg=cur, elem_size=H,
                                transpose=False, queue_num=chunk_no % 4,
                                single_packet=False)
                        chunk_no += 1
                        mk0 = mpool.tile([P, ch], bf16, tag="mk0")
                        if not dbg_no_mask0:
                            nc.sync.dma_start(out=mk0[:, 0:cur],
                                              in_=mask0[:, s0:s0 + cur])
                        if dbg_no_compute:
                            g_global += cur // 512
                            col += cur // P
                            pos += cur
                            continue
                        yt = rpool.tile([P, ch], bf16, tag="yt")
                        assert cur % 1024 == 0
                        for gp in range(cur // 1024):
                            if dbg_no_onehot:
                                g_global += 2
                                nc.vector.tensor_copy(
                                    out=yt[:, gp * 1024:(gp + 1) * 1024],
                                    in_=bt[:, gp * 1024:(gp + 1) * 1024])
                                continue
                            # two 512-slot groups share one 2-bank PSUM tile
                            ps = ppool2.tile([P, 2 * GRP * H], f32, tag="eps")
                            for half in range(2):
                                gi = gp * 2 + half
                                wins = sched[g_global]
                                parts = []
                                for ki, w in enumerate(wins):
                                    if ki == 0:
                                        mk_t, moff = mk0, gi * 512
                                    else:
                                        mk_t = xpool.tile([P, 512], bf16,
                                                          tag="mx")
                                        nc.sync.dma_start(
                                            out=mk_t[:],
                                            in_=maskx[:, aux_no * 512:
                                                      (aux_no + 1) * 512])
                                        moff = 0
                                        aux_no += 1
                                    parts.append((mk_t, moff, get_window(w)))
                                # per PSUM segment, open and close the PE
                                # accumulation group before moving on (groups
                                # must not interleave across segments)
                                for t in range(GRP):
                                    seg = (half * GRP + t) * H
                                    for ki, (mk_t, moff, wt) in enumerate(parts):
                                        nc.tensor.matmul(
                                            out=ps[:, seg:seg + H],
                                            lhsT=mk_t[:, moff + t * P:
                                                      moff + (t + 1) * P],
                                            rhs=wt[:],
                                            start=(ki == 0),
                                            stop=(ki == len(parts) - 1),
                                            skip_group_check=True)
                                g_global += 1
                            nc.vector.tensor_add(
                                out=yt[:, gp * 1024:(gp + 1) * 1024],
                                in0=ps[:],
                                in1=bt[:, gp * 1024:(gp + 1) * 1024])
                        nc.scalar.activation(
                            out=yt[:, 0:cur], in_=yt[:, 0:cur],
                            func=mybir.ActivationFunctionType.Relu)
                        rv = yt[:, 0:cur].rearrange("p (a h) -> p a h", h=H)
                        na = cur // P
                        a0 = 0
                        while a0 < na:
                            if col % BCOLS == 0:
                                lg_blk = lpool.tile([P, BCOLS], f32, tag="lg")
                            c4 = col % BCOLS
                            seg = int(min(na - a0, BCOLS - c4))
                            lgp = spool.tile([P, ch // P], f32, tag="lgp")
                            lgn = spool.tile([P, ch // P], f32, tag="lgn")
                            nc.vector.tensor_reduce(
                                out=lgp[:, 0:seg], in_=rv[:, a0:a0 + seg, 0:Hp],
                                axis=mybir.AxisListType.X,
                                op=mybir.AluOpType.add)
                            nc.vector.tensor_reduce(
                                out=lgn[:, 0:seg], in_=rv[:, a0:a0 + seg, Hp:H],
                                axis=mybir.AxisListType.X,
                                op=mybir.AluOpType.add)
                            nc.vector.tensor_sub(
                                out=lg_blk[:, c4:c4 + seg],
                                in0=lgp[:, 0:seg], in1=lgn[:, 0:seg])
                            col += seg
                            a0 += seg
                            if col % BCOLS == 0 or col == ncols:
                                drain_block(col)
                        pos += cur
    nc.finalize()
    return nc


def _prepare_v7(z_user, z_movie, edge_index, W1, b1, W2, b2,
                n_cores=N_CORES):
    import ml_dtypes
    bf16 = ml_dtypes.bfloat16
    z_user = np.asarray(z_user, dtype=np.float32)
    z_movie = np.asarray(z_movie, dtype=np.float32)
    edge_index = np.asarray(edge_index)
    W1 = np.asarray(W1, dtype=np.float32)
    b1 = np.asarray(b1, dtype=np.float32)
    W2 = np.asarray(W2, dtype=np.float32)
    b2 = np.asarray(b2, dtype=np.float32)

    E = edge_index.shape[1]
    rows = edge_index[0].astype(np.int64)
    cols = edge_index[1].astype(np.int64)
    if E and (rows.max() >= NT or cols.max() >= NT):
        raise ValueError("edge index out of v7 table range")
    nwin = UPC7 // P

    # per-core split (by user range), then by movie bank, then by window
    core_of = rows // UPC7
    per_core = []
    wcnt = np.zeros((n_cores, 2, nwin), dtype=np.int64)
    for c in range(n_cores):
        eids = np.nonzero(core_of == c)[0]
        r, co = rows[eids], cols[eids]
        u = r - c * UPC7
        mb = co // BANK
        w = u // P
        order = np.lexsort((w, mb))
        eids, u, co, mb, w = eids[order], u[order], co[order], mb[order], w[order]
        for b in range(2):
            wcnt[c, b] = np.bincount(w[mb == b], minlength=nwin)
        per_core.append((eids, u, co, mb, w))

    # shared window capacities: max over cores, round to 16; run mult of 512
    caps = np.maximum(_roundup(wcnt.max(axis=0), 16), 16)
    for b in range(2):
        tot = int(caps[b].sum())
        caps[b][-1] += _roundup(tot, 1024) - tot
    run_len = [int(caps[b].sum()) for b in range(2)]
    L = sum(run_len)
    starts = np.zeros((2, nwin), dtype=np.int64)
    for b in range(2):
        starts[b] = sum(run_len[:b]) + np.concatenate(
            [[0], np.cumsum(caps[b])[:-1]])

    sched, naux = _v7_schedule([list(map(int, caps[0])),
                                list(map(int, caps[1]))])
    # aux ordinal lookup: (group, window) -> ordinal for non-first windows
    aux_of = {}
    k = 0
    for g, ws in enumerate(sched):
        for wi in ws[1:]:
            aux_of[(g, wi)] = k
            k += 1
    assert k == naux

    w2v = W2.reshape(-1)
    perm = np.argsort(w2v < 0, kind="stable")
    Hp = int((w2v >= 0).sum())
    w2sc = np.abs(w2v[perm])
    W1p = W1[perm] * w2sc[:, None]
    b1p = b1[perm] * w2sc

    nuse_m = min(z_movie.shape[0], NT)
    zmT = np.zeros((P, NT), dtype=bf16)
    zmT[:, :nuse_m] = z_movie[:nuse_m].T.astype(bf16)
    shared = {"zmT": zmT,
              "w1ut": np.ascontiguousarray(W1p[:, :H].T).astype(bf16),
              "w1mt": np.ascontiguousarray(W1p[:, H:].T).astype(bf16),
              "b1r4": np.ascontiguousarray(
                  np.tile(b1p, (P, GRP)).astype(np.float32)),
              "b2c": np.full((P, 1), float(b2.reshape(-1)[0]), np.float32)}

    sched_w0 = np.array([ws[0] for ws in sched], dtype=np.int64)
    in_maps, backmaps = [], []
    for c in range(n_cores):
        eids, u, co, mb, w = per_core[c]
        # slot: within-window rank
        wk = mb * nwin + w
        ordr = np.argsort(wk, kind="stable")   # already sorted; rank within
        kk = np.arange(len(u)) - np.concatenate(
            [[0], np.cumsum(np.bincount(wk, minlength=2 * nwin))])[wk]
        slot = starts[mb, w] + kk
        g = slot // 512
        wg = mb * nwin + w                     # global window id of each edge
        is_first = wg == sched_w0[g]
        urow = (u % P).astype(np.int64)
        m0 = np.zeros((P, L), dtype=bf16)
        m0[urow[is_first], slot[is_first]] = 1
        mx = np.zeros((P, max(naux, 1) * 512), dtype=bf16)
        nf = np.nonzero(~is_first)[0]
        if len(nf):
            aux_idx = np.array([aux_of[(int(g[i]), int(wg[i]))] for i in nf],
                               dtype=np.int64)
            mx[urow[nf], aux_idx * 512 + (slot[nf] % 512)] = 1
        im = np.zeros(L, np.int16)
        im[slot] = (co % BANK).astype(np.int16)
        im_w = np.ascontiguousarray(np.tile(im.reshape(L // 16, 16).T, (8, 1)))
        zuTc = np.zeros((P, UPC7P), dtype=bf16)
        lo = c * UPC7
        hi = min((c + 1) * UPC7, z_user.shape[0])
        if hi > lo:
            zuTc[:, :hi - lo] = z_user[lo:hi].T.astype(bf16)
        in_maps.append({**shared, "zuTc": zuTc, "idxM": im_w,
                        "mask0": m0, "maskx": mx})
        backmaps.append((eids, slot))
    caps_py = [list(map(int, caps[0])), list(map(int, caps[1]))]
    return in_maps, dict(caps=caps_py, sched=sched, naux=naux, L=L, E=E,
                         Hp=Hp, backmaps=backmaps)


def _unpack_v7(res, meta):
    out = np.empty(meta["E"], dtype=np.float32)
    for c, (eids, slot) in enumerate(meta["backmaps"]):
        flat = np.asarray(res.results[c]["out"], dtype=np.float32).reshape(-1)
        tc_ = slot // P
        p = slot % P
        fidx = (tc_ // BCOLS) * (P * BCOLS) + p * BCOLS + (tc_ % BCOLS)
        out[eids] = flat[fidx]
    return out


# ---------------------------------------------------------------------------
# v1 (fallback): precomputed tables + per-column indirect DMA gathers
# ---------------------------------------------------------------------------

def _build_nc(C, NA, NB, Hp, repeat=1, repeat_pre=None, repeat_gather=None):
    """C: edge cols per core (edges = 128*C). NA/NB: padded table rows. Hp: # pos-w2 units.
    repeat>1 re-runs the compute phases (identical results) for slope-based timing."""
    f32 = mybir.dt.float32
    i32 = mybir.dt.int32
    nc = bacc.Bacc(None, target_bir_lowering=False)

    zTu = nc.dram_tensor("zTu", [P, NA], f32, kind="ExternalInput")
    zTm = nc.dram_tensor("zTm", [P, NB], f32, kind="ExternalInput")
    w1ut = nc.dram_tensor("w1ut", [P, H], f32, kind="ExternalInput")
    w1mt = nc.dram_tensor("w1mt", [P, H], f32, kind="ExternalInput")
    b1rep = nc.dram_tensor("b1rep", [P, H], f32, kind="ExternalInput")
    b2rep = nc.dram_tensor("b2rep", [P, 1], f32, kind="ExternalInput")
    idxA = nc.dram_tensor("idxA", [P, C], i32, kind="ExternalInput")
    idxB = nc.dram_tensor("idxB", [P, C], i32, kind="ExternalInput")
    out_d = nc.dram_tensor("out", [P, C], f32, kind="ExternalOutput")

    tabA = nc.dram_tensor("tabA", [NA, H], f32)
    tabB = nc.dram_tensor("tabB", [NB, H], f32)
    # tile-linearized write view: table row (p*(N/128) + m) <-> partition p, col block m
    tabA_v = tabA[:].rearrange("(p m) d -> p (m d)", p=P)
    tabB_v = tabB[:].rearrange("(p m) d -> p (m d)", p=P)

    with tile.TileContext(nc) as tc:
        with (
            tc.tile_pool(name="const", bufs=1) as cpool,
            tc.tile_pool(name="work", bufs=3) as wpool,
            tc.tile_pool(name="psum", bufs=4, space="PSUM") as ppool,
        ):
            w1ut_t = cpool.tile([P, H], f32)
            w1mt_t = cpool.tile([P, H], f32)
            b1rep_t = cpool.tile([P, H], f32)
            b2rep_t = cpool.tile([P, 1], f32)
            idxA_t = cpool.tile([P, C], i32)
            idxB_t = cpool.tile([P, C], i32)
            logits = cpool.tile([P, C], f32)
            nc.sync.dma_start(out=w1ut_t[:], in_=w1ut[:])
            nc.sync.dma_start(out=w1mt_t[:], in_=w1mt[:])
            nc.sync.dma_start(out=b1rep_t[:], in_=b1rep[:])
            nc.sync.dma_start(out=b2rep_t[:], in_=b2rep[:])
            nc.sync.dma_start(out=idxA_t[:], in_=idxA[:])
            nc.sync.dma_start(out=idxB_t[:], in_=idxB[:])

            # ---- precompute tables ----
            for (zT, w1t, tab_v, npad, addb1) in (
                (zTu, w1ut_t, tabA_v, NA, True),
                (zTm, w1mt_t, tabB_v, NB, False),
            ) * (repeat_pre if repeat_pre is not None else repeat):
                with tc.For_i(0, npad, ZBODY) as iv:
                    zstage = wpool.tile([P, ZBODY], f32, tag="zstage")
                    nc.sync.dma_start(out=zstage[:], in_=zT[:, bass.ds(iv, ZBODY)])
                    astage = wpool.tile([P, ZBODY], f32, tag="astage")
                    for k in range(ZBODY // P):
                        ps = ppool.tile([P, H], f32, tag="ps")
                        nc.tensor.matmul(
                            out=ps[:],
                            lhsT=zstage[:, k * P:(k + 1) * P],
                            rhs=w1t[:],
                            start=True, stop=True,
                        )
                        sl = astage[:, k * H:(k + 1) * H]
                        if addb1:
                            nc.vector.tensor_add(out=sl, in0=ps[:], in1=b1rep_t[:])
                        else:
                            nc.scalar.copy(out=sl, in_=ps[:])
                    nc.sync.dma_start(out=tab_v[:, bass.ds(iv, ZBODY)], in_=astage[:])

            # ---- edge gather + MLP ----
            def gather_body(iv):
                rstage = wpool.tile([P, G], i32, tag="rstage")
                cstage = wpool.tile([P, G], i32, tag="cstage")
                nc.vector.tensor_copy(out=rstage[:], in_=idxA_t[:, bass.ds(iv, G)])
                nc.vector.tensor_copy(out=cstage[:], in_=idxB_t[:, bass.ds(iv, G)])
                ct = wpool.tile([P, G * H], f32, tag="ct")
                for j in range(G):
                    sl = ct[:, j * H:(j + 1) * H]
                    nc.gpsimd.indirect_dma_start(
                        out=sl, out_offset=None, in_=tabA[:],
                        in_offset=bass.IndirectOffsetOnAxis(ap=rstage[:, j:j + 1], axis=0),
                    )
                    nc.gpsimd.indirect_dma_start(
                        out=sl, out_offset=None, in_=tabB[:],
                        in_offset=bass.IndirectOffsetOnAxis(ap=cstage[:, j:j + 1], axis=0),
                        compute_op=mybir.AluOpType.add,
                    )
                cc = ct[:].rearrange("p (g h) -> p g h", h=H)
                if Hp > 0:
                    nc.vector.tensor_scalar_max(out=cc[:, :, 0:Hp], in0=cc[:, :, 0:Hp], scalar1=0.0)
                if Hp < H:
                    nc.vector.tensor_scalar_min(out=cc[:, :, Hp:H], in0=cc[:, :, Hp:H], scalar1=0.0)
                lsl = logits[:, bass.ds(iv, G)]
                if Hp == H or Hp == 0:
                    nc.vector.tensor_reduce(out=lsl, in_=cc[:, :, :], axis=mybir.AxisListType.X,
                                            op=mybir.AluOpType.add)
                else:
                    pos = wpool.tile([P, G], f32, tag="pos")
                    nc.vector.tensor_reduce(out=pos[:], in_=cc[:, :, 0:Hp],
                                            axis=mybir.AxisListType.X, op=mybir.AluOpType.add)
                    neg = wpool.tile([P, G], f32, tag="neg")
                    nc.vector.tensor_reduce(out=neg[:], in_=cc[:, :, Hp:H],
                                            axis=mybir.AxisListType.X, op=mybir.AluOpType.add)
                    nc.vector.tensor_add(out=lsl, in0=pos[:], in1=neg[:])

            for _rep in range(repeat_gather if repeat_gather is not None else repeat):
                with tc.For_i(0, C, G) as iv:
                    gather_body(iv)

            # ---- sigmoid tail ----
            sig = cpool.tile([P, C], f32)
            nc.scalar.activation(out=sig[:], in_=logits[:],
                                 func=mybir.ActivationFunctionType.Sigmoid,
                                 bias=b2rep_t[:, 0:1], scale=1.0)
            nc.scalar.mul(out=sig[:], in_=sig[:], mul=5.0)
            nc.sync.dma_start(out=out_d[:], in_=sig[:])
    nc.finalize()
    return nc


def _pad_cols(n, mult):
    return ((n + mult - 1) // mult) * mult


def _prepare(z_user, z_movie, edge_index, W1, b1, W2, b2, n_cores=N_CORES):
    z_user = np.asarray(z_user, dtype=np.float32)
    z_movie = np.asarray(z_movie, dtype=np.float32)
    edge_index = np.asarray(edge_index)
    W1 = np.asarray(W1, dtype=np.float32)
    b1 = np.asarray(b1, dtype=np.float32)
    W2 = np.asarray(W2, dtype=np.float32)
    b2 = np.asarray(b2, dtype=np.float32)

    E = edge_index.shape[1]
    rows = edge_index[0].astype(np.int64)
    cols = edge_index[1].astype(np.int64)

    NAr = int(rows.max()) + 1 if E else 1          # referenced user rows
    NBr = z_movie.shape[0]
    NA = _pad_cols(max(NAr, ZBODY), ZBODY)
    NB = _pad_cols(max(NBr, ZBODY), ZBODY)

    # hidden permutation: positive-w2 units first; fold signed w2 and b1 into tables
    w2 = W2.reshape(-1)
    perm = np.argsort(w2 < 0, kind="stable")       # stable: positives (False) first
    Hp = int((w2 >= 0).sum())
    W1p = W1[perm]                                  # [H, 2H]
    b1p = b1[perm]
    scale = w2[perm]  # signed: w2*relu(x) = max0(w2*x) for w2>0, min0(w2*x) for w2<0
    w1ut = np.ascontiguousarray((W1p[:, :H] * scale[:, None]).T)   # [in, h]
    w1mt = np.ascontiguousarray((W1p[:, H:] * scale[:, None]).T)
    b1rep = np.tile(b1p * scale, (P, 1)).astype(np.float32)
    b2rep = np.full((P, 1), float(b2.reshape(-1)[0]), dtype=np.float32)

    # transposed, padded node features
    zTu = np.zeros((P, NA), dtype=np.float32)
    zTu[:, :NAr] = z_user[:NAr].T
    zTm = np.zeros((P, NB), dtype=np.float32)
    zTm[:, :NBr] = z_movie.T

    # tile-linearized table row index: u -> (u%128)*(N/128) + u//128
    mA, mB = NA // P, NB // P
    idxA_full = ((rows % P) * mA + rows // P).astype(np.int32)
    idxB_full = ((cols % P) * mB + cols // P).astype(np.int32)

    # shard edges: per core 128*C edges, C divisible by G
    C = _pad_cols(-(-E // (n_cores * P)), G)
    Epc = P * C
    Etot = n_cores * Epc
    idxA_pad = np.zeros(Etot, dtype=np.int32)
    idxA_pad[:E] = idxA_full
    idxB_pad = np.zeros(Etot, dtype=np.int32)
    idxB_pad[:E] = idxB_full

    in_maps = []
    for c in range(n_cores):
        sl = slice(c * Epc, (c + 1) * Epc)
        in_maps.append({
            "zTu": zTu, "zTm": zTm, "w1ut": w1ut, "w1mt": w1mt,
            "b1rep": b1rep, "b2rep": b2rep,
            "idxA": idxA_pad[sl].reshape(P, C),
            "idxB": idxB_pad[sl].reshape(P, C),
        })
    return in_maps, dict(C=C, NA=NA, NB=NB, Hp=Hp, E=E)


def kernel(z_user, z_movie, edge_index, W1, b1, W2, b2):
    import os
    if os.environ.get("EDGE_KERNEL_V4") == "1":  # correct but ~5x slower on HW than v3
        try:
            in_maps, meta = _prepare_v4(z_user, z_movie, edge_index, W1, b1, W2, b2)
            nc = _build_nc_v4(meta["nwin"], meta["lsp_caps"], meta["Hp"])
            res = run_bass_kernel_spmd(nc, in_maps, core_ids=list(range(N_CORES)))
            out = _unpack_v4(res, meta)
            _LAST_STATS.update(exec_time_ns=res.exec_time_ns, nc=nc,
                               in_maps=in_maps, meta=meta, version="v4")
            return out
        except Exception as e:
            import traceback
            traceback.print_exc()
            print(f"[kernel] v4 path failed ({type(e).__name__}: {e}); falling back to v3",
                  file=sys.stderr)
    if os.environ.get("EDGE_KERNEL_V7") == "1":
        try:
            in_maps, meta = _prepare_v7(z_user, z_movie, edge_index, W1, b1, W2, b2)
            nc = _build_nc_v7(meta["caps"], meta["Hp"], meta["sched"],
                              meta["naux"])
            res = run_bass_kernel_spmd(nc, in_maps, core_ids=list(range(N_CORES)))
            out = _unpack_v7(res, meta)
            _LAST_STATS.update(exec_time_ns=res.exec_time_ns, nc=nc,
                               in_maps=in_maps, meta=meta, version="v7")
            return out
        except Exception as e:
            import traceback
            traceback.print_exc()
            print(f"[kernel] v7 path failed ({type(e).__name__}: {e}); falling back",
                  file=sys.stderr)
    if os.environ.get("EDGE_KERNEL_V6", "1") == "1":
        try:
            in_maps, meta = _prepare_v6(z_user, z_movie, edge_index, W1, b1, W2, b2)
            nc = _build_nc_v6(meta["rows_u"], meta["rows_m"], meta["caps"],
                              meta["Hp"])
            res = run_bass_kernel_spmd(nc, in_maps, core_ids=list(range(N_CORES)))
            out = _unpack_v3(res, meta)
            _LAST_STATS.update(exec_time_ns=res.exec_time_ns, nc=nc,
                               in_maps=in_maps, meta=meta, version="v6")
            return out
        except Exception as e:
            import traceback
            traceback.print_exc()
            print(f"[kernel] v6 path failed ({type(e).__name__}: {e}); falling back to v3",
                  file=sys.stderr)
    if os.environ.get("EDGE_KERNEL_V1") != "1":
        try:
            in_maps, meta = _prepare_v3(z_user, z_movie, edge_index, W1, b1, W2, b2)
            nc = _build_nc_v3(meta["rows_u"], meta["rows_m"], meta["caps"], meta["Hp"])
            res = run_bass_kernel_spmd(nc, in_maps, core_ids=list(range(N_CORES)))
            out = _unpack_v3(res, meta)
            _LAST_STATS.update(exec_time_ns=res.exec_time_ns, nc=nc,
                               in_maps=in_maps, meta=meta, version="v3")
            return out
        except Exception as e:
            import traceback
            traceback.print_exc()
            print(f"[kernel] v3 path failed ({type(e).__name__}: {e}); falling back to v1",
                  file=sys.stderr)
    in_maps, meta = _prepare(z_user, z_movie, edge_index, W1, b1, W2, b2)
    nc = _build_nc(meta["C"], meta["NA"], meta["NB"], meta["Hp"])
    res = run_bass_kernel_spmd(nc, in_maps, core_ids=list(range(N_CORES)))
    out = np.concatenate([res.results[c]["out"].reshape(-1) for c in range(N_CORES)])
    _LAST_STATS.update(exec_time_ns=res.exec_time_ns, nc=nc,
                       in_maps=in_maps, meta=meta, version="v1")
    return out[:meta["E"]].astype(np.float32)


# ---------------------------------------------------------------------------
# v4: user-range sharding + movie-sorted windows; tabB streamed and expanded
# on PE via on-chip one-hot (colrep broadcast + DVE is_equal vs iota), tabA
# gathered per edge (non-transpose). Spill edges (window overflow) gather both
# tables. Tables precomputed on device in bf16 with w2/b1 folded.
# ---------------------------------------------------------------------------

WCAP = 640       # edge slots per 128-movie window (uniform across cores)
UPC = 12500      # users per core (100000 / 8)
NAC = 12800      # padded per-core tabA rows
NBP = 50176      # padded tabB rows (392 windows)


def _build_nc_v4(nwin, lsp_caps, Hp, ch=CH, repeat=1):
    """nwin: movie windows. lsp_caps: spill caps per movie-bank bucket
    (multiples of TILE). Hp: pos-w2 unit count."""
    f32 = mybir.dt.float32
    bf16 = mybir.dt.bfloat16
    i16 = mybir.dt.int16
    Lw = nwin * WCAP
    assert Lw % TILE == 0
    Lsp = int(sum(lsp_caps))
    L = Lw + Lsp
    ncols = L // P
    nblk = -(-ncols // BCOLS)
    ngrp = Lw // TILE

    nc = bacc.Bacc(None, target_bir_lowering=False,
                   dynamic_dma_scratch_size=32768)

    zTuc = nc.dram_tensor("zTuc", [P, NAC], bf16, kind="ExternalInput")
    zTmf = nc.dram_tensor("zTmf", [P, NBP], bf16, kind="ExternalInput")
    w1utF = nc.dram_tensor("w1utF", [H, H], bf16, kind="ExternalInput")
    w1mtF = nc.dram_tensor("w1mtF", [H, H], bf16, kind="ExternalInput")
    b1pre = nc.dram_tensor("b1pre", [P, GRP * H], f32, kind="ExternalInput")
    b2c = nc.dram_tensor("b2c", [P, 1], f32, kind="ExternalInput")
    iotas = nc.dram_tensor("iotas", [P, 2 * TILE], f32, kind="ExternalInput")
    ones1 = nc.dram_tensor("ones1", [1, P], bf16, kind="ExternalInput")
    colloc = nc.dram_tensor("colloc", [1, Lw], bf16, kind="ExternalInput")
    idxU = nc.dram_tensor("idxU", [P, L // 16], i16, kind="ExternalInput")
    idxMsp = nc.dram_tensor("idxMsp", [P, max(Lsp, 16) // 16], i16,
                            kind="ExternalInput")
    out_d = nc.dram_tensor("out", [nblk, P, BCOLS], f32, kind="ExternalOutput")

    tabA = nc.dram_tensor("tabA", [NAC, H], bf16)
    tabB = nc.dram_tensor("tabB", [NBP, H], bf16)
    # tabA is gather-only: store tile-linearized (row p*(NAC//P)+m <-> strip
    # node s*512+t*128+p at m = s*4+t); host linearizes gather indices.
    tabA_v = tabA[:].rearrange("(p m) h -> p (m h)", p=P)

    with tile.TileContext(nc) as tc:
        with (
            tc.tile_pool(name="const", bufs=1) as cpool,
            tc.tile_pool(name="pre", bufs=3) as prepool,
            tc.tile_pool(name="gat", bufs=2) as gpool,
            tc.tile_pool(name="idx", bufs=3) as ipool,
            tc.tile_pool(name="win", bufs=4) as wpool,
            tc.tile_pool(name="rel", bufs=4) as rpool,
            tc.tile_pool(name="lgb", bufs=2) as lpool,
            tc.tile_pool(name="obuf", bufs=2) as opool,
            tc.tile_pool(name="psum", bufs=3, space="PSUM") as ppool,
            tc.tile_pool(name="crps", bufs=2, space="PSUM") as crpool,
        ):
            nc.gpsimd.load_library(library_config.mlp)
            w1ut_t = cpool.tile([H, H], bf16)
            w1mt_t = cpool.tile([H, H], bf16)
            b1p_t = cpool.tile([P, GRP * H], f32)
            b2_t = cpool.tile([P, 1], f32)
            iota_t = cpool.tile([P, 2 * TILE], f32)
            ones_t = cpool.tile([1, P], bf16)
            nc.sync.dma_start(out=w1ut_t[:], in_=w1utF[:])
            nc.sync.dma_start(out=w1mt_t[:], in_=w1mtF[:])
            nc.sync.dma_start(out=b1p_t[:], in_=b1pre[:])
            nc.sync.dma_start(out=b2_t[:], in_=b2c[:])
            nc.sync.dma_start(out=iota_t[:], in_=iotas[:])
            nc.sync.dma_start(out=ones_t[:], in_=ones1[:])

            # ---- precompute tabA (b1 folded) and tabB ----
            for (zT, w1t, natural, npad, addb1) in (
                (zTuc, w1ut_t, False, NAC, True),
                (zTmf, w1mt_t, True, NBP, False),
            ):
                for s in range(npad // TILE):
                    zst = prepool.tile([P, TILE], bf16, tag="zst")
                    nc.sync.dma_start(out=zst[:],
                                      in_=zT[:, s * TILE:(s + 1) * TILE])
                    pps = ppool.tile([P, GRP * H], f32, tag="ps")
                    if addb1:
                        nc.scalar.copy(out=pps[:], in_=b1p_t[:])
                    for t in range(GRP):
                        nc.tensor.matmul(out=pps[:, t * H:(t + 1) * H],
                                         lhsT=zst[:, t * P:(t + 1) * P],
                                         rhs=w1t[:], start=not addb1,
                                         stop=True, skip_group_check=True)
                    ast = prepool.tile([P, GRP * H], bf16, tag="ast")
                    nc.scalar.copy(out=ast[:], in_=pps[:])
                    if natural:
                        nc.sync.dma_start(
                            out=tabB[s * TILE:(s + 1) * TILE, :].rearrange(
                                "(t p) h -> p t h", p=P),
                            in_=ast[:].rearrange("p (t h) -> p t h", h=H))
                    else:
                        nc.sync.dma_start(
                            out=tabA_v[:, s * GRP * H:(s + 1) * GRP * H],
                            in_=ast[:])

            for _rep in range(repeat):
                col = 0
                lg_blk = None

                def drain_block(c0):
                    nco = c0 % BCOLS if c0 % BCOLS else BCOLS
                    blk = (c0 - 1) // BCOLS
                    ot = opool.tile([P, BCOLS], f32, tag="ot")
                    nc.scalar.activation(
                        out=ot[:, 0:nco], in_=lg_blk[:, 0:nco],
                        func=mybir.ActivationFunctionType.Sigmoid,
                        bias=b2_t[:, 0:1], scale=1.0)
                    nc.scalar.mul(out=ot[:, 0:nco], in_=ot[:, 0:nco], mul=5.0)
                    nc.sync.dma_start(out=out_d[blk, :, 0:nco], in_=ot[:, 0:nco])

                def reduce_emit(yv, c4):
                    # yv: [P, GRP, H] bf16 view; write logits to lg_blk cols
                    if Hp > 0:
                        nc.vector.tensor_scalar_max(out=yv[:, :, 0:Hp],
                                                    in0=yv[:, :, 0:Hp],
                                                    scalar1=0.0)
                    if Hp < H:
                        nc.vector.tensor_scalar_min(out=yv[:, :, Hp:H],
                                                    in0=yv[:, :, Hp:H],
                                                    scalar1=0.0)
                    lgp = rpool.tile([P, GRP], f32, tag="lgp")
                    lgn = rpool.tile([P, GRP], f32, tag="lgn")
                    if Hp > 0:
                        nc.vector.tensor_reduce(out=lgp[:], in_=yv[:, :, 0:Hp],
                                                axis=mybir.AxisListType.X,
                                                op=mybir.AluOpType.add)
                    if Hp < H:
                        nc.vector.tensor_reduce(out=lgn[:], in_=yv[:, :, Hp:H],
                                                axis=mybir.AxisListType.X,
                                                op=mybir.AluOpType.add)
                    if Hp == H:
                        nc.vector.tensor_copy(out=lg_blk[:, c4:c4 + GRP], in_=lgp[:])
                    elif Hp == 0:
                        nc.vector.tensor_copy(out=lg_blk[:, c4:c4 + GRP], in_=lgn[:])
                    else:
                        nc.vector.tensor_add(out=lg_blk[:, c4:c4 + GRP],
                                             in0=lgp[:], in1=lgn[:])

                # ---- window region ----
                for gbase in range(0, ngrp, ch // TILE):
                    gend = min(gbase + ch // TILE, ngrp)
                    nsl = (gend - gbase) * TILE
                    s0 = gbase * TILE
                    iu_t = ipool.tile([P, ch // 16], i16, tag="iu")
                    nc.sync.dma_start(out=iu_t[:, 0:nsl // 16],
                                      in_=idxU[:, s0 // 16:(s0 + nsl) // 16])
                    at = gpool.tile([P, nsl], bf16, tag="at")
                    nc.gpsimd.dma_gather(
                        out_ap=at[:].rearrange("p (a n) -> p a n", a=nsl // P),
                        in_ap=tabA[:], idxs_ap=iu_t[:, 0:nsl // 16],
                        num_idxs=nsl, num_idxs_reg=nsl, elem_size=H,
                        transpose=False, queue_num=0, single_packet=False)
                    cl_t = ipool.tile([1, ch], bf16, tag="cl")
                    nc.sync.dma_start(out=cl_t[0:1, 0:nsl],
                                      in_=colloc[0:1, s0:s0 + nsl])
                    for g in range(gbase, gend):
                        w0 = (g * TILE) // WCAP
                        straddle = (g * TILE + TILE - 1) // WCAP > w0
                        wins = [w0, w0 + 1] if straddle and w0 + 1 < nwin else [w0]
                        go = (g - gbase) * TILE
                        crp = crpool.tile([P, TILE], f32, tag="cr")
                        nc.tensor.matmul(out=crp[:], lhsT=ones_t[:],
                                         rhs=cl_t[0:1, go:go + TILE],
                                         start=True, stop=True)
                        bps = ppool.tile([P, GRP * H], f32, tag="ps")
                        sks, tbws = [], []
                        for ki, w in enumerate(wins):
                            sk = wpool.tile([P, TILE], bf16, tag="sk")
                            nc.vector.tensor_tensor(
                                out=sk[:], in0=crp[:],
                                in1=iota_t[:, ki * TILE:(ki + 1) * TILE],
                                op=mybir.AluOpType.is_equal)
                            tbw = wpool.tile([P, H], bf16, tag="tbw")
                            nc.sync.dma_start(out=tbw[:],
                                              in_=tabB[w * P:(w + 1) * P, :])
                            sks.append(sk)
                            tbws.append(tbw)
                        for t in range(GRP):
                            for ki in range(len(wins)):
                                nc.tensor.matmul(
                                    out=bps[:, t * H:(t + 1) * H],
                                    lhsT=sks[ki][:, t * P:(t + 1) * P],
                                    rhs=tbws[ki][:],
                                    start=(ki == 0), stop=(ki == len(wins) - 1),
                                    skip_group_check=True)
                        y = rpool.tile([P, GRP * H], bf16, tag="y")
                        ab = (g - gbase) * GRP * H
                        nc.vector.tensor_add(out=y[:], in0=bps[:],
                                             in1=at[:, ab:ab + GRP * H])
                        if col % BCOLS == 0:
                            lg_blk = lpool.tile([P, BCOLS], f32, tag="lg")
                        reduce_emit(y[:].rearrange("p (g h) -> p g h", h=H),
                                    col % BCOLS)
                        col += GRP
                        if col % BCOLS == 0 or col == ncols:
                            drain_block(col)

                # ---- spill region: gather both tables ----
                for b, cap in enumerate(lsp_caps):
                    sbase = Lw + int(sum(lsp_caps[:b]))
                    pos = 0
                    while pos < cap:
                        cur = int(min(ch, cap - pos))
                        s0 = sbase + pos
                        iu_t = ipool.tile([P, ch // 16], i16, tag="iu")
                        nc.sync.dma_start(out=iu_t[:, 0:cur // 16],
                                          in_=idxU[:, s0 // 16:(s0 + cur) // 16])
                        im_t = ipool.tile([P, ch // 16], i16, tag="im")
                        nc.sync.dma_start(
                            out=im_t[:, 0:cur // 16],
                            in_=idxMsp[:, (s0 - Lw) // 16:(s0 - Lw + cur) // 16])
                        at = gpool.tile([P, cur], bf16, tag="at")
                        nc.gpsimd.dma_gather(
                            out_ap=at[:].rearrange("p (a n) -> p a n", a=cur // P),
                            in_ap=tabA[:], idxs_ap=iu_t[:, 0:cur // 16],
                            num_idxs=cur, num_idxs_reg=cur, elem_size=H,
                            transpose=False, queue_num=0, single_packet=False)
                        bt = gpool.tile([P, cur], bf16, tag="bt")
                        nc.gpsimd.dma_gather(
                            out_ap=bt[:].rearrange("p (a n) -> p a n", a=cur // P),
                            in_ap=tabB[min(b * BANK, NBP - P):min((b + 1) * BANK, NBP), :],
                            idxs_ap=im_t[:, 0:cur // 16],
                            num_idxs=cur, num_idxs_reg=cur, elem_size=H,
                            transpose=False, queue_num=0, single_packet=False)
                        for g in range(cur // TILE):
                            go = g * TILE
                            y = rpool.tile([P, GRP * H], bf16, tag="y")
                            ab = g * GRP * H
                            nc.vector.tensor_add(out=y[:],
                                                 in0=at[:, ab:ab + GRP * H],
                                                 in1=bt[:, ab:ab + GRP * H])
                            if col % BCOLS == 0:
                                lg_blk = lpool.tile([P, BCOLS], f32, tag="lg")
                            reduce_emit(y[:].rearrange("p (g h) -> p g h", h=H),
                                        col % BCOLS)
                            col += GRP
                            if col % BCOLS == 0 or col == ncols:
                                drain_block(col)
                        pos += cur
    nc.finalize()
    return nc


def _prepare_v4(z_user, z_movie, edge_index, W1, b1, W2, b2,
                n_cores=N_CORES, upc=UPC, wcap=WCAP):
    import ml_dtypes
    bf16 = ml_dtypes.bfloat16
    z_user = np.asarray(z_user, dtype=np.float32)
    z_movie = np.asarray(z_movie, dtype=np.float32)
    edge_index = np.asarray(edge_index)
    W1 = np.asarray(W1, dtype=np.float32)
    b1 = np.asarray(b1, dtype=np.float32)
    W2 = np.asarray(W2, dtype=np.float32)
    b2 = np.asarray(b2, dtype=np.float32)
    E = edge_index.shape[1]
    rows = edge_index[0].astype(np.int64)
    cols = edge_index[1].astype(np.int64)
    NM = z_movie.shape[0]
    nwin = NBP // P
    assert NM <= NBP and z_user.shape[0] <= n_cores * upc

    w2 = W2.reshape(-1)
    perm = np.argsort(w2 < 0, kind="stable")
    Hp = int((w2 >= 0).sum())
    w2sc = w2[perm]                  # signed: max0 pos-range, min0 neg-range
    W1p = W1[perm] * w2sc[:, None]
    b1p = b1[perm] * w2sc

    zmT = np.zeros((P, NBP), dtype=bf16)
    zmT[:, :NM] = z_movie.T.astype(bf16)
    shared = {"zTmf": zmT,
              "w1utF": np.ascontiguousarray(W1p[:, :H].T).astype(bf16),
              "w1mtF": np.ascontiguousarray(W1p[:, H:].T).astype(bf16),
              "b1pre": np.ascontiguousarray(np.tile(b1p, (P, GRP)).astype(np.float32)),
              "b2c": np.full((P, 1), float(b2.reshape(-1)[0]), np.float32),
              "iotas": np.ascontiguousarray(np.concatenate(
                  [np.tile(np.arange(P, dtype=np.float32)[:, None], (1, TILE)),
                   np.tile(np.arange(P, 2 * P, dtype=np.float32)[:, None], (1, TILE))],
                  axis=1)),
              "ones1": np.ones((1, P), dtype=bf16)}

    core_ids = rows // upc
    Lw = nwin * wcap
    per_core = []
    spill_cnt = np.zeros((n_cores, 2), dtype=np.int64)
    for c in range(n_cores):
        m = core_ids == c
        eids = np.nonzero(m)[0]
        r, co = rows[eids], cols[eids]
        order = np.argsort(co, kind="stable")
        eids, r, co = eids[order], r[order], co[order]
        win = co // P
        wstart = np.searchsorted(win, np.arange(nwin))
        wend = np.searchsorted(win, np.arange(nwin), side="right")
        k = np.arange(len(co)) - wstart[win]
        in_window = k < wcap
        spill_bank = (co // BANK).astype(np.int64)
        for bk in range(2):
            spill_cnt[c, bk] = int(np.count_nonzero(~in_window & (spill_bank == bk)))
        per_core.append((eids, r, co, win, k, in_window, spill_bank))

    lsp_caps = [int(_roundup(max(int(spill_cnt[:, bk].max()), 1), TILE))
                for bk in range(2)]
    Lsp = sum(lsp_caps)
    L = Lw + Lsp

    # static group->w0 for collocal encoding
    slot_arr = np.arange(Lw)
    grp_w0 = (slot_arr // TILE * TILE) // wcap     # w0 of each slot's group

    in_maps, backmaps = [], []
    for c in range(n_cores):
        eids, r, co, win, k, in_window, spill_bank = per_core[c]
        iu = np.zeros(L, np.int16)
        clv = np.full(Lw, 512.0, np.float32)
        imsp = np.zeros(max(Lsp, 16), np.int16)
        slot = np.empty(len(eids), np.int64)
        # window slots
        mA = NAC // P
        def lin(u):
            return ((u % P) * mA + u // P).astype(np.int16)
        wi = np.nonzero(in_window)[0]
        ws = win[wi] * wcap + k[wi]
        slot[wi] = ws
        iu[ws] = lin(r[wi] - c * upc)
        clv[ws] = (co[wi] - grp_w0[ws] * P).astype(np.float32)
        # spill slots
        off = 0
        for bk in range(2):
            si = np.nonzero(~in_window & (spill_bank == bk))[0]
            ss = Lw + off + np.arange(len(si))
            slot[si] = ss
            iu[ss] = lin(r[si] - c * upc)
            imsp[ss - Lw] = (co[si] % BANK).astype(np.int16)
            off += lsp_caps[bk]
        zuT = np.zeros((P, NAC), dtype=bf16)
        ncr = min((c + 1) * upc, z_user.shape[0]) - c * upc
        zuT[:, :ncr] = z_user[c * upc:c * upc + ncr].T.astype(bf16)
        iu_w = np.ascontiguousarray(np.tile(iu.reshape(L // 16, 16).T, (8, 1)))
        im_w = np.ascontiguousarray(
            np.tile(imsp.reshape(len(imsp) // 16, 16).T, (8, 1)))
        in_maps.append({**shared, "zTuc": zuT,
                        "colloc": np.ascontiguousarray(clv[None, :]).astype(bf16),
                        "idxU": iu_w, "idxMsp": im_w})
        backmaps.append((eids, slot))
    return in_maps, dict(nwin=nwin, lsp_caps=lsp_caps, L=L, E=E, Hp=Hp,
                         backmaps=backmaps)


def _unpack_v4(res, meta):
    out = np.empty(meta["E"], dtype=np.float32)
    for c, (eids, slot) in enumerate(meta["backmaps"]):
        flat = np.asarray(res.results[c]["out"], dtype=np.float32).reshape(-1)
        tc_ = slot // P
        p = slot % P
        fidx = (tc_ // BCOLS) * (P * BCOLS) + p * BCOLS + (tc_ % BCOLS)
        out[eids] = flat[fidx]
    return out



# revision 42
# speedup vs baseline: 3.7445x; 1.1997x over previous
"""EdgeDecoder Trainium2 kernel: out = 5*sigmoid(w2 . relu([z_u[row]; z_m[col]] @ W1.T + b1) + b2).

v3 strategy (8 NeuronCores, data-parallel over edges):
  No precomputed node tables. Per edge, gather the raw bf16 z_user[row] and
  z_movie[col] rows straight from HBM with batched dma_gather(transpose=True)
  (one SWDGE instruction per 2048 edges instead of one indirect DMA per 128
  edges), which lands z-components on partitions. The gathered tiles feed the
  PE as the *stationary* operand so edges land on PSUM partitions: per
  512-edge group (4 tiles of 128 edges sharing one PSUM bank), ACT preloads
  b1*w2 into the bank, 8 matmuls (4 tiles x {W1u', W1m'}) accumulate on top
  (W1 columns pre-scaled by |w2| with positive-w2 hidden units permuted
  first), one ACT relu drains the bank to SBUF, and DVE does two free-dim
  tensor_reduces (pos / neg ranges) + subtract -> per-edge logits in an SBUF
  block. Every 512 tile-columns: ACT sigmoid(+b2)*5 and one DMA out.

  dma_gather indices are int16, so node tables are split into <=32768-row
  banks and edges are bucketed by (user-bank, movie-bank) on the host; bucket
  capacities are padded to the max across cores so all 8 cores share one
  compiled program.

v1 (fallback): precomputed A/B tables + per-128-edge indirect DMA gathers.
"""
import sys
import numpy as np

sys.path.insert(0, '/opt/trn_rl_repo')

import concourse.bass as bass
import concourse.bacc as bacc
import concourse.mybir as mybir
import concourse.tile as tile
from concourse import library_config
from concourse.bass_utils import run_bass_kernel_spmd

N_CORES = 8
P = 128
H = 128          # hidden
BANK = 32768     # rows per gather bank (int16 index limit)
CH = 8192        # edges per dma_gather call
TILE = 512       # edges per PE tile (psum bank = 512 f32)
G = 32           # v1: gather-loop cols per iteration
ZBODY = 1024     # v1: precompute rows per loop body

_LAST_STATS = {}


# ---------------------------------------------------------------------------
# v3
# ---------------------------------------------------------------------------

GRP = 4          # 128-edge tiles per PSUM group (group = 512 edges = 1 bank)
BCOLS = 512      # logit-block tile-columns (block = 65536 edges)


def _build_nc_v3(rows_u, rows_m, caps, Hp, ch=CH, repeat=1):
    """rows_u/rows_m: rows per user/movie bank. caps: per-bucket edge capacity
    (each a multiple of TILE; bucket b = ubank*len(rows_m) + mbank).
    Hp: # hidden units with w2 >= 0 (after the pos-first permutation)."""
    f32 = mybir.dt.float32
    bf16 = mybir.dt.bfloat16
    i16 = mybir.dt.int16
    nbM = len(rows_m)
    L = int(sum(caps))
    ncols = L // P                   # total tile-columns
    nblk = -(-ncols // BCOLS)

    import os
    dbg_no_gather = os.environ.get("EDGE_V3_NO_GATHER") == "1"
    dbg_no_preload = os.environ.get("EDGE_V3_NO_PRELOAD") == "1"
    dbg_no_compute = os.environ.get("EDGE_V3_NO_COMPUTE") == "1"
    dbg_two_queue = os.environ.get("EDGE_V3_TWO_QUEUE") == "1"

    nc = bacc.Bacc(None, target_bir_lowering=False,
                   dynamic_dma_scratch_size=32768, num_swdge_queues=4)

    if dbg_no_gather:
        zdummy = nc.dram_tensor("zdummy", [P, ch], bf16, kind="ExternalInput")
    zu_b = [nc.dram_tensor(f"zu{i}", [r, H], bf16, kind="ExternalInput")
            for i, r in enumerate(rows_u)]
    zm_b = [nc.dram_tensor(f"zm{i}", [r, H], bf16, kind="ExternalInput")
            for i, r in enumerate(rows_m)]
    w1ut = nc.dram_tensor("w1ut", [H, H], bf16, kind="ExternalInput")
    w1mt = nc.dram_tensor("w1mt", [H, H], bf16, kind="ExternalInput")
    b1r4 = nc.dram_tensor("b1r4", [P, GRP * H], f32, kind="ExternalInput")
    b2c = nc.dram_tensor("b2c", [P, 1], f32, kind="ExternalInput")
    idxU = nc.dram_tensor("idxU", [P, L // 16], i16, kind="ExternalInput")
    idxM = nc.dram_tensor("idxM", [P, L // 16], i16, kind="ExternalInput")
    out_d = nc.dram_tensor("out", [nblk, P, BCOLS], f32, kind="ExternalOutput")

    with tile.TileContext(nc) as tc:
        with (
            tc.tile_pool(name="const", bufs=1) as cpool,
            tc.tile_pool(name="gat", bufs=4) as gpool,
            tc.tile_pool(name="idx", bufs=4) as ipool,
            tc.tile_pool(name="rel", bufs=4) as rpool,
            tc.tile_pool(name="lgb", bufs=2) as lpool,
            tc.tile_pool(name="obuf", bufs=2) as opool,
            tc.tile_pool(name="psum", bufs=4, space="PSUM") as ppool,
        ):
            nc.gpsimd.load_library(library_config.mlp)
            w1ut_t = cpool.tile([H, H], bf16)
            w1mt_t = cpool.tile([H, H], bf16)
            b1r4_t = cpool.tile([P, GRP * H], f32)
            b2_t = cpool.tile([P, 1], f32)
            nc.sync.dma_start(out=w1ut_t[:], in_=w1ut[:])
            nc.sync.dma_start(out=w1mt_t[:], in_=w1mt[:])
            nc.sync.dma_start(out=b1r4_t[:], in_=b1r4[:])
            nc.sync.dma_start(out=b2_t[:], in_=b2c[:])

            for _rep in range(repeat):
                col = 0              # global tile-column index
                lg_blk = None

                def drain_block(c0):
                    nco = c0 % BCOLS if c0 % BCOLS else BCOLS
                    blk = (c0 - 1) // BCOLS
                    ot = opool.tile([P, BCOLS], f32, tag="ot")
                    nc.scalar.activation(
                        out=ot[:, 0:nco], in_=lg_blk[:, 0:nco],
                        func=mybir.ActivationFunctionType.Sigmoid,
                        bias=b2_t[:, 0:1], scale=1.0)
                    nc.scalar.mul(out=ot[:, 0:nco], in_=ot[:, 0:nco], mul=5.0)
                    nc.sync.dma_start(out=out_d[blk, :, 0:nco], in_=ot[:, 0:nco])

                chunk_no = 0
                for b, cap in enumerate(caps):
                    bu, bm = divmod(b, nbM)
                    base = int(sum(caps[:b]))
                    pos = 0
                    while pos < cap:
                        cur = int(min(ch, cap - pos))
                        o16 = (base + pos) // 16
                        iu_t = ipool.tile([P, ch // 16], i16, tag="iu")
                        im_t = ipool.tile([P, ch // 16], i16, tag="im")
                        nc.sync.dma_start(out=iu_t[:, 0:cur // 16],
                                          in_=idxU[:, o16:o16 + cur // 16])
                        nc.sync.dma_start(out=im_t[:, 0:cur // 16],
                                          in_=idxM[:, o16:o16 + cur // 16])
                        ut = gpool.tile([P, cur], bf16, tag="ut")
                        mt = gpool.tile([P, cur], bf16, tag="mt")
                        if dbg_no_gather:
                            nc.sync.dma_start(out=ut[:], in_=zdummy[:, 0:cur])
                            nc.sync.dma_start(out=mt[:], in_=zdummy[:, 0:cur])
                        else:
                            # NOTE: transpose gathers corrupt data when run
                            # concurrently on multiple queues (shared xbar
                            # scratch) - keep both on queue 0.
                            nc.gpsimd.dma_gather(
                                out_ap=ut[:].rearrange("p (a n) -> p a n", a=1),
                                in_ap=zu_b[bu][:],
                                idxs_ap=iu_t[:, 0:cur // 16],
                                num_idxs=cur, num_idxs_reg=cur, elem_size=H,
                                transpose=True, queue_num=0,
                                single_packet=False)
                            nc.gpsimd.dma_gather(
                                out_ap=mt[:].rearrange("p (a n) -> p a n", a=1),
                                in_ap=zm_b[bm][:],
                                idxs_ap=im_t[:, 0:cur // 16],
                                num_idxs=cur, num_idxs_reg=cur, elem_size=H,
                                transpose=True, queue_num=0,
                                single_packet=False)
                        chunk_no += 1
                        for g in range(cur // TILE):
                            if col % BCOLS == 0:
                                lg_blk = lpool.tile([P, BCOLS], f32, tag="lg")
                            if dbg_no_compute:
                                col += GRP
                                continue
                            ps = ppool.tile([P, GRP * H], f32, tag="ps")
                            if not dbg_no_preload:
                                nc.scalar.copy(out=ps[:], in_=b1r4_t[:])
                            for t in range(GRP):
                                e0 = (g * GRP + t) * P
                                nc.tensor.matmul(
                                    out=ps[:, t * H:(t + 1) * H],
                                    lhsT=ut[:, e0:e0 + P], rhs=w1ut_t[:],
                                    start=dbg_no_preload, stop=False,
                                    skip_group_check=True)
                                nc.tensor.matmul(
                                    out=ps[:, t * H:(t + 1) * H],
                                    lhsT=mt[:, e0:e0 + P], rhs=w1mt_t[:],
                                    start=False, stop=True,
                                    skip_group_check=True)
                            rl = rpool.tile([P, GRP * H], bf16, tag="rl")
                            nc.scalar.activation(
                                out=rl[:], in_=ps[:],
                                func=mybir.ActivationFunctionType.Relu)
                            rv = rl[:].rearrange("p (g h) -> p g h", h=H)
                            c4 = col % BCOLS
                            if Hp == H:
                                nc.vector.tensor_reduce(
                                    out=lg_blk[:, c4:c4 + GRP], in_=rv[:, :, :],
                                    axis=mybir.AxisListType.X,
                                    op=mybir.AluOpType.add)
                            else:
                                lgp = rpool.tile([P, GRP], f32, tag="lgp")
                                lgn = rpool.tile([P, GRP], f32, tag="lgn")
                                if Hp > 0:
                                    nc.vector.tensor_reduce(
                                        out=lgp[:], in_=rv[:, :, 0:Hp],
                                        axis=mybir.AxisListType.X,
                                        op=mybir.AluOpType.add)
                                nc.vector.tensor_reduce(
                                    out=lgn[:], in_=rv[:, :, Hp:H],
                                    axis=mybir.AxisListType.X,
                                    op=mybir.AluOpType.add)
                                if Hp > 0:
                                    nc.vector.tensor_sub(
                                        out=lg_blk[:, c4:c4 + GRP],
                                        in0=lgp[:], in1=lgn[:])
                                else:
                                    nc.vector.tensor_scalar_mul(
                                        out=lg_blk[:, c4:c4 + GRP],
                                        in0=lgn[:], scalar1=-1.0)
                            col += GRP
                            if col % BCOLS == 0 or col == ncols:
                                drain_block(col)
                        pos += cur
    nc.finalize()
    return nc


def _roundup(n, m):
    return ((n + m - 1) // m) * m


def _prepare_v3(z_user, z_movie, edge_index, W1, b1, W2, b2,
                n_cores=N_CORES, bank=BANK):
    import ml_dtypes
    bf16 = ml_dtypes.bfloat16
    z_user = np.asarray(z_user, dtype=np.float32)
    z_movie = np.asarray(z_movie, dtype=np.float32)
    edge_index = np.asarray(edge_index)
    W1 = np.asarray(W1, dtype=np.float32)
    b1 = np.asarray(b1, dtype=np.float32)
    W2 = np.asarray(W2, dtype=np.float32)
    b2 = np.asarray(b2, dtype=np.float32)

    E = edge_index.shape[1]
    rows = edge_index[0].astype(np.int64)
    cols = edge_index[1].astype(np.int64)
    NU, NM = z_user.shape[0], z_movie.shape[0]
    nbU, nbM = -(-NU // bank), -(-NM // bank)
    nbkt = nbU * nbM
    Epc = -(-E // n_cores)

    per_core = []
    cnts = np.zeros((n_cores, nbkt), dtype=np.int64)
    for c in range(n_cores):
        sl = slice(c * Epc, min((c + 1) * Epc, E))
        r, co = rows[sl], cols[sl]
        bkt = (r // bank) * nbM + (co // bank)
        order = np.argsort(bkt, kind="stable")
        cnts[c] = np.bincount(bkt, minlength=nbkt)
        per_core.append((sl, order, r, co, bkt))

    caps = np.maximum(_roundup(cnts.max(axis=0), TILE), TILE)
    offs = np.concatenate([[0], np.cumsum(caps)])
    L = int(offs[-1])

    # permute hidden units w2>=0 first; fold |w2| into W1 rows and b1.
    # logit = sum_pos relu(|w2|y) - sum_neg relu(|w2|y)
    w2 = W2.reshape(-1)
    perm = np.argsort(w2 < 0, kind="stable")
    Hp = int((w2 >= 0).sum())
    w2sc = np.abs(w2[perm])
    W1p = W1[perm] * w2sc[:, None]          # [h', 2H]
    b1p = b1[perm] * w2sc                   # [h']

    zu16 = np.ascontiguousarray(z_user.astype(bf16))
    zm16 = np.ascontiguousarray(z_movie.astype(bf16))
    shared = {"w1ut": np.ascontiguousarray(W1p[:, :H].T).astype(bf16),
              "w1mt": np.ascontiguousarray(W1p[:, H:].T).astype(bf16),
              "b1r4": np.ascontiguousarray(
                  np.tile(b1p, (P, GRP)).astype(np.float32)),
              "b2c": np.full((P, 1), float(b2.reshape(-1)[0]), np.float32)}
    rows_u, rows_m = [], []
    for i in range(nbU):
        bk = np.ascontiguousarray(zu16[i * bank:(i + 1) * bank])
        shared[f"zu{i}"] = bk
        rows_u.append(bk.shape[0])
    for i in range(nbM):
        bk = np.ascontiguousarray(zm16[i * bank:(i + 1) * bank])
        shared[f"zm{i}"] = bk
        rows_m.append(bk.shape[0])

    in_maps, backmaps = [], []
    for c in range(n_cores):
        sl, order, r, co, bkt = per_core[c]
        n = len(r)
        starts = np.concatenate([[0], np.cumsum(cnts[c])])
        sorted_bkt = bkt[order]
        k = np.arange(n) - starts[sorted_bkt]
        spos = offs[sorted_bkt] + k          # slot of edge order[i]
        iu = np.zeros(L, np.int16)
        im = np.zeros(L, np.int16)
        iu[spos] = (r[order] % bank).astype(np.int16)
        im[spos] = (co[order] % bank).astype(np.int16)
        slot = np.empty(n, np.int64)
        slot[order] = spos
        iu_w = np.ascontiguousarray(np.tile(iu.reshape(L // 16, 16).T, (8, 1)))
        im_w = np.ascontiguousarray(np.tile(im.reshape(L // 16, 16).T, (8, 1)))
        in_maps.append({**shared, "idxU": iu_w, "idxM": im_w})
        backmaps.append((sl, slot))
    return in_maps, dict(rows_u=rows_u, rows_m=rows_m,
                         caps=[int(x) for x in caps], L=L, E=E, Hp=Hp,
                         backmaps=backmaps)


def _unpack_v3(res, meta):
    out = np.empty(meta["E"], dtype=np.float32)
    for c, (sl, slot) in enumerate(meta["backmaps"]):
        flat = np.asarray(res.results[c]["out"], dtype=np.float32).reshape(-1)
        # edge at stream slot s -> tile-column s//128, partition s%128;
        # out tensor is [nblk, 128, BCOLS]
        tc_ = slot // P
        p = slot % P
        fidx = (tc_ // BCOLS) * (P * BCOLS) + p * BCOLS + (tc_ % BCOLS)
        out[sl] = flat[fidx]
    return out


# ---------------------------------------------------------------------------
# v6: device-precomputed A/B node tables (A=W1u z_u + b1, B=W1m z_m, |w2|
# folded, pos-w2-first permutation) + per-edge dual NON-transpose dma_gather
# spread over all 4 SWDGE queues (measured: 1 queue = 7.9 ns/row, 4 queues =
# 1.79 ns/row). Edges land on partitions, H on free dim: DVE add, ACT relu,
# DVE pos/neg reduces -> logits. No per-edge PE work.
# Slot mapping identical to v3 (slot s -> partition s%128, tile-col s//128).
# ---------------------------------------------------------------------------

NT = 50176       # referenced node rows padded to 98*512 (indices < 50000)


def _build_nc_v6(rows_u, rows_m, caps, Hp, ch=CH, repeat=1):
    """rows_u/rows_m: rows per user/movie table bank (sum = NT each).
    caps: per-bucket edge capacity (multiples of TILE; bucket b =
    ubank*len(rows_m) + mbank). Hp: # hidden units with w2 >= 0."""
    f32 = mybir.dt.float32
    bf16 = mybir.dt.bfloat16
    i16 = mybir.dt.int16
    nbM = len(rows_m)
    L = int(sum(caps))
    ncols = L // P                   # total tile-columns
    nblk = -(-ncols // BCOLS)
    assert 0 < Hp < H

    import os
    dbg_no_gather = os.environ.get("EDGE_V6_NO_GATHER") == "1"
    dbg_no_precomp = os.environ.get("EDGE_V6_NO_PRECOMP") == "1"
    dbg_no_compute = os.environ.get("EDGE_V6_NO_COMPUTE") == "1"

    nc = bacc.Bacc(None, target_bir_lowering=False,
                   dynamic_dma_scratch_size=32768, num_swdge_queues=4)

    zuT = nc.dram_tensor("zuT", [P, NT], bf16, kind="ExternalInput")
    zmT = nc.dram_tensor("zmT", [P, NT], bf16, kind="ExternalInput")
    w1ut = nc.dram_tensor("w1ut", [H, H], bf16, kind="ExternalInput")
    w1mt = nc.dram_tensor("w1mt", [H, H], bf16, kind="ExternalInput")
    b1r4 = nc.dram_tensor("b1r4", [P, GRP * H], f32, kind="ExternalInput")
    b2c = nc.dram_tensor("b2c", [P, 1], f32, kind="ExternalInput")
    idxU = nc.dram_tensor("idxU", [P, L // 16], i16, kind="ExternalInput")
    idxM = nc.dram_tensor("idxM", [P, L // 16], i16, kind="ExternalInput")
    out_d = nc.dram_tensor("out", [nblk, P, BCOLS], f32, kind="ExternalOutput")

    tabs_u = [nc.dram_tensor(f"tabU{i}", [r, H], bf16) for i, r in enumerate(rows_u)]
    tabs_m = [nc.dram_tensor(f"tabM{i}", [r, H], bf16) for i, r in enumerate(rows_m)]

    with tile.TileContext(nc) as tc:
        with (
            tc.tile_pool(name="const", bufs=1) as cpool,
            tc.tile_pool(name="pre", bufs=2) as prepool,
            tc.tile_pool(name="gat", bufs=3) as gpool,
            tc.tile_pool(name="idx", bufs=4) as ipool,
            tc.tile_pool(name="rel", bufs=2) as rpool,
            tc.tile_pool(name="lgs", bufs=3) as spool,
            tc.tile_pool(name="lgb", bufs=2) as lpool,
            tc.tile_pool(name="obuf", bufs=2) as opool,
            tc.tile_pool(name="psum", bufs=4, space="PSUM") as ppool,
        ):
            nc.gpsimd.load_library(library_config.mlp)
            w1ut_t = cpool.tile([H, H], bf16)
            w1mt_t = cpool.tile([H, H], bf16)
            b1r4_t = cpool.tile([P, GRP * H], f32)
            b2_t = cpool.tile([P, 1], f32)
            nc.sync.dma_start(out=w1ut_t[:], in_=w1ut[:])
            nc.sync.dma_start(out=w1mt_t[:], in_=w1mt[:])
            nc.sync.dma_start(out=b1r4_t[:], in_=b1r4[:])
            nc.sync.dma_start(out=b2_t[:], in_=b2c[:])

            for _rep in range(repeat):
                # ---- precompute node tables (bank-interleaved U0,M0,U1,M1
                # so bucket (0,0) gathers can start early) ----
                gno = 0
                ZB = 4096            # z columns staged per DMA (8 groups)
                for bi in range(len(rows_u) if not dbg_no_precomp else 0):
                    for (zT, w1t, tabs, rows, addb1, goff) in (
                        (zuT, w1ut_t, tabs_u, rows_u, True, 0),
                        (zmT, w1mt_t, tabs_m, rows_m, False, 0),
                    ):
                        base = int(sum(rows[:bi]))
                        for z0 in range(0, rows[bi], ZB):
                            zn = min(ZB, rows[bi] - z0)
                            zbig = prepool.tile([P, ZB], bf16, tag="zst")
                            nc.sync.dma_start(
                                out=zbig[:, 0:zn],
                                in_=zT[:, base + z0:base + z0 + zn])
                            for s in range(zn // TILE):
                                so = z0 // TILE + s
                                pps = ppool.tile([P, GRP * H], f32, tag="ps")
                                if addb1:
                                    nc.scalar.copy(out=pps[:], in_=b1r4_t[:])
                                for t in range(GRP):
                                    nc.tensor.matmul(
                                        out=pps[:, t * H:(t + 1) * H],
                                        lhsT=zbig[:, s * TILE + t * P:
                                                  s * TILE + (t + 1) * P],
                                        rhs=w1t[:], start=not addb1,
                                        stop=True, skip_group_check=True)
                                ast = prepool.tile([P, GRP * H], bf16,
                                                   tag="ast")
                                if gno % 2 == 0:
                                    nc.scalar.copy(out=ast[:], in_=pps[:])
                                else:
                                    nc.vector.tensor_copy(out=ast[:],
                                                          in_=pps[:])
                                gno += 1
                                nc.sync.dma_start(
                                    out=tabs[bi][so * TILE:(so + 1) * TILE, :]
                                    .rearrange("(t p) h -> p t h", p=P),
                                    in_=ast[:].rearrange("p (t h) -> p t h",
                                                         h=H))

                # ---- edge phase ----
                col = 0              # global tile-column index
                lg_blk = None

                def drain_block(c0):
                    nco = c0 % BCOLS if c0 % BCOLS else BCOLS
                    blk = (c0 - 1) // BCOLS
                    ot = opool.tile([P, BCOLS], f32, tag="ot")
                    nc.scalar.activation(
                        out=ot[:, 0:nco], in_=lg_blk[:, 0:nco],
                        func=mybir.ActivationFunctionType.Sigmoid,
                        bias=b2_t[:, 0:1], scale=1.0)
                    nc.scalar.mul(out=ot[:, 0:nco], in_=ot[:, 0:nco], mul=5.0)
                    nc.sync.dma_start(out=out_d[blk, :, 0:nco], in_=ot[:, 0:nco])

                chunk_no = 0
                for b, cap in enumerate(caps):
                    bu, bm = divmod(b, nbM)
                    base = int(sum(caps[:b]))
                    pos = 0
                    while pos < cap:
                        cur = int(min(ch, cap - pos))
                        o16 = (base + pos) // 16
                        na = cur // P        # tile-cols in this chunk
                        iu_t = ipool.tile([P, ch // 16], i16, tag="iu")
                        im_t = ipool.tile([P, ch // 16], i16, tag="im")
                        nc.sync.dma_start(out=iu_t[:, 0:cur // 16],
                                          in_=idxU[:, o16:o16 + cur // 16])
                        nc.sync.dma_start(out=im_t[:, 0:cur // 16],
                                          in_=idxM[:, o16:o16 + cur // 16])
                        ut = gpool.tile([P, ch], bf16, tag="ut")
                        mt = gpool.tile([P, ch], bf16, tag="mt")
                        if not dbg_no_gather:
                            nc.gpsimd.dma_gather(
                                out_ap=ut[:, 0:cur].rearrange(
                                    "p (a n) -> p a n", a=na),
                                in_ap=tabs_u[bu][:],
                                idxs_ap=iu_t[:, 0:cur // 16],
                                num_idxs=cur, num_idxs_reg=cur, elem_size=H,
                                transpose=False,
                                queue_num=(2 * chunk_no) % 4,
                                single_packet=False)
                            nc.gpsimd.dma_gather(
                                out_ap=mt[:, 0:cur].rearrange(
                                    "p (a n) -> p a n", a=na),
                                in_ap=tabs_m[bm][:],
                                idxs_ap=im_t[:, 0:cur // 16],
                                num_idxs=cur, num_idxs_reg=cur, elem_size=H,
                                transpose=False,
                                queue_num=(2 * chunk_no + 1) % 4,
                                single_packet=False)
                        chunk_no += 1
                        if dbg_no_compute:
                            col += na
                            if col % BCOLS == 0 or col >= ncols:
                                pass
                            pos += cur
                            continue
                        yt = rpool.tile([P, ch], bf16, tag="yt")
                        nc.vector.tensor_add(out=yt[:, 0:cur], in0=ut[:, 0:cur],
                                             in1=mt[:, 0:cur])
                        nc.scalar.activation(
                            out=yt[:, 0:cur], in_=yt[:, 0:cur],
                            func=mybir.ActivationFunctionType.Relu)
                        rv = yt[:, 0:cur].rearrange("p (a h) -> p a h", h=H)
                        # pos/neg reduces -> logits, split at block boundaries
                        a0 = 0
                        while a0 < na:
                            if col % BCOLS == 0:
                                lg_blk = lpool.tile([P, BCOLS], f32, tag="lg")
                            c4 = col % BCOLS
                            seg = int(min(na - a0, BCOLS - c4))
                            lgp = spool.tile([P, ch // P], f32, tag="lgp")
                            lgn = spool.tile([P, ch // P], f32, tag="lgn")
                            nc.vector.tensor_reduce(
                                out=lgp[:, 0:seg], in_=rv[:, a0:a0 + seg, 0:Hp],
                                axis=mybir.AxisListType.X,
                                op=mybir.AluOpType.add)
                            nc.vector.tensor_reduce(
                                out=lgn[:, 0:seg], in_=rv[:, a0:a0 + seg, Hp:H],
                                axis=mybir.AxisListType.X,
                                op=mybir.AluOpType.add)
                            nc.vector.tensor_sub(
                                out=lg_blk[:, c4:c4 + seg],
                                in0=lgp[:, 0:seg], in1=lgn[:, 0:seg])
                            col += seg
                            a0 += seg
                            if col % BCOLS == 0 or col == ncols:
                                drain_block(col)
                        pos += cur
    nc.finalize()
    return nc


def _prepare_v6(z_user, z_movie, edge_index, W1, b1, W2, b2,
                n_cores=N_CORES, bank=BANK):
    import ml_dtypes
    bf16 = ml_dtypes.bfloat16
    z_user = np.asarray(z_user, dtype=np.float32)
    z_movie = np.asarray(z_movie, dtype=np.float32)
    edge_index = np.asarray(edge_index)
    W1 = np.asarray(W1, dtype=np.float32)
    b1 = np.asarray(b1, dtype=np.float32)
    W2 = np.asarray(W2, dtype=np.float32)
    b2 = np.asarray(b2, dtype=np.float32)

    E = edge_index.shape[1]
    rows = edge_index[0].astype(np.int64)
    cols = edge_index[1].astype(np.int64)
    if E and (rows.max() >= NT or cols.max() >= NT):
        raise ValueError("edge index out of v6 table range")
    nbU = nbM = -(-NT // bank)
    nbkt = nbU * nbM
    Epc = -(-E // n_cores)

    per_core = []
    cnts = np.zeros((n_cores, nbkt), dtype=np.int64)
    for c in range(n_cores):
        sl = slice(c * Epc, min((c + 1) * Epc, E))
        r, co = rows[sl], cols[sl]
        bkt = (r // bank) * nbM + (co // bank)
        order = np.argsort(bkt, kind="stable")
        cnts[c] = np.bincount(bkt, minlength=nbkt)
        per_core.append((sl, order, r, co, bkt))

    caps = np.maximum(_roundup(cnts.max(axis=0), TILE), TILE)
    offs = np.concatenate([[0], np.cumsum(caps)])
    L = int(offs[-1])

    # permute hidden units w2>=0 first; fold |w2| into W1 rows and b1.
    w2 = W2.reshape(-1)
    perm = np.argsort(w2 < 0, kind="stable")
    Hp = int((w2 >= 0).sum())
    w2sc = np.abs(w2[perm])
    W1p = W1[perm] * w2sc[:, None]          # [h', 2H]
    b1p = b1[perm] * w2sc                   # [h']

    nuse_u = min(z_user.shape[0], NT)
    nuse_m = min(z_movie.shape[0], NT)
    zuT = np.zeros((P, NT), dtype=bf16)
    zuT[:, :nuse_u] = z_user[:nuse_u].T.astype(bf16)
    zmT = np.zeros((P, NT), dtype=bf16)
    zmT[:, :nuse_m] = z_movie[:nuse_m].T.astype(bf16)
    shared = {"zuT": zuT, "zmT": zmT,
              "w1ut": np.ascontiguousarray(W1p[:, :H].T).astype(bf16),
              "w1mt": np.ascontiguousarray(W1p[:, H:].T).astype(bf16),
              "b1r4": np.ascontiguousarray(
                  np.tile(b1p, (P, GRP)).astype(np.float32)),
              "b2c": np.full((P, 1), float(b2.reshape(-1)[0]), np.float32)}
    rows_u = [min(bank, NT - i * bank) for i in range(nbU)]
    rows_m = [min(bank, NT - i * bank) for i in range(nbM)]

    in_maps, backmaps = [], []
    for c in range(n_cores):
        sl, order, r, co, bkt = per_core[c]
        n = len(r)
        starts = np.concatenate([[0], np.cumsum(cnts[c])])
        sorted_bkt = bkt[order]
        k = np.arange(n) - starts[sorted_bkt]
        spos = offs[sorted_bkt] + k          # slot of edge order[i]
        iu = np.zeros(L, np.int16)
        im = np.zeros(L, np.int16)
        iu[spos] = (r[order] % bank).astype(np.int16)
        im[spos] = (co[order] % bank).astype(np.int16)
        slot = np.empty(n, np.int64)
        slot[order] = spos
        iu_w = np.ascontiguousarray(np.tile(iu.reshape(L // 16, 16).T, (8, 1)))
        im_w = np.ascontiguousarray(np.tile(im.reshape(L // 16, 16).T, (8, 1)))
        in_maps.append({**shared, "idxU": iu_w, "idxM": im_w})
        backmaps.append((sl, slot))
    return in_maps, dict(rows_u=rows_u, rows_m=rows_m,
                         caps=[int(x) for x in caps], L=L, E=E, Hp=Hp,
                         backmaps=backmaps)


# ---------------------------------------------------------------------------
# v7: user-range sharding. A-side (user) via PE one-hot expansion: edges
# sorted by (movie-bank, local user window); host streams bf16 one-hot masks
# (index-derived only); window tiles of the per-core A table feed PE as rhs.
# B-side (movie) via non-transpose dma_gather over all 4 SWDGE queues.
# Tables precomputed on device (A per-core slice w/ b1+|w2| fold; B full).
# Slot mapping identical to v3/v6 (slot s -> partition s%128, col s//128).
# ---------------------------------------------------------------------------

UPC7 = 6272      # users per core (50176/8); window = 128 users, 49/core
UPC7P = 7168     # padded to 14*512 for the paired 1024-row precompute groups


def _v7_schedule(caps):
    """caps: [2][49] window slot capacities (each mult of 16; run totals mult
    of 512). Returns (sched, naux): sched = per 512-slot group the list of
    window ids (global: mb*49 + w); naux = total aux mask tiles."""
    nwin = len(caps[0])
    sched = []
    base = 0
    for mb in range(2):
        run = int(sum(caps[mb]))
        assert run % 512 == 0
        starts = np.concatenate([[0], np.cumsum(caps[mb])])
        for g0 in range(run // 512):
            lo, hi = g0 * 512, (g0 + 1) * 512
            w_lo = int(np.searchsorted(starts, lo, side="right") - 1)
            w_hi = int(np.searchsorted(starts, hi - 1, side="right") - 1)
            sched.append([mb * nwin + w for w in range(w_lo, w_hi + 1)])
        base += run
    naux = sum(len(ws) - 1 for ws in sched)
    return sched, naux


def _build_nc_v7(caps, Hp, sched, naux, ch=4096, repeat=1):
    """caps: [2][nwin] window capacities. sched/naux: from _v7_schedule."""
    f32 = mybir.dt.float32
    bf16 = mybir.dt.bfloat16
    i16 = mybir.dt.int16
    nwin = len(caps[0])
    run_len = [int(sum(caps[mb])) for mb in range(2)]
    L = sum(run_len)
    ncols = L // P
    nblk = -(-ncols // BCOLS)
    NTU = nwin * P               # per-core A rows (6272)
    rows_m = [BANK, NT - BANK]
    assert 0 < Hp < H and L % 512 == 0

    import os
    dbg_no_gather = os.environ.get("EDGE_V7_NO_GATHER") == "1"
    dbg_no_onehot = os.environ.get("EDGE_V7_NO_ONEHOT") == "1"
    dbg_no_mask0 = os.environ.get("EDGE_V7_NO_MASK0") == "1"
    dbg_no_compute = os.environ.get("EDGE_V7_NO_COMPUTE") == "1"
    dbg_no_precomp = os.environ.get("EDGE_V7_NO_PRECOMP") == "1"

    # max aux (straddle) tiles needed by any single chunk, for tile sizing
    AXC = 1
    gg = 0
    for mb in range(2):
        pos = 0
        while pos < run_len[mb]:
            cur = int(min(ch, run_len[mb] - pos))
            AXC = max(AXC, sum(len(sched[gg + i]) - 1
                               for i in range(cur // 512)))
            gg += cur // 512
            pos += cur

    nc = bacc.Bacc(None, target_bir_lowering=False,
                   dynamic_dma_scratch_size=32768, num_swdge_queues=4)

    zuTc = nc.dram_tensor("zuTc", [P, UPC7P], bf16, kind="ExternalInput")
    zmT = nc.dram_tensor("zmT", [P, NT], bf16, kind="ExternalInput")
    w1ut = nc.dram_tensor("w1ut", [H, H], bf16, kind="ExternalInput")
    w1mt = nc.dram_tensor("w1mt", [H, H], bf16, kind="ExternalInput")
    b1r4 = nc.dram_tensor("b1r4", [P, GRP * H], f32, kind="ExternalInput")
    b2c = nc.dram_tensor("b2c", [P, 1], f32, kind="ExternalInput")
    idxM = nc.dram_tensor("idxM", [P, L // 16], i16, kind="ExternalInput")
    mask0 = nc.dram_tensor("mask0", [P, L], bf16, kind="ExternalInput")
    maskx = nc.dram_tensor("maskx", [P, max(naux, 1) * 512], bf16,
                           kind="ExternalInput")
    out_d = nc.dram_tensor("out", [nblk, P, BCOLS], f32, kind="ExternalOutput")

    tabU = nc.dram_tensor("tabU", [UPC7P, H], bf16)
    tabs_m = [nc.dram_tensor(f"tabM{i}", [r, H], bf16)
              for i, r in enumerate(rows_m)]

    with tile.TileContext(nc) as tc:
        with (
            tc.tile_pool(name="const", bufs=1) as cpool,
            tc.tile_pool(name="pre", bufs=2) as prepool,
            tc.tile_pool(name="gat", bufs=8) as gpool,
            tc.tile_pool(name="msk", bufs=4) as mpool,
            tc.tile_pool(name="idx", bufs=8) as ipool,
            tc.tile_pool(name="win", bufs=1) as wpool,
            tc.tile_pool(name="aux", bufs=3) as xpool,
            tc.tile_pool(name="rel", bufs=2) as rpool,
            tc.tile_pool(name="lgs", bufs=4) as spool,
            tc.tile_pool(name="lgb", bufs=2) as lpool,
            tc.tile_pool(name="obuf", bufs=2) as opool,
            tc.tile_pool(name="psue", bufs=4, space="PSUM") as ppool2,
        ):
            nc.gpsimd.load_library(library_config.mlp)
            w1ut_t = cpool.tile([H, H], bf16)
            w1mt_t = cpool.tile([H, H], bf16)
            b1r4_t = cpool.tile([P, GRP * H], f32)
            b2_t = cpool.tile([P, 1], f32)
            nc.sync.dma_start(out=w1ut_t[:], in_=w1ut[:])
            nc.sync.dma_start(out=w1mt_t[:], in_=w1mt[:])
            nc.sync.dma_start(out=b1r4_t[:], in_=b1r4[:])
            nc.sync.dma_start(out=b2_t[:], in_=b2c[:])

            for _rep in range(repeat):
                # ---- precompute: tabM bank0, tabU slice, tabM bank1 ----
                gno = 0

                ZB = 4096            # z columns staged per DMA (4 group-pairs)

                def pre_groups(zT, w1t, tab, zoff, n512, addb1):
                    # paired: 1024 node rows per PSUM tile / drain / tab write
                    nonlocal gno
                    assert n512 % 2 == 0
                    for z0 in range(0, n512 * TILE, ZB):
                        zn = min(ZB, n512 * TILE - z0)
                        zbig = prepool.tile([P, ZB], bf16, tag="zst")
                        nc.sync.dma_start(
                            out=zbig[:, 0:zn],
                            in_=zT[:, zoff + z0:zoff + z0 + zn])
                        for s in range(zn // (2 * TILE)):
                            so = z0 // (2 * TILE) + s
                            pps = ppool2.tile([P, 2 * GRP * H], f32, tag="eps")
                            if addb1:
                                nc.scalar.copy(out=pps[:, 0:GRP * H],
                                               in_=b1r4_t[:])
                                nc.scalar.copy(out=pps[:, GRP * H:],
                                               in_=b1r4_t[:])
                            for t in range(2 * GRP):
                                nc.tensor.matmul(
                                    out=pps[:, t * H:(t + 1) * H],
                                    lhsT=zbig[:, s * 2 * TILE + t * P:
                                              s * 2 * TILE + (t + 1) * P],
                                    rhs=w1t[:], start=not addb1,
                                    stop=True, skip_group_check=True)
                            ast = prepool.tile([P, 2 * GRP * H], bf16,
                                               tag="ast")
                            if gno % 2 == 0:
                                nc.scalar.copy(out=ast[:], in_=pps[:])
                            else:
                                nc.vector.tensor_copy(out=ast[:], in_=pps[:])
                            gno += 1
                            nc.sync.dma_start(
                                out=tab[so * 2 * TILE:(so + 1) * 2 * TILE, :]
                                .rearrange("(t p) h -> p t h", p=P),
                                in_=ast[:].rearrange("p (t h) -> p t h", h=H))

                if not dbg_no_precomp:
                    pre_groups(zmT, w1mt_t, tabs_m[0], 0, BANK // TILE, False)
                    pre_groups(zuTc, w1ut_t, tabU, 0, UPC7P // TILE, True)
                    pre_groups(zmT, w1mt_t, tabs_m[1], BANK,
                               (NT - BANK) // TILE, False)

                # ---- edge phase ----
                col = 0
                lg_blk = None
                aux_no = 0
                g_global = 0
                # resident A-panel: window w rows on partitions, cols [wH,(w+1)H)
                panel = wpool.tile([P, nwin * H], bf16, tag="panel")
                nc.sync.dma_start(
                    out=panel[:].rearrange("p (w h) -> p w h", h=H),
                    in_=tabU[0:nwin * P, :].rearrange("(w p) h -> p w h", p=P))

                def get_window(w):
                    w = w % nwin
                    return panel[:, w * H:(w + 1) * H]

                def drain_block(c0):
                    nco = c0 % BCOLS if c0 % BCOLS else BCOLS
                    blk = (c0 - 1) // BCOLS
                    ot = opool.tile([P, BCOLS], f32, tag="ot")
                    nc.scalar.activation(
                        out=ot[:, 0:nco], in_=lg_blk[:, 0:nco],
                        func=mybir.ActivationFunctionType.Sigmoid,
                        bias=b2_t[:, 0:1], scale=1.0)
                    nc.scalar.mul(out=ot[:, 0:nco], in_=ot[:, 0:nco], mul=5.0)
                    nc.sync.dma_start(out=out_d[blk, :, 0:nco], in_=ot[:, 0:nco])

                chunk_no = 0
                for mb in range(2):
                    base = sum(run_len[:mb])
                    cap = run_len[mb]
                    pos = 0
                    while pos < cap:
                        cur = int(min(ch, cap - pos))
                        s0 = base + pos
                        im_t = ipool.tile([P, ch // 16], i16, tag="im")
                        nc.sync.dma_start(
                            out=im_t[:, 0:cur // 16],
                            in_=idxM[:, s0 // 16:(s0 + cur) // 16])
                        bt = gpool.tile([P, ch], bf16, tag="bt")
                        if not dbg_no_gather:
                            nc.gpsimd.dma_gather(
                                out_ap=bt[:, 0:cur].rearrange(
                                    "p (a n) -> p a n", a=cur // P),
                                in_ap=tabs_m[mb][:],
                                idxs_ap=im_t[:, 0:cur // 16],
                                num_idxs=cur, num_idxs_reg=cur, elem_size=H,
                                transpose=False, queue_num=chunk_no % 4,
                                single_packet=False)
                        chunk_no += 1
                        mk0 = mpool.tile([P, ch], bf16, tag="mk0")
                        if not dbg_no_mask0:
                            nc.sync.dma_start(out=mk0[:, 0:cur],
                                              in_=mask0[:, s0:s0 + cur])
                        if dbg_no_compute:
                            g_global += cur // 512
                            col += cur // P
                            pos += cur
                            continue
                        yt = rpool.tile([P, ch], bf16, tag="yt")
                        assert cur % 1024 == 0
                        # batched aux (straddle) mask load for this chunk
                        ax_n = sum(len(sched[g_global + i]) - 1
                                   for i in range(cur // 512))
                        ax_base = aux_no
                        mkx = None
                        if ax_n and not dbg_no_onehot:
                            mkx = xpool.tile([P, AXC * 512], bf16, tag="mx")
                            nc.sync.dma_start(
                                out=mkx[:, 0:ax_n * 512],
                                in_=maskx[:, aux_no * 512:
                                          (aux_no + ax_n) * 512])
                        for gp in range(cur // 1024):
                            if dbg_no_onehot:
                                g_global += 2
                                nc.vector.tensor_copy(
                                    out=yt[:, gp * 1024:(gp + 1) * 1024],
                                    in_=bt[:, gp * 1024:(gp + 1) * 1024])
                                continue
                            # two 512-slot groups share one 2-bank PSUM tile
                            ps = ppool2.tile([P, 2 * GRP * H], f32, tag="eps")
                            for half in range(2):
                                gi = gp * 2 + half
                                wins = sched[g_global]
                                parts = []
                                for ki, w in enumerate(wins):
                                    if ki == 0:
                                        mk_t, moff = mk0, gi * 512
                                    else:
                                        mk_t = mkx
                                        moff = (aux_no - ax_base) * 512
                                        aux_no += 1
                                    parts.append((mk_t, moff, get_window(w)))
                                # per PSUM segment, open and close the PE
                                # accumulation group before moving on (groups
                                # must not interleave across segments)
                                for t in range(GRP):
                                    seg = (half * GRP + t) * H
                                    for ki, (mk_t, moff, wt) in enumerate(parts):
                                        nc.tensor.matmul(
                                            out=ps[:, seg:seg + H],
                                            lhsT=mk_t[:, moff + t * P:
                                                      moff + (t + 1) * P],
                                            rhs=wt,
                                            start=(ki == 0),
                                            stop=(ki == len(parts) - 1),
                                            skip_group_check=True)
                                g_global += 1
                            nc.vector.tensor_add(
                                out=yt[:, gp * 1024:(gp + 1) * 1024],
                                in0=ps[:],
                                in1=bt[:, gp * 1024:(gp + 1) * 1024])
                        nc.scalar.activation(
                            out=yt[:, 0:cur], in_=yt[:, 0:cur],
                            func=mybir.ActivationFunctionType.Relu)
                        rv = yt[:, 0:cur].rearrange("p (a h) -> p a h", h=H)
                        na = cur // P
                        a0 = 0
                        while a0 < na:
                            if col % BCOLS == 0:
                                lg_blk = lpool.tile([P, BCOLS], f32, tag="lg")
                            c4 = col % BCOLS
                            seg = int(min(na - a0, BCOLS - c4))
                            lgp = spool.tile([P, ch // P], f32, tag="lgp")
                            lgn = spool.tile([P, ch // P], f32, tag="lgn")
                            nc.vector.tensor_reduce(
                                out=lgp[:, 0:seg], in_=rv[:, a0:a0 + seg, 0:Hp],
                                axis=mybir.AxisListType.X,
                                op=mybir.AluOpType.add)
                            nc.vector.tensor_reduce(
                                out=lgn[:, 0:seg], in_=rv[:, a0:a0 + seg, Hp:H],
                                axis=mybir.AxisListType.X,
                                op=mybir.AluOpType.add)
                            nc.vector.tensor_sub(
                                out=lg_blk[:, c4:c4 + seg],
                                in0=lgp[:, 0:seg], in1=lgn[:, 0:seg])
                            col += seg
                            a0 += seg
                            if col % BCOLS == 0 or col == ncols:
                                drain_block(col)
                        pos += cur
    nc.finalize()
    return nc


def _prepare_v7(z_user, z_movie, edge_index, W1, b1, W2, b2,
                n_cores=N_CORES):
    import ml_dtypes
    bf16 = ml_dtypes.bfloat16
    z_user = np.asarray(z_user, dtype=np.float32)
    z_movie = np.asarray(z_movie, dtype=np.float32)
    edge_index = np.asarray(edge_index)
    W1 = np.asarray(W1, dtype=np.float32)
    b1 = np.asarray(b1, dtype=np.float32)
    W2 = np.asarray(W2, dtype=np.float32)
    b2 = np.asarray(b2, dtype=np.float32)

    E = edge_index.shape[1]
    rows = edge_index[0].astype(np.int64)
    cols = edge_index[1].astype(np.int64)
    if E and (rows.max() >= NT or cols.max() >= NT):
        raise ValueError("edge index out of v7 table range")
    nwin = UPC7 // P

    # per-core split (by user range), then by movie bank, then by window
    core_of = rows // UPC7
    per_core = []
    wcnt = np.zeros((n_cores, 2, nwin), dtype=np.int64)
    for c in range(n_cores):
        eids = np.nonzero(core_of == c)[0]
        r, co = rows[eids], cols[eids]
        u = r - c * UPC7
        mb = co // BANK
        w = u // P
        order = np.lexsort((w, mb))
        eids, u, co, mb, w = eids[order], u[order], co[order], mb[order], w[order]
        for b in range(2):
            wcnt[c, b] = np.bincount(w[mb == b], minlength=nwin)
        per_core.append((eids, u, co, mb, w))

    # shared window capacities: max over cores, round to 16; run mult of 512
    caps = np.maximum(_roundup(wcnt.max(axis=0), 16), 16)
    for b in range(2):
        tot = int(caps[b].sum())
        caps[b][-1] += _roundup(tot, 1024) - tot
    run_len = [int(caps[b].sum()) for b in range(2)]
    L = sum(run_len)
    starts = np.zeros((2, nwin), dtype=np.int64)
    for b in range(2):
        starts[b] = sum(run_len[:b]) + np.concatenate(
            [[0], np.cumsum(caps[b])[:-1]])

    sched, naux = _v7_schedule([list(map(int, caps[0])),
                                list(map(int, caps[1]))])
    # aux ordinal lookup: (group, window) -> ordinal for non-first windows
    aux_of = {}
    k = 0
    for g, ws in enumerate(sched):
        for wi in ws[1:]:
            aux_of[(g, wi)] = k
            k += 1
    assert k == naux

    w2v = W2.reshape(-1)
    perm = np.argsort(w2v < 0, kind="stable")
    Hp = int((w2v >= 0).sum())
    w2sc = np.abs(w2v[perm])
    W1p = W1[perm] * w2sc[:, None]
    b1p = b1[perm] * w2sc

    nuse_m = min(z_movie.shape[0], NT)
    zmT = np.zeros((P, NT), dtype=bf16)
    zmT[:, :nuse_m] = z_movie[:nuse_m].T.astype(bf16)
    shared = {"zmT": zmT,
              "w1ut": np.ascontiguousarray(W1p[:, :H].T).astype(bf16),
              "w1mt": np.ascontiguousarray(W1p[:, H:].T).astype(bf16),
              "b1r4": np.ascontiguousarray(
                  np.tile(b1p, (P, GRP)).astype(np.float32)),
              "b2c": np.full((P, 1), float(b2.reshape(-1)[0]), np.float32)}

    sched_w0 = np.array([ws[0] for ws in sched], dtype=np.int64)
    in_maps, backmaps = [], []
    for c in range(n_cores):
        eids, u, co, mb, w = per_core[c]
        # slot: within-window rank
        wk = mb * nwin + w
        ordr = np.argsort(wk, kind="stable")   # already sorted; rank within
        kk = np.arange(len(u)) - np.concatenate(
            [[0], np.cumsum(np.bincount(wk, minlength=2 * nwin))])[wk]
        slot = starts[mb, w] + kk
        g = slot // 512
        wg = mb * nwin + w                     # global window id of each edge
        is_first = wg == sched_w0[g]
        urow = (u % P).astype(np.int64)
        m0 = np.zeros((P, L), dtype=bf16)
        m0[urow[is_first], slot[is_first]] = 1
        mx = np.zeros((P, max(naux, 1) * 512), dtype=bf16)
        nf = np.nonzero(~is_first)[0]
        if len(nf):
            aux_idx = np.array([aux_of[(int(g[i]), int(wg[i]))] for i in nf],
                               dtype=np.int64)
            mx[urow[nf], aux_idx * 512 + (slot[nf] % 512)] = 1
        im = np.zeros(L, np.int16)
        im[slot] = (co % BANK).astype(np.int16)
        im_w = np.ascontiguousarray(np.tile(im.reshape(L // 16, 16).T, (8, 1)))
        zuTc = np.zeros((P, UPC7P), dtype=bf16)
        lo = c * UPC7
        hi = min((c + 1) * UPC7, z_user.shape[0])
        if hi > lo:
            zuTc[:, :hi - lo] = z_user[lo:hi].T.astype(bf16)
        in_maps.append({**shared, "zuTc": zuTc, "idxM": im_w,
                        "mask0": m0, "maskx": mx})
        backmaps.append((eids, slot))
    caps_py = [list(map(int, caps[0])), list(map(int, caps[1]))]
    return in_maps, dict(caps=caps_py, sched=sched, naux=naux, L=L, E=E,
                         Hp=Hp, backmaps=backmaps)


def _unpack_v7(res, meta):
    out = np.empty(meta["E"], dtype=np.float32)
    for c, (eids, slot) in enumerate(meta["backmaps"]):
        flat = np.asarray(res.results[c]["out"], dtype=np.float32).reshape(-1)
        tc_ = slot // P
        p = slot % P
        fidx = (tc_ // BCOLS) * (P * BCOLS) + p * BCOLS + (tc_ % BCOLS)
        out[eids] = flat[fidx]
    return out


# ---------------------------------------------------------------------------
# v1 (fallback): precomputed tables + per-column indirect DMA gathers
# ---------------------------------------------------------------------------

def _build_nc(C, NA, NB, Hp, repeat=1, repeat_pre=None, repeat_gather=None):
    """C: edge cols per core (edges = 128*C). NA/NB: padded table rows. Hp: # pos-w2 units.
    repeat>1 re-runs the compute phases (identical results) for slope-based timing."""
    f32 = mybir.dt.float32
    i32 = mybir.dt.int32
    nc = bacc.Bacc(None, target_bir_lowering=False)

    zTu = nc.dram_tensor("zTu", [P, NA], f32, kind="ExternalInput")
    zTm = nc.dram_tensor("zTm", [P, NB], f32, kind="ExternalInput")
    w1ut = nc.dram_tensor("w1ut", [P, H], f32, kind="ExternalInput")
    w1mt = nc.dram_tensor("w1mt", [P, H], f32, kind="ExternalInput")
    b1rep = nc.dram_tensor("b1rep", [P, H], f32, kind="ExternalInput")
    b2rep = nc.dram_tensor("b2rep", [P, 1], f32, kind="ExternalInput")
    idxA = nc.dram_tensor("idxA", [P, C], i32, kind="ExternalInput")
    idxB = nc.dram_tensor("idxB", [P, C], i32, kind="ExternalInput")
    out_d = nc.dram_tensor("out", [P, C], f32, kind="ExternalOutput")

    tabA = nc.dram_tensor("tabA", [NA, H], f32)
    tabB = nc.dram_tensor("tabB", [NB, H], f32)
    # tile-linearized write view: table row (p*(N/128) + m) <-> partition p, col block m
    tabA_v = tabA[:].rearrange("(p m) d -> p (m d)", p=P)
    tabB_v = tabB[:].rearrange("(p m) d -> p (m d)", p=P)

    with tile.TileContext(nc) as tc:
        with (
            tc.tile_pool(name="const", bufs=1) as cpool,
            tc.tile_pool(name="work", bufs=3) as wpool,
            tc.tile_pool(name="psum", bufs=4, space="PSUM") as ppool,
        ):
            w1ut_t = cpool.tile([P, H], f32)
            w1mt_t = cpool.tile([P, H], f32)
            b1rep_t = cpool.tile([P, H], f32)
            b2rep_t = cpool.tile([P, 1], f32)
            idxA_t = cpool.tile([P, C], i32)
            idxB_t = cpool.tile([P, C], i32)
            logits = cpool.tile([P, C], f32)
            nc.sync.dma_start(out=w1ut_t[:], in_=w1ut[:])
            nc.sync.dma_start(out=w1mt_t[:], in_=w1mt[:])
            nc.sync.dma_start(out=b1rep_t[:], in_=b1rep[:])
            nc.sync.dma_start(out=b2rep_t[:], in_=b2rep[:])
            nc.sync.dma_start(out=idxA_t[:], in_=idxA[:])
            nc.sync.dma_start(out=idxB_t[:], in_=idxB[:])

            # ---- precompute tables ----
            for (zT, w1t, tab_v, npad, addb1) in (
                (zTu, w1ut_t, tabA_v, NA, True),
                (zTm, w1mt_t, tabB_v, NB, False),
            ) * (repeat_pre if repeat_pre is not None else repeat):
                with tc.For_i(0, npad, ZBODY) as iv:
                    zstage = wpool.tile([P, ZBODY], f32, tag="zstage")
                    nc.sync.dma_start(out=zstage[:], in_=zT[:, bass.ds(iv, ZBODY)])
                    astage = wpool.tile([P, ZBODY], f32, tag="astage")
                    for k in range(ZBODY // P):
                        ps = ppool.tile([P, H], f32, tag="ps")
                        nc.tensor.matmul(
                            out=ps[:],
                            lhsT=zstage[:, k * P:(k + 1) * P],
                            rhs=w1t[:],
                            start=True, stop=True,
                        )
                        sl = astage[:, k * H:(k + 1) * H]
                        if addb1:
                            nc.vector.tensor_add(out=sl, in0=ps[:], in1=b1rep_t[:])
                        else:
                            nc.scalar.copy(out=sl, in_=ps[:])
                    nc.sync.dma_start(out=tab_v[:, bass.ds(iv, ZBODY)], in_=astage[:])

            # ---- edge gather + MLP ----
            def gather_body(iv):
                rstage = wpool.tile([P, G], i32, tag="rstage")
                cstage = wpool.tile([P, G], i32, tag="cstage")
                nc.vector.tensor_copy(out=rstage[:], in_=idxA_t[:, bass.ds(iv, G)])
                nc.vector.tensor_copy(out=cstage[:], in_=idxB_t[:, bass.ds(iv, G)])
                ct = wpool.tile([P, G * H], f32, tag="ct")
                for j in range(G):
                    sl = ct[:, j * H:(j + 1) * H]
                    nc.gpsimd.indirect_dma_start(
                        out=sl, out_offset=None, in_=tabA[:],
                        in_offset=bass.IndirectOffsetOnAxis(ap=rstage[:, j:j + 1], axis=0),
                    )
                    nc.gpsimd.indirect_dma_start(
                        out=sl, out_offset=None, in_=tabB[:],
                        in_offset=bass.IndirectOffsetOnAxis(ap=cstage[:, j:j + 1], axis=0),
                        compute_op=mybir.AluOpType.add,
                    )
                cc = ct[:].rearrange("p (g h) -> p g h", h=H)
                if Hp > 0:
                    nc.vector.tensor_scalar_max(out=cc[:, :, 0:Hp], in0=cc[:, :, 0:Hp], scalar1=0.0)
                if Hp < H:
                    nc.vector.tensor_scalar_min(out=cc[:, :, Hp:H], in0=cc[:, :, Hp:H], scalar1=0.0)
                lsl = logits[:, bass.ds(iv, G)]
                if Hp == H or Hp == 0:
                    nc.vector.tensor_reduce(out=lsl, in_=cc[:, :, :], axis=mybir.AxisListType.X,
                                            op=mybir.AluOpType.add)
                else:
                    pos = wpool.tile([P, G], f32, tag="pos")
                    nc.vector.tensor_reduce(out=pos[:], in_=cc[:, :, 0:Hp],
                                            axis=mybir.AxisListType.X, op=mybir.AluOpType.add)
                    neg = wpool.tile([P, G], f32, tag="neg")
                    nc.vector.tensor_reduce(out=neg[:], in_=cc[:, :, Hp:H],
                                            axis=mybir.AxisListType.X, op=mybir.AluOpType.add)
                    nc.vector.tensor_add(out=lsl, in0=pos[:], in1=neg[:])

            for _rep in range(repeat_gather if repeat_gather is not None else repeat):
                with tc.For_i(0, C, G) as iv:
                    gather_body(iv)

            # ---- sigmoid tail ----
            sig = cpool.tile([P, C], f32)
            nc.scalar.activation(out=sig[:], in_=logits[:],
                                 func=mybir.ActivationFunctionType.Sigmoid,
                                 bias=b2rep_t[:, 0:1], scale=1.0)
            nc.scalar.mul(out=sig[:], in_=sig[:], mul=5.0)
            nc.sync.dma_start(out=out_d[:], in_=sig[:])
    nc.finalize()
    return nc


def _pad_cols(n, mult):
    return ((n + mult - 1) // mult) * mult


def _prepare(z_user, z_movie, edge_index, W1, b1, W2, b2, n_cores=N_CORES):
    z_user = np.asarray(z_user, dtype=np.float32)
    z_movie = np.asarray(z_movie, dtype=np.float32)
    edge_index = np.asarray(edge_index)
    W1 = np.asarray(W1, dtype=np.float32)
    b1 = np.asarray(b1, dtype=np.float32)
    W2 = np.asarray(W2, dtype=np.float32)
    b2 = np.asarray(b2, dtype=np.float32)

    E = edge_index.shape[1]
    rows = edge_index[0].astype(np.int64)
    cols = edge_index[1].astype(np.int64)

    NAr = int(rows.max()) + 1 if E else 1          # referenced user rows
    NBr = z_movie.shape[0]
    NA = _pad_cols(max(NAr, ZBODY), ZBODY)
    NB = _pad_cols(max(NBr, ZBODY), ZBODY)

    # hidden permutation: positive-w2 units first; fold signed w2 and b1 into tables
    w2 = W2.reshape(-1)
    perm = np.argsort(w2 < 0, kind="stable")       # stable: positives (False) first
    Hp = int((w2 >= 0).sum())
    W1p = W1[perm]                                  # [H, 2H]
    b1p = b1[perm]
    scale = w2[perm]  # signed: w2*relu(x) = max0(w2*x) for w2>0, min0(w2*x) for w2<0
    w1ut = np.ascontiguousarray((W1p[:, :H] * scale[:, None]).T)   # [in, h]
    w1mt = np.ascontiguousarray((W1p[:, H:] * scale[:, None]).T)
    b1rep = np.tile(b1p * scale, (P, 1)).astype(np.float32)
    b2rep = np.full((P, 1), float(b2.reshape(-1)[0]), dtype=np.float32)

    # transposed, padded node features
    zTu = np.zeros((P, NA), dtype=np.float32)
    zTu[:, :NAr] = z_user[:NAr].T
    zTm = np.zeros((P, NB), dtype=np.float32)
    zTm[:, :NBr] = z_movie.T

    # tile-linearized table row index: u -> (u%128)*(N/128) + u//128
    mA, mB = NA // P, NB // P
    idxA_full = ((rows % P) * mA + rows // P).astype(np.int32)
    idxB_full = ((cols % P) * mB + cols // P).astype(np.int32)

    # shard edges: per core 128*C edges, C divisible by G
    C = _pad_cols(-(-E // (n_cores * P)), G)
    Epc = P * C
    Etot = n_cores * Epc
    idxA_pad = np.zeros(Etot, dtype=np.int32)
    idxA_pad[:E] = idxA_full
    idxB_pad = np.zeros(Etot, dtype=np.int32)
    idxB_pad[:E] = idxB_full

    in_maps = []
    for c in range(n_cores):
        sl = slice(c * Epc, (c + 1) * Epc)
        in_maps.append({
            "zTu": zTu, "zTm": zTm, "w1ut": w1ut, "w1mt": w1mt,
            "b1rep": b1rep, "b2rep": b2rep,
            "idxA": idxA_pad[sl].reshape(P, C),
            "idxB": idxB_pad[sl].reshape(P, C),
        })
    return in_maps, dict(C=C, NA=NA, NB=NB, Hp=Hp, E=E)


def kernel(z_user, z_movie, edge_index, W1, b1, W2, b2):
    import os
    if os.environ.get("EDGE_KERNEL_V4") == "1":  # correct but ~5x slower on HW than v3
        try:
            in_maps, meta = _prepare_v4(z_user, z_movie, edge_index, W1, b1, W2, b2)
            nc = _build_nc_v4(meta["nwin"], meta["lsp_caps"], meta["Hp"])
            res = run_bass_kernel_spmd(nc, in_maps, core_ids=list(range(N_CORES)))
            out = _unpack_v4(res, meta)
            _LAST_STATS.update(exec_time_ns=res.exec_time_ns, nc=nc,
                               in_maps=in_maps, meta=meta, version="v4")
            return out
        except Exception as e:
            import traceback
            traceback.print_exc()
            print(f"[kernel] v4 path failed ({type(e).__name__}: {e}); falling back to v3",
                  file=sys.stderr)
    if os.environ.get("EDGE_KERNEL_V7") == "1":
        try:
            in_maps, meta = _prepare_v7(z_user, z_movie, edge_index, W1, b1, W2, b2)
            nc = _build_nc_v7(meta["caps"], meta["Hp"], meta["sched"],
                              meta["naux"])
            res = run_bass_kernel_spmd(nc, in_maps, core_ids=list(range(N_CORES)))
            out = _unpack_v7(res, meta)
            _LAST_STATS.update(exec_time_ns=res.exec_time_ns, nc=nc,
                               in_maps=in_maps, meta=meta, version="v7")
            return out
        except Exception as e:
            import traceback
            traceback.print_exc()
            print(f"[kernel] v7 path failed ({type(e).__name__}: {e}); falling back",
                  file=sys.stderr)
    if os.environ.get("EDGE_KERNEL_V6", "1") == "1":
        try:
            in_maps, meta = _prepare_v6(z_user, z_movie, edge_index, W1, b1, W2, b2)
            nc = _build_nc_v6(meta["rows_u"], meta["rows_m"], meta["caps"],
                              meta["Hp"])
            res = run_bass_kernel_spmd(nc, in_maps, core_ids=list(range(N_CORES)))
            out = _unpack_v3(res, meta)
            _LAST_STATS.update(exec_time_ns=res.exec_time_ns, nc=nc,
                               in_maps=in_maps, meta=meta, version="v6")
            return out
        except Exception as e:
            import traceback
            traceback.print_exc()
            print(f"[kernel] v6 path failed ({type(e).__name__}: {e}); falling back to v3",
                  file=sys.stderr)
    if os.environ.get("EDGE_KERNEL_V1") != "1":
        try:
            in_maps, meta = _prepare_v3(z_user, z_movie, edge_index, W1, b1, W2, b2)
            nc = _build_nc_v3(meta["rows_u"], meta["rows_m"], meta["caps"], meta["Hp"])
            res = run_bass_kernel_spmd(nc, in_maps, core_ids=list(range(N_CORES)))
            out = _unpack_v3(res, meta)
            _LAST_STATS.update(exec_time_ns=res.exec_time_ns, nc=nc,
                               in_maps=in_maps, meta=meta, version="v3")
            return out
        except Exception as e:
            import traceback
            traceback.print_exc()
            print(f"[kernel] v3 path failed ({type(e).__name__}: {e}); falling back to v1",
                  file=sys.stderr)
    in_maps, meta = _prepare(z_user, z_movie, edge_index, W1, b1, W2, b2)
    nc = _build_nc(meta["C"], meta["NA"], meta["NB"], meta["Hp"])
    res = run_bass_kernel_spmd(nc, in_maps, core_ids=list(range(N_CORES)))
    out = np.concatenate([res.results[c]["out"].reshape(-1) for c in range(N_CORES)])
    _LAST_STATS.update(exec_time_ns=res.exec_time_ns, nc=nc,
                       in_maps=in_maps, meta=meta, version="v1")
    return out[:meta["E"]].astype(np.float32)


# ---------------------------------------------------------------------------
# v4: user-range sharding + movie-sorted windows; tabB streamed and expanded
# on PE via on-chip one-hot (colrep broadcast + DVE is_equal vs iota), tabA
# gathered per edge (non-transpose). Spill edges (window overflow) gather both
# tables. Tables precomputed on device in bf16 with w2/b1 folded.
# ---------------------------------------------------------------------------

WCAP = 640       # edge slots per 128-movie window (uniform across cores)
UPC = 12500      # users per core (100000 / 8)
NAC = 12800      # padded per-core tabA rows
NBP = 50176      # padded tabB rows (392 windows)


def _build_nc_v4(nwin, lsp_caps, Hp, ch=CH, repeat=1):
    """nwin: movie windows. lsp_caps: spill caps per movie-bank bucket
    (multiples of TILE). Hp: pos-w2 unit count."""
    f32 = mybir.dt.float32
    bf16 = mybir.dt.bfloat16
    i16 = mybir.dt.int16
    Lw = nwin * WCAP
    assert Lw % TILE == 0
    Lsp = int(sum(lsp_caps))
    L = Lw + Lsp
    ncols = L // P
    nblk = -(-ncols // BCOLS)
    ngrp = Lw // TILE

    nc = bacc.Bacc(None, target_bir_lowering=False,
                   dynamic_dma_scratch_size=32768)

    zTuc = nc.dram_tensor("zTuc", [P, NAC], bf16, kind="ExternalInput")
    zTmf = nc.dram_tensor("zTmf", [P, NBP], bf16, kind="ExternalInput")
    w1utF = nc.dram_tensor("w1utF", [H, H], bf16, kind="ExternalInput")
    w1mtF = nc.dram_tensor("w1mtF", [H, H], bf16, kind="ExternalInput")
    b1pre = nc.dram_tensor("b1pre", [P, GRP * H], f32, kind="ExternalInput")
    b2c = nc.dram_tensor("b2c", [P, 1], f32, kind="ExternalInput")
    iotas = nc.dram_tensor("iotas", [P, 2 * TILE], f32, kind="ExternalInput")
    ones1 = nc.dram_tensor("ones1", [1, P], bf16, kind="ExternalInput")
    colloc = nc.dram_tensor("colloc", [1, Lw], bf16, kind="ExternalInput")
    idxU = nc.dram_tensor("idxU", [P, L // 16], i16, kind="ExternalInput")
    idxMsp = nc.dram_tensor("idxMsp", [P, max(Lsp, 16) // 16], i16,
                            kind="ExternalInput")
    out_d = nc.dram_tensor("out", [nblk, P, BCOLS], f32, kind="ExternalOutput")

    tabA = nc.dram_tensor("tabA", [NAC, H], bf16)
    tabB = nc.dram_tensor("tabB", [NBP, H], bf16)
    # tabA is gather-only: store tile-linearized (row p*(NAC//P)+m <-> strip
    # node s*512+t*128+p at m = s*4+t); host linearizes gather indices.
    tabA_v = tabA[:].rearrange("(p m) h -> p (m h)", p=P)

    with tile.TileContext(nc) as tc:
        with (
            tc.tile_pool(name="const", bufs=1) as cpool,
            tc.tile_pool(name="pre", bufs=3) as prepool,
            tc.tile_pool(name="gat", bufs=2) as gpool,
            tc.tile_pool(name="idx", bufs=3) as ipool,
            tc.tile_pool(name="win", bufs=4) as wpool,
            tc.tile_pool(name="rel", bufs=4) as rpool,
            tc.tile_pool(name="lgb", bufs=2) as lpool,
            tc.tile_pool(name="obuf", bufs=2) as opool,
            tc.tile_pool(name="psum", bufs=3, space="PSUM") as ppool,
            tc.tile_pool(name="crps", bufs=2, space="PSUM") as crpool,
        ):
            nc.gpsimd.load_library(library_config.mlp)
            w1ut_t = cpool.tile([H, H], bf16)
            w1mt_t = cpool.tile([H, H], bf16)
            b1p_t = cpool.tile([P, GRP * H], f32)
            b2_t = cpool.tile([P, 1], f32)
            iota_t = cpool.tile([P, 2 * TILE], f32)
            ones_t = cpool.tile([1, P], bf16)
            nc.sync.dma_start(out=w1ut_t[:], in_=w1utF[:])
            nc.sync.dma_start(out=w1mt_t[:], in_=w1mtF[:])
            nc.sync.dma_start(out=b1p_t[:], in_=b1pre[:])
            nc.sync.dma_start(out=b2_t[:], in_=b2c[:])
            nc.sync.dma_start(out=iota_t[:], in_=iotas[:])
            nc.sync.dma_start(out=ones_t[:], in_=ones1[:])

            # ---- precompute tabA (b1 folded) and tabB ----
            for (zT, w1t, natural, npad, addb1) in (
                (zTuc, w1ut_t, False, NAC, True),
                (zTmf, w1mt_t, True, NBP, False),
            ):
                for s in range(npad // TILE):
                    zst = prepool.tile([P, TILE], bf16, tag="zst")
                    nc.sync.dma_start(out=zst[:],
                                      in_=zT[:, s * TILE:(s + 1) * TILE])
                    pps = ppool.tile([P, GRP * H], f32, tag="ps")
                    if addb1:
                        nc.scalar.copy(out=pps[:], in_=b1p_t[:])
                    for t in range(GRP):
                        nc.tensor.matmul(out=pps[:, t * H:(t + 1) * H],
                                         lhsT=zst[:, t * P:(t + 1) * P],
                                         rhs=w1t[:], start=not addb1,
                                         stop=True, skip_group_check=True)
                    ast = prepool.tile([P, GRP * H], bf16, tag="ast")
                    nc.scalar.copy(out=ast[:], in_=pps[:])
                    if natural:
                        nc.sync.dma_start(
                            out=tabB[s * TILE:(s + 1) * TILE, :].rearrange(
                                "(t p) h -> p t h", p=P),
                            in_=ast[:].rearrange("p (t h) -> p t h", h=H))
                    else:
                        nc.sync.dma_start(
                            out=tabA_v[:, s * GRP * H:(s + 1) * GRP * H],
                            in_=ast[:])

            for _rep in range(repeat):
                col = 0
                lg_blk = None

                def drain_block(c0):
                    nco = c0 % BCOLS if c0 % BCOLS else BCOLS
                    blk = (c0 - 1) // BCOLS
                    ot = opool.tile([P, BCOLS], f32, tag="ot")
                    nc.scalar.activation(
                        out=ot[:, 0:nco], in_=lg_blk[:, 0:nco],
                        func=mybir.ActivationFunctionType.Sigmoid,
                        bias=b2_t[:, 0:1], scale=1.0)
                    nc.scalar.mul(out=ot[:, 0:nco], in_=ot[:, 0:nco], mul=5.0)
                    nc.sync.dma_start(out=out_d[blk, :, 0:nco], in_=ot[:, 0:nco])

                def reduce_emit(yv, c4):
                    # yv: [P, GRP, H] bf16 view; write logits to lg_blk cols
                    if Hp > 0:
                        nc.vector.tensor_scalar_max(out=yv[:, :, 0:Hp],
                                                    in0=yv[:, :, 0:Hp],
                                                    scalar1=0.0)
                    if Hp < H:
                        nc.vector.tensor_scalar_min(out=yv[:, :, Hp:H],
                                                    in0=yv[:, :, Hp:H],
                                                    scalar1=0.0)
                    lgp = rpool.tile([P, GRP], f32, tag="lgp")
                    lgn = rpool.tile([P, GRP], f32, tag="lgn")
                    if Hp > 0:
                        nc.vector.tensor_reduce(out=lgp[:], in_=yv[:, :, 0:Hp],
                                                axis=mybir.AxisListType.X,
                                                op=mybir.AluOpType.add)
                    if Hp < H:
                        nc.vector.tensor_reduce(out=lgn[:], in_=yv[:, :, Hp:H],
                                                axis=mybir.AxisListType.X,
                                                op=mybir.AluOpType.add)
                    if Hp == H:
                        nc.vector.tensor_copy(out=lg_blk[:, c4:c4 + GRP], in_=lgp[:])
                    elif Hp == 0:
                        nc.vector.tensor_copy(out=lg_blk[:, c4:c4 + GRP], in_=lgn[:])
                    else:
                        nc.vector.tensor_add(out=lg_blk[:, c4:c4 + GRP],
                                             in0=lgp[:], in1=lgn[:])

                # ---- window region ----
                for gbase in range(0, ngrp, ch // TILE):
                    gend = min(gbase + ch // TILE, ngrp)
                    nsl = (gend - gbase) * TILE
                    s0 = gbase * TILE
                    iu_t = ipool.tile([P, ch // 16], i16, tag="iu")
                    nc.sync.dma_start(out=iu_t[:, 0:nsl // 16],
                                      in_=idxU[:, s0 // 16:(s0 + nsl) // 16])
                    at = gpool.tile([P, nsl], bf16, tag="at")
                    nc.gpsimd.dma_gather(
                        out_ap=at[:].rearrange("p (a n) -> p a n", a=nsl // P),
                        in_ap=tabA[:], idxs_ap=iu_t[:, 0:nsl // 16],
                        num_idxs=nsl, num_idxs_reg=nsl, elem_size=H,
                        transpose=False, queue_num=0, single_packet=False)
                    cl_t = ipool.tile([1, ch], bf16, tag="cl")
                    nc.sync.dma_start(out=cl_t[0:1, 0:nsl],
                                      in_=colloc[0:1, s0:s0 + nsl])
                    for g in range(gbase, gend):
                        w0 = (g * TILE) // WCAP
                        straddle = (g * TILE + TILE - 1) // WCAP > w0
                        wins = [w0, w0 + 1] if straddle and w0 + 1 < nwin else [w0]
                        go = (g - gbase) * TILE
                        crp = crpool.tile([P, TILE], f32, tag="cr")
                        nc.tensor.matmul(out=crp[:], lhsT=ones_t[:],
                                         rhs=cl_t[0:1, go:go + TILE],
                                         start=True, stop=True)
                        bps = ppool.tile([P, GRP * H], f32, tag="ps")
                        sks, tbws = [], []
                        for ki, w in enumerate(wins):
                            sk = wpool.tile([P, TILE], bf16, tag="sk")
                            nc.vector.tensor_tensor(
                                out=sk[:], in0=crp[:],
                                in1=iota_t[:, ki * TILE:(ki + 1) * TILE],
                                op=mybir.AluOpType.is_equal)
                            tbw = wpool.tile([P, H], bf16, tag="tbw")
                            nc.sync.dma_start(out=tbw[:],
                                              in_=tabB[w * P:(w + 1) * P, :])
                            sks.append(sk)
                            tbws.append(tbw)
                        for t in range(GRP):
                            for ki in range(len(wins)):
                                nc.tensor.matmul(
                                    out=bps[:, t * H:(t + 1) * H],
                                    lhsT=sks[ki][:, t * P:(t + 1) * P],
                                    rhs=tbws[ki][:],
                                    start=(ki == 0), stop=(ki == len(wins) - 1),
                                    skip_group_check=True)
                        y = rpool.tile([P, GRP * H], bf16, tag="y")
                        ab = (g - gbase) * GRP * H
                        nc.vector.tensor_add(out=y[:], in0=bps[:],
                                             in1=at[:, ab:ab + GRP * H])
                        if col % BCOLS == 0:
                            lg_blk = lpool.tile([P, BCOLS], f32, tag="lg")
                        reduce_emit(y[:].rearrange("p (g h) -> p g h", h=H),
                                    col % BCOLS)
                        col += GRP
                        if col % BCOLS == 0 or col == ncols:
                            drain_block(col)

                # ---- spill region: gather both tables ----
                for b, cap in enumerate(lsp_caps):
                    sbase = Lw + int(sum(lsp_caps[:b]))
                    pos = 0
                    while pos < cap:
                        cur = int(min(ch, cap - pos))
                        s0 = sbase + pos
                        iu_t = ipool.tile([P, ch // 16], i16, tag="iu")
                        nc.sync.dma_start(out=iu_t[:, 0:cur // 16],
                                          in_=idxU[:, s0 // 16:(s0 + cur) // 16])
                        im_t = ipool.tile([P, ch // 16], i16, tag="im")
                        nc.sync.dma_start(
                            out=im_t[:, 0:cur // 16],
                            in_=idxMsp[:, (s0 - Lw) // 16:(s0 - Lw + cur) // 16])
                        at = gpool.tile([P, cur], bf16, tag="at")
                        nc.gpsimd.dma_gather(
                            out_ap=at[:].rearrange("p (a n) -> p a n", a=cur // P),
                            in_ap=tabA[:], idxs_ap=iu_t[:, 0:cur // 16],
                            num_idxs=cur, num_idxs_reg=cur, elem_size=H,
                            transpose=False, queue_num=0, single_packet=False)
                        bt = gpool.tile([P, cur], bf16, tag="bt")
                        nc.gpsimd.dma_gather(
                            out_ap=bt[:].rearrange("p (a n) -> p a n", a=cur // P),
                            in_ap=tabB[min(b * BANK, NBP - P):min((b + 1) * BANK, NBP), :],
                            idxs_ap=im_t[:, 0:cur // 16],
                            num_idxs=cur, num_idxs_reg=cur, elem_size=H,
                            transpose=False, queue_num=0, single_packet=False)
                        for g in range(cur // TILE):
                            go = g * TILE
                            y = rpool.tile([P, GRP * H], bf16, tag="y")
                            ab = g * GRP * H
                            nc.vector.tensor_add(out=y[:],
                                                 in0=at[:, ab:ab + GRP * H],
                                                 in1=bt[:, ab:ab + GRP * H])
                            if col % BCOLS == 0:
                                lg_blk = lpool.tile([P, BCOLS], f32, tag="lg")
                            reduce_emit(y[:].rearrange("p (g h) -> p g h", h=H),
                                        col % BCOLS)
                            col += GRP
                            if col % BCOLS == 0 or col == ncols:
                                drain_block(col)
                        pos += cur
    nc.finalize()
    return nc


def _prepare_v4(z_user, z_movie, edge_index, W1, b1, W2, b2,
                n_cores=N_CORES, upc=UPC, wcap=WCAP):
    import ml_dtypes
    bf16 = ml_dtypes.bfloat16
    z_user = np.asarray(z_user, dtype=np.float32)
    z_movie = np.asarray(z_movie, dtype=np.float32)
    edge_index = np.asarray(edge_index)
    W1 = np.asarray(W1, dtype=np.float32)
    b1 = np.asarray(b1, dtype=np.float32)
    W2 = np.asarray(W2, dtype=np.float32)
    b2 = np.asarray(b2, dtype=np.float32)
    E = edge_index.shape[1]
    rows = edge_index[0].astype(np.int64)
    cols = edge_index[1].astype(np.int64)
    NM = z_movie.shape[0]
    nwin = NBP // P
    assert NM <= NBP and z_user.shape[0] <= n_cores * upc

    w2 = W2.reshape(-1)
    perm = np.argsort(w2 < 0, kind="stable")
    Hp = int((w2 >= 0).sum())
    w2sc = w2[perm]                  # signed: max0 pos-range, min0 neg-range
    W1p = W1[perm] * w2sc[:, None]
    b1p = b1[perm] * w2sc

    zmT = np.zeros((P, NBP), dtype=bf16)
    zmT[:, :NM] = z_movie.T.astype(bf16)
    shared = {"zTmf": zmT,
              "w1utF": np.ascontiguousarray(W1p[:, :H].T).astype(bf16),
              "w1mtF": np.ascontiguousarray(W1p[:, H:].T).astype(bf16),
              "b1pre": np.ascontiguousarray(np.tile(b1p, (P, GRP)).astype(np.float32)),
              "b2c": np.full((P, 1), float(b2.reshape(-1)[0]), np.float32),
              "iotas": np.ascontiguousarray(np.concatenate(
                  [np.tile(np.arange(P, dtype=np.float32)[:, None], (1, TILE)),
                   np.tile(np.arange(P, 2 * P, dtype=np.float32)[:, None], (1, TILE))],
                  axis=1)),
              "ones1": np.ones((1, P), dtype=bf16)}

    core_ids = rows // upc
    Lw = nwin * wcap
    per_core = []
    spill_cnt = np.zeros((n_cores, 2), dtype=np.int64)
    for c in range(n_cores):
        m = core_ids == c
        eids = np.nonzero(m)[0]
        r, co = rows[eids], cols[eids]
        order = np.argsort(co, kind="stable")
        eids, r, co = eids[order], r[order], co[order]
        win = co // P
        wstart = np.searchsorted(win, np.arange(nwin))
        wend = np.searchsorted(win, np.arange(nwin), side="right")
        k = np.arange(len(co)) - wstart[win]
        in_window = k < wcap
        spill_bank = (co // BANK).astype(np.int64)
        for bk in range(2):
            spill_cnt[c, bk] = int(np.count_nonzero(~in_window & (spill_bank == bk)))
        per_core.append((eids, r, co, win, k, in_window, spill_bank))

    lsp_caps = [int(_roundup(max(int(spill_cnt[:, bk].max()), 1), TILE))
                for bk in range(2)]
    Lsp = sum(lsp_caps)
    L = Lw + Lsp

    # static group->w0 for collocal encoding
    slot_arr = np.arange(Lw)
    grp_w0 = (slot_arr // TILE * TILE) // wcap     # w0 of each slot's group

    in_maps, backmaps = [], []
    for c in range(n_cores):
        eids, r, co, win, k, in_window, spill_bank = per_core[c]
        iu = np.zeros(L, np.int16)
        clv = np.full(Lw, 512.0, np.float32)
        imsp = np.zeros(max(Lsp, 16), np.int16)
        slot = np.empty(len(eids), np.int64)
        # window slots
        mA = NAC // P
        def lin(u):
            return ((u % P) * mA + u // P).astype(np.int16)
        wi = np.nonzero(in_window)[0]
        ws = win[wi] * wcap + k[wi]
        slot[wi] = ws
        iu[ws] = lin(r[wi] - c * upc)
        clv[ws] = (co[wi] - grp_w0[ws] * P).astype(np.float32)
        # spill slots
        off = 0
        for bk in range(2):
            si = np.nonzero(~in_window & (spill_bank == bk))[0]
            ss = Lw + off + np.arange(len(si))
            slot[si] = ss
            iu[ss] = lin(r[si] - c * upc)
            imsp[ss - Lw] = (co[si] % BANK).astype(np.int16)
            off += lsp_caps[bk]
        zuT = np.zeros((P, NAC), dtype=bf16)
        ncr = min((c + 1) * upc, z_user.shape[0]) - c * upc
        zuT[:, :ncr] = z_user[c * upc:c * upc + ncr].T.astype(bf16)
        iu_w = np.ascontiguousarray(np.tile(iu.reshape(L // 16, 16).T, (8, 1)))
        im_w = np.ascontiguousarray(
            np.tile(imsp.reshape(len(imsp) // 16, 16).T, (8, 1)))
        in_maps.append({**shared, "zTuc": zuT,
                        "colloc": np.ascontiguousarray(clv[None, :]).astype(bf16),
                        "idxU": iu_w, "idxMsp": im_w})
        backmaps.append((eids, slot))
    return in_maps, dict(nwin=nwin, lsp_caps=lsp_caps, L=L, E=E, Hp=Hp,
                         backmaps=backmaps)


def _unpack_v4(res, meta):
    out = np.empty(meta["E"], dtype=np.float32)
    for c, (eids, slot) in enumerate(meta["backmaps"]):
        flat = np.asarray(res.results[c]["out"], dtype=np.float32).reshape(-1)
        tc_ = slot // P
        p = slot % P
        fidx = (tc_ // BCOLS) * (P * BCOLS) + p * BCOLS + (tc_ % BCOLS)
        out[eids] = flat[fidx]
    return out

